# revision 1
# baseline (speedup 1.0000x reference)
"""GAT (bipartite GATConv + mean-pool + 2 FC) on 8 Trainium2 NeuronCores.

Strategy: shard destination nodes across the 8 cores (each core owns N/8 dst
nodes and all edges pointing at them) so the segment softmax is fully local to
a core — no collectives needed.  Per core:

  Phase A: dense matmuls build a node table  row[n] = [h_s[n] (36) | a_s[n] (3)]
           in core-local DRAM (h_s = x_s @ W, a_s folded as x_s @ (W*att_src)),
           plus per-dst-node a_t values kept in SBUF.
  Phase B: dst nodes are processed in tiles of 128 (one node per partition,
           nodes degree-sorted so tiles have uniform run lengths).  Each node's
           incoming edges occupy L slots along its partition's free dimension;
           slot data arrives via indirect DMA row gathers from the table.
           The segment softmax (skipping the max-subtraction: logits are
           bounded, exp is safe in fp32) and the weighted message sum are
           strided DVE/ACT ops along each partition's run.  A one-hot matmul
           pools relu(out)·W2 and node counts into per-batch partials.

Host work is limited to index manipulation (edge sorting / padding / layout),
weight folding, and the final unsharding reduction of 8 x [128,2] partials.

Perf note (2026-08-09 session): the kernel is at the per-slot indirect-DMA
floor of this platform.  Each [P,1]-offset indirect gather costs ~1104ns of
SWDGE descriptor generation on the Pool engine plus ~309ns of fixed
inter-instruction overhead; with ~3160 edge-slots per core that is ~4.5ms,
which bounds the total.  Multi-index indirect DMA is broken in the walrus
lowering, and the batched-gather custom instructions (InstDMAGatherAnt /
InstIndirectCopy / ap_gather) are excluded from this image (BEDROCK=1) or
fail to compile, so no faster gather primitive is available.
"""

import numpy as np

import concourse.bacc as bacc
import concourse.tile as tile
from concourse import mybir
from concourse.bass import IndirectOffsetOnAxis
from concourse.bass_utils import run_bass_kernel_spmd

F32 = mybir.dt.float32
I32 = mybir.dt.int32

N_CORES = 8
P = 128
HEADS = 3
CH = 12
HC = HEADS * CH          # 36
ROW = HC + 4             # table row: 36 h | 3 a_s | 1 pad  (40 f32 = 160B)
PAD_A = -300.0           # pad-slot a_s value: exp(0.2*-300) = e^-60 ~ 0
NEG_SLOPE = 0.2

_nc_cache = {}


def _build_nc(in_dim, n_src_tiles, n_dst_tiles, L_list, slot_tot, n_xt_cols):
    """Build the SPMD Bass program (identical for all cores)."""
    key = (in_dim, n_src_tiles, n_dst_tiles, tuple(L_list), slot_tot, n_xt_cols)
    if key in _nc_cache:
        return _nc_cache[key]

    table_rows = n_src_tiles * P + 1          # +1 pad row
    pad_row_idx = n_src_tiles * P
    xs_cols = n_src_tiles * P
    half_tiles = n_src_tiles // 2             # n_src_tiles forced even by caller

    nc = bacc.Bacc("TRN2", target_bir_lowering=False, debug=False)
    d_xs = nc.dram_tensor("xs_t", [2 * in_dim, xs_cols // 2], F32, kind="ExternalInput")
    d_xt = nc.dram_tensor("xt_t", [in_dim, n_xt_cols], F32, kind="ExternalInput")
    d_idx = nc.dram_tensor("idxs", [P, slot_tot], I32, kind="ExternalInput")
    d_bc = nc.dram_tensor("bidcnt", [P, n_dst_tiles * 2], F32, kind="ExternalInput")
    d_wf = nc.dram_tensor("wfold", [in_dim, ROW], F32, kind="ExternalInput")
    d_wt = nc.dram_tensor("wat", [in_dim, 4], F32, kind="ExternalInput")
    d_w2 = nc.dram_tensor("w2b", [P, HC], F32, kind="ExternalInput")
    d_bb = nc.dram_tensor("biasb", [P, HC], F32, kind="ExternalInput")
    d_pr = nc.dram_tensor("padrow", [1, ROW], F32, kind="ExternalInput")
    d_q = nc.dram_tensor("q_out", [P, 2], F32, kind="ExternalOutput")

    with tile.TileContext(nc) as tc:
        with tc.tile_pool(name="const", bufs=1) as cpool, \
             tc.tile_pool(name="dram", bufs=1, space="DRAM") as dpool, \
             tc.tile_pool(name="xload", bufs=3) as xpool, \
             tc.tile_pool(name="tabout", bufs=4) as topool, \
             tc.tile_pool(name="gat", bufs=6) as gpool, \
             tc.tile_pool(name="work", bufs=3) as wpool, \
             tc.tile_pool(name="psA", bufs=4, space="PSUM") as psA, \
             tc.tile_pool(name="psB", bufs=2, space="PSUM") as psB, \
             tc.tile_pool(name="psT", bufs=2, space="PSUM") as psT:

            table = dpool.tile([table_rows, ROW], F32)

            # ---- constants into SBUF ----
            t_wf = cpool.tile([in_dim, ROW], F32)
            nc.sync.dma_start(t_wf[:], d_wf[:])
            t_wf2 = cpool.tile([2 * in_dim, ROW], F32)
            nc.sync.dma_start(t_wf2[0:in_dim, :], d_wf[:])
            nc.sync.dma_start(t_wf2[in_dim:2 * in_dim, :], d_wf[:])
            t_wt = cpool.tile([in_dim, 4], F32)
            nc.sync.dma_start(t_wt[:], d_wt[:])
            t_w2 = cpool.tile([P, HC], F32)
            nc.sync.dma_start(t_w2[:], d_w2[:])
            t_bb = cpool.tile([P, HC], F32)
            nc.sync.dma_start(t_bb[:], d_bb[:])
            t_pr = cpool.tile([1, ROW], F32)
            nc.sync.dma_start(t_pr[:], d_pr[:])
            t_idx = cpool.tile([P, slot_tot], I32)
            nc.sync.dma_start(t_idx[:], d_idx[:])
            t_bc = cpool.tile([P, n_dst_tiles * 2], F32)
            nc.sync.dma_start(t_bc[:], d_bc[:])
            t_xt = cpool.tile([in_dim, n_xt_cols], F32)
            nc.sync.dma_start(t_xt[:], d_xt[:])

            t_iota_i = cpool.tile([P, P], I32)
            nc.gpsimd.iota(t_iota_i[:], pattern=[[1, P]], base=0, channel_multiplier=0)
            t_iota = cpool.tile([P, P], F32)
            nc.vector.tensor_copy(t_iota[:], t_iota_i[:])

            t_qacc = cpool.tile([P, 2], F32)
            nc.vector.memset(t_qacc[:], 0.0)

            # ---- phase A2: a_t per dst tile -> resident SBUF ----
            t_at = cpool.tile([P, n_dst_tiles * 4], F32)
            for t in range(n_dst_tiles):
                ps = psT.tile([P, 4], F32, space="PSUM", tag="psat")
                nc.tensor.matmul(
                    ps[:], lhsT=t_xt[:, t * P:(t + 1) * P], rhs=t_wt[:],
                    start=True, stop=True)
                nc.scalar.copy(t_at[:, t * 4:(t + 1) * 4], ps[:])

            # ---- phase A: node table (h_s | a_s) ----
            # x packed [128, half]: partitions 0:64 = tiles [0, half), 64:128 =
            # tiles [half, 2*half). Two K=64 matmuls per slice in separate PE
            # row groups; 4 tiles batched per psum bank per half.
            XB = 8  # half-tiles per x-chunk load
            for blk in range(0, half_tiles, XB):
                nb = min(XB, half_tiles - blk)
                xs_sb = xpool.tile([2 * in_dim, XB * P], F32, tag="xs")
                nc.sync.dma_start(
                    xs_sb[:, : nb * P], d_xs[:, blk * P:(blk + nb) * P])
                for g in range(0, nb, 4):
                    ng = min(4, nb - g)
                    for hf in range(2):
                        ps = psA.tile([P, 4 * ROW], F32, space="PSUM", tag="psa")
                        for j in range(ng):
                            nc.tensor.matmul(
                                ps[:, j * ROW:(j + 1) * ROW],
                                lhsT=xs_sb[hf * in_dim:(hf + 1) * in_dim,
                                           (g + j) * P:(g + j + 1) * P],
                                rhs=t_wf2[hf * in_dim:(hf + 1) * in_dim, :],
                                start=True, stop=True)
                        ob = topool.tile([P, 4 * ROW], F32, tag="tab")
                        nc.vector.tensor_copy(ob[:, : ng * ROW], ps[:, : ng * ROW])
                        base = (hf * half_tiles + blk + g) * P
                        out_ap = table[:][base:base + ng * P, :]
                        out_ap = out_ap.rearrange("(j p) c -> p j c", p=P)
                        nc.scalar.dma_start(
                            out_ap,
                            ob[:, : ng * ROW].rearrange("p (j c) -> p j c", c=ROW))
            # pad row
            nc.scalar.dma_start(table[:][pad_row_idx:pad_row_idx + 1, :], t_pr[:])

            # ---- phase B ----
            off = 0
            for t in range(n_dst_tiles):
                L = L_list[t]
                g = gpool.tile([P, L * ROW], F32, tag="G")
                for s in range(L):
                    nc.gpsimd.indirect_dma_start(
                        out=g[:, s * ROW:(s + 1) * ROW],
                        out_offset=None,
                        in_=table[:],
                        in_offset=IndirectOffsetOnAxis(
                            ap=t_idx[:, off + s:off + s + 1], axis=0),
                    )
                off += L
                g3 = g[:].rearrange("p (l c) -> p l c", c=ROW)

                # logits l = a_s + a_t  (per head, a_t per-partition scalar)
                tT = wpool.tile([P, L * HEADS], F32, tag="T")
                T3 = tT[:].rearrange("p (l h) -> p l h", h=HEADS)
                for h in range(HEADS):
                    nc.vector.tensor_scalar_add(
                        T3[:, :, h], g3[:, :, HC + h], t_at[:, t * 4 + h:t * 4 + h + 1])
                # e = exp(leaky_relu(l))
                tE = wpool.tile([P, L * HEADS], F32, tag="E")
                nc.vector.tensor_scalar_mul(tE[:], tT[:], NEG_SLOPE)
                nc.vector.tensor_tensor(
                    out=tE[:], in0=tE[:], in1=tT[:], op=mybir.AluOpType.max)
                nc.scalar.activation(tE[:], tE[:], mybir.ActivationFunctionType.Exp)
                E3 = tE[:].rearrange("p (l h) -> p l h", h=HEADS)

                # denom + reciprocal
                t_den = wpool.tile([P, HEADS], F32, tag="den")
                nc.vector.tensor_reduce(
                    out=t_den[:], in_=E3.transpose([0, 2, 1]),
                    axis=mybir.AxisListType.X, op=mybir.AluOpType.add)
                nc.vector.tensor_scalar_max(t_den[:], t_den[:], 1e-30)
                t_rec = wpool.tile([P, HEADS], F32, tag="rec")
                nc.vector.reciprocal(t_rec[:], t_den[:])

                # weighted message sum U = sum_l e * h
                tM = wpool.tile([P, L * HC], F32, tag="M")
                M3 = tM[:].rearrange("p (l j) -> p l j", j=HC)
                e_b = E3.unsqueeze(3).to_broadcast((P, L, HEADS, CH))
                nc.vector.tensor_tensor(
                    out=M3[:], in0=g3[:, :, 0:HC], in1=e_b, op=mybir.AluOpType.mult)
                tU = wpool.tile([P, HC], F32, tag="U")
                nc.vector.tensor_reduce(
                    out=tU[:], in_=M3.transpose([0, 2, 1]),
                    axis=mybir.AxisListType.X, op=mybir.AluOpType.add)

                # out = relu(U / denom + bias)
                tV = wpool.tile([P, HC], F32, tag="V")
                rec_b = t_rec[:].unsqueeze(2).to_broadcast((P, HEADS, CH))
                nc.vector.tensor_tensor(
                    out=tV[:].rearrange("p (h c) -> p h c", c=CH),
                    in0=tU[:].rearrange("p (h c) -> p h c", c=CH),
                    in1=rec_b, op=mybir.AluOpType.mult)
                nc.vector.tensor_tensor(
                    out=tV[:], in0=tV[:], in1=t_bb[:], op=mybir.AluOpType.add)
                nc.scalar.activation(tV[:], tV[:], mybir.ActivationFunctionType.Relu)

                # rv = sum(V * W2); RV = [rv | cnt]
                tR = wpool.tile([P, HC], F32, tag="R")
                nc.vector.tensor_tensor(
                    out=tR[:], in0=tV[:], in1=t_w2[:], op=mybir.AluOpType.mult)
                tRV = wpool.tile([P, 2], F32, tag="RV")
                nc.vector.tensor_reduce(
                    out=tRV[:, 0:1], in_=tR[:], axis=mybir.AxisListType.X,
                    op=mybir.AluOpType.add)
                nc.scalar.copy(tRV[:, 1:2], t_bc[:, 2 * t + 1:2 * t + 2])

                # pool into batches: q += onehot(bid)^T @ RV
                t_oh = wpool.tile([P, P], F32, tag="oh")
                nc.vector.tensor_scalar(
                    out=t_oh[:], in0=t_iota[:], scalar1=t_bc[:, 2 * t:2 * t + 1],
                    scalar2=None, op0=mybir.AluOpType.is_equal)
                ps_q = psB.tile([P, 2], F32, space="PSUM", tag="q")
                nc.tensor.matmul(ps_q[:], lhsT=t_oh[:], rhs=tRV[:],
                                 start=True, stop=True)
                nc.vector.tensor_tensor(
                    out=t_qacc[:], in0=t_qacc[:], in1=ps_q[:],
                    op=mybir.AluOpType.add)

            nc.sync.dma_start(d_q[:], t_qacc[:])
    nc.finalize()
    _nc_cache[key] = nc
    return nc


def kernel(**inputs):
    x_s = np.asarray(inputs["x_s"], np.float32)
    x_t = np.asarray(inputs["x_t"], np.float32)
    edge_index = np.asarray(inputs["edge_index"])
    x_s_batch = np.asarray(inputs["x_s_batch"]).astype(np.int64)
    W = np.asarray(inputs["W"], np.float32)
    att_src = np.asarray(inputs["att_src"], np.float32)
    att_dst = np.asarray(inputs["att_dst"], np.float32)
    bias = np.asarray(inputs["bias"], np.float32)
    fc1_w = np.asarray(inputs["fc1_w"], np.float32)
    fc1_b = np.asarray(inputs["fc1_b"], np.float32)
    fc3_w = np.asarray(inputs["fc3_w"], np.float32)
    fc3_b = np.asarray(inputs["fc3_b"], np.float32)

    n_nodes, in_dim = x_s.shape
    src = edge_index[0].astype(np.int64)
    dst = edge_index[1].astype(np.int64)

    # ---- host: edge bucketing by destination (layout prep only) ----
    deg = np.bincount(dst, minlength=n_nodes)
    order = np.argsort(-deg, kind="stable")      # nodes by degree desc
    # round-robin deal over cores: core c gets ranks c, c+8, ...
    nodes_per_core = (n_nodes + N_CORES - 1) // N_CORES
    n_dst_tiles = (nodes_per_core + P - 1) // P
    # per-tile run length: max degree in the global rank band of the tile
    L_list = []
    for t in range(n_dst_tiles):
        r0 = t * P * N_CORES
        L_list.append(max(1, int(deg[order[min(r0, n_nodes - 1)]])))
    slot_tot = int(np.sum(L_list))

    n_src_tiles = (n_nodes + P - 1) // P
    if n_src_tiles % 2:
        n_src_tiles += 1
    pad_row_idx = n_src_tiles * P
    xs_cols = n_src_tiles * P
    n_xt_cols = n_dst_tiles * P

    # edges sorted by dst -> per-node contiguous src runs
    e_order = np.argsort(dst, kind="stable")
    src_sorted = src[e_order].astype(np.int32)
    starts = np.searchsorted(dst[e_order], np.arange(n_nodes))
    ends = np.searchsorted(dst[e_order], np.arange(n_nodes) + 1)

    xs_t = np.zeros((in_dim, xs_cols), np.float32)
    xs_t[:, :n_nodes] = x_s.T
    half_cols = xs_cols // 2
    xs_t = np.concatenate([xs_t[:, :half_cols], xs_t[:, half_cols:]], axis=0)
    xs_t = np.ascontiguousarray(xs_t)

    # fold weights (host weight prep)
    wa_t = np.einsum("khc,hc->kh", W.reshape(in_dim, HEADS, CH), att_dst).astype(np.float32)
    wa_s = np.einsum("khc,hc->kh", W.reshape(in_dim, HEADS, CH), att_src).astype(np.float32)
    wfold = np.zeros((in_dim, ROW), np.float32)
    wfold[:, :HC] = W
    wfold[:, HC:HC + HEADS] = wa_s
    wat = np.zeros((in_dim, 4), np.float32)
    wat[:, :HEADS] = wa_t
    w2 = (fc1_w @ fc3_w)[:, 0].astype(np.float32)      # [36]
    w2b = np.tile(w2[None, :], (P, 1))
    biasb = np.tile(bias[None, :], (P, 1))
    padrow = np.zeros((1, ROW), np.float32)
    padrow[0, HC:HC + HEADS] = PAD_A

    in_maps = []
    for c in range(N_CORES):
        node_ids = order[c::N_CORES]             # this core's dst nodes, deg-sorted
        ncnt = len(node_ids)
        idxs = np.full((P, slot_tot), pad_row_idx, np.int32)
        bidcnt = np.zeros((P, n_dst_tiles * 2), np.float32)
        xt_t = np.zeros((in_dim, n_xt_cols), np.float32)
        off = 0
        for t in range(n_dst_tiles):
            L = L_list[t]
            for i in range(P):
                k = t * P + i
                if k >= ncnt:
                    continue
                node = node_ids[k]
                s0, s1 = starts[node], ends[node]
                d = s1 - s0
                idxs[i, off:off + d] = src_sorted[s0:s1]
                bidcnt[i, 2 * t] = float(x_s_batch[node])
                bidcnt[i, 2 * t + 1] = 1.0
            off += L
        valid = min(ncnt, n_dst_tiles * P)
        xt_t[:, :valid] = x_t[node_ids[:valid]].T
        in_maps.append({
            "xs_t": xs_t, "xt_t": xt_t, "idxs": idxs, "bidcnt": bidcnt,
            "wfold": wfold, "wat": wat, "w2b": w2b, "biasb": biasb,
            "padrow": padrow,
        })

    nc = _build_nc(in_dim, n_src_tiles, n_dst_tiles, L_list, slot_tot, n_xt_cols)
    res = run_bass_kernel_spmd(nc, in_maps, core_ids=list(range(N_CORES)))

    q = np.zeros((P, 2), np.float64)
    for c in range(N_CORES):
        q += res.results[c]["q_out"]
    cnt = np.maximum(q[:, 1], 1.0)
    out = q[:, 0] / cnt
    const = float(fc1_b @ fc3_w[:, 0] + fc3_b[0])
    return (out + const).astype(np.float32)



# revision 11
# speedup vs baseline: 6.3005x; 6.3005x over previous
"""GAT (bipartite GATConv + mean-pool + 2 FC) on 8 Trainium2 NeuronCores.

Strategy: edges are sharded per destination node; destination nodes are
dealt round-robin (degree-sorted) across the 8 cores so the segment softmax
is fully local to a core.  Per the sharding hint each device holds its edge
shard with the source-node features replicated into matmul-ready per-edge
layout (host does only index manipulation / np.take layout; every model
FLOP runs on device):

  Phase A2: a_t = x_t @ (W att_dst) for this core's dst nodes (PE).
  Phase B: dst nodes are processed in tiles of 128 (one node per partition,
           nodes degree-sorted so tiles have uniform run lengths L).  The
           per-edge source features arrive as bf16 lhsT tiles [2*64, L/2*128]
           (two K=64 PE row groups); one matmul per edge-slot computes
           row[p, s] = [h_s (36) | a_s (3) | pad] directly into PSUM.
           The segment softmax (max-subtraction skipped: logits bounded,
           fp32-safe) and the weighted message sum are DVE/ACT ops along
           each partition's run.  Pad slots carry x=0 => h=0, a_s=0; their
           exp(LR(a_t)) contribution to the denominator is subtracted
           exactly via a host-provided pad count.  A one-hot matmul pools
           relu(out)·W2 and node counts into per-batch partials.

This replaces the previous indirect-DMA gather design: one [P,1]-offset
indirect gather costs ~1.1us of SWDGE descriptor generation on the Pool
engine (994ns fixed + 0.34ns/desc, 128 descriptors max per instruction)
and the batched-gather ucode (InstDMAGatherAnt etc.) is excluded from
bedrock images, so any device-side per-edge gather is floored at ~3.3ms.
Sequential streaming of the pre-laid-out edge shard runs at full DMA
bandwidth instead.
"""

import os

import numpy as np
import ml_dtypes

import concourse.bacc as bacc
import concourse.tile as tile
from concourse import mybir
from concourse.bass_utils import run_bass_kernel_spmd

F32 = mybir.dt.float32
BF16 = mybir.dt.bfloat16

N_CORES = 8
P = 128
HEADS = 3
CH = 12
HC = HEADS * CH          # 36
ROW = HC + 4             # matmul output row: 36 h | 3 a_s | 1 pad = 40
NEG_SLOPE = 0.2
GS = 16                  # slots per PSUM group pair (8*40 = 320 f32 per bank)

_nc_cache = {}


def _build_nc(in_dim, n_dst_tiles, L_list, half_tot, n_xt_cols):
    key = (in_dim, n_dst_tiles, tuple(L_list), half_tot, n_xt_cols)
    if key in _nc_cache:
        return _nc_cache[key]

    nc = bacc.Bacc("TRN2", target_bir_lowering=False, debug=False)
    d_xe = nc.dram_tensor("xe", [2 * in_dim, half_tot * P], BF16, kind="ExternalInput")
    d_xt = nc.dram_tensor("xt_t", [in_dim, n_xt_cols], F32, kind="ExternalInput")
    d_bc = nc.dram_tensor("bidcnt", [P, n_dst_tiles * 4], F32, kind="ExternalInput")
    d_wf = nc.dram_tensor("wfold2", [2 * in_dim, ROW], BF16, kind="ExternalInput")
    d_wt = nc.dram_tensor("wat", [in_dim, 4], F32, kind="ExternalInput")
    d_w2 = nc.dram_tensor("w2b", [P, HC], F32, kind="ExternalInput")
    d_bb = nc.dram_tensor("biasb", [P, HC], F32, kind="ExternalInput")
    d_io = nc.dram_tensor("iota", [P, P], F32, kind="ExternalInput")
    d_q = nc.dram_tensor("q_out", [P, 2], F32, kind="ExternalOutput")

    with tile.TileContext(nc) as tc:
        with tc.tile_pool(name="const", bufs=1) as cpool, \
             tc.tile_pool(name="xload", bufs=3) as xpool, \
             tc.tile_pool(name="gat", bufs=3) as gpool, \
             tc.tile_pool(name="work", bufs=3) as wpool, \
             tc.tile_pool(name="psA", bufs=2, space="PSUM") as psA, \
             tc.tile_pool(name="psB", bufs=2, space="PSUM") as psB, \
             tc.tile_pool(name="psT", bufs=2, space="PSUM") as psT:

            # ---- constants into SBUF ----
            t_wf2 = cpool.tile([2 * in_dim, ROW], BF16)
            nc.sync.dma_start(t_wf2[:], d_wf[:])
            t_wt = cpool.tile([in_dim, 4], F32)
            nc.sync.dma_start(t_wt[:], d_wt[:])
            t_w2 = cpool.tile([P, HC], F32)
            nc.sync.dma_start(t_w2[:], d_w2[:])
            t_bb = cpool.tile([P, HC], F32)
            nc.sync.dma_start(t_bb[:], d_bb[:])
            t_io = cpool.tile([P, P], F32)
            nc.sync.dma_start(t_io[:], d_io[:])
            t_bc = cpool.tile([P, n_dst_tiles * 4], F32)
            nc.sync.dma_start(t_bc[:], d_bc[:])
            t_xt = cpool.tile([in_dim, n_xt_cols], F32)
            nc.sync.dma_start(t_xt[:], d_xt[:])

            t_qacc = cpool.tile([P, 2], F32)
            nc.vector.memset(t_qacc[:], 0.0)

            # ---- phase A2: a_t per dst tile -> resident SBUF ----
            t_at = cpool.tile([P, n_dst_tiles * 4], F32)
            for t in range(n_dst_tiles):
                ps = psT.tile([P, 4], F32, space="PSUM", tag="psat")
                nc.tensor.matmul(
                    ps[:], lhsT=t_xt[:, t * P:(t + 1) * P], rhs=t_wt[:],
                    start=True, stop=True)
                nc.scalar.copy(t_at[:, t * 4:(t + 1) * 4], ps[:])

            # ep = exp(leaky_relu(a_t)) for the pad-slot denominator correction
            t_ep = cpool.tile([P, n_dst_tiles * 4], F32)
            nc.vector.tensor_scalar_mul(t_ep[:], t_at[:], NEG_SLOPE)
            nc.vector.tensor_tensor(
                out=t_ep[:], in0=t_ep[:], in1=t_at[:], op=mybir.AluOpType.max)
            nc.scalar.activation(t_ep[:], t_ep[:], mybir.ActivationFunctionType.Exp)

            # ---- phase B ----
            nb_tiles = int(os.environ.get("KB_TILES", n_dst_tiles))
            stage = int(os.environ.get("KB_STAGE", 4))
            off_h = 0
            for t in range(nb_tiles):
                L = L_list[t]          # even
                Lh = L // 2
                xe_sb = xpool.tile([2 * in_dim, Lh * P], BF16, tag="xe")
                nc.sync.dma_start(
                    xe_sb[:], d_xe[:, off_h * P:(off_h + Lh) * P])
                off_h += Lh

                # per-edge rows via PE.  Slot order within a partition is
                # irrelevant to the segment softmax, so slots are laid out
                # as [even-parity block | odd-parity block]: parity par uses
                # PE row group par, and consecutive same-parity matmuls land
                # in their own PSUM tile (switching row groups inside one
                # PSUM tile crashes the PE — probed on HW).
                g = gpool.tile([P, L * ROW], F32, tag="G")
                if stage < 2:
                    nc.vector.memset(g[:], 0.0)
                GH = GS // 2
                for h0 in range(0, Lh if stage >= 2 else 0, GH):
                    nh = min(GH, Lh - h0)
                    ps_e = psA.tile([P, GH * ROW], F32, space="PSUM", tag="pse")
                    ps_o = psA.tile([P, GH * ROW], F32, space="PSUM", tag="pso")
                    for par, pst in ((0, ps_e), (1, ps_o)):
                        for j in range(nh):
                            nc.tensor.matmul(
                                pst[:, j * ROW:(j + 1) * ROW],
                                lhsT=xe_sb[par * in_dim:(par + 1) * in_dim,
                                           (h0 + j) * P:(h0 + j + 1) * P],
                                rhs=t_wf2[par * in_dim:(par + 1) * in_dim, :],
                                start=True, stop=True)
                    nc.scalar.copy(
                        g[:, h0 * ROW:(h0 + nh) * ROW], ps_e[:, :nh * ROW])
                    nc.scalar.copy(
                        g[:, (Lh + h0) * ROW:(Lh + h0 + nh) * ROW],
                        ps_o[:, :nh * ROW])

                g3 = g[:].rearrange("p (l c) -> p l c", c=ROW)
                if stage < 3:
                    continue

                # logits l = a_s + a_t  (per head, a_t per-partition scalar)
                tT = wpool.tile([P, L * HEADS], F32, tag="T")
                T3 = tT[:].rearrange("p (l h) -> p l h", h=HEADS)
                for h in range(HEADS):
                    nc.vector.tensor_scalar_add(
                        T3[:, :, h], g3[:, :, HC + h],
                        t_at[:, t * 4 + h:t * 4 + h + 1])
                # e = exp(leaky_relu(l))
                tE = wpool.tile([P, L * HEADS], F32, tag="E")
                nc.vector.tensor_scalar_mul(tE[:], tT[:], NEG_SLOPE)
                nc.vector.tensor_tensor(
                    out=tE[:], in0=tE[:], in1=tT[:], op=mybir.AluOpType.max)
                nc.scalar.activation(tE[:], tE[:], mybir.ActivationFunctionType.Exp)
                E3 = tE[:].rearrange("p (l h) -> p l h", h=HEADS)

                # denom (pad slots contribute exp(LR(a_t)) each; subtract)
                t_den = wpool.tile([P, HEADS], F32, tag="den")
                nc.vector.tensor_reduce(
                    out=t_den[:], in_=E3.transpose([0, 2, 1]),
                    axis=mybir.AxisListType.X, op=mybir.AluOpType.add)
                t_cor = wpool.tile([P, HEADS], F32, tag="cor")
                nc.vector.tensor_scalar(
                    out=t_cor[:], in0=t_ep[:, t * 4:t * 4 + HEADS],
                    scalar1=t_bc[:, 4 * t + 2:4 * t + 3], scalar2=None,
                    op0=mybir.AluOpType.mult)
                nc.vector.tensor_tensor(
                    out=t_den[:], in0=t_den[:], in1=t_cor[:],
                    op=mybir.AluOpType.subtract)
                nc.vector.tensor_scalar_max(t_den[:], t_den[:], 1e-30)
                t_rec = wpool.tile([P, HEADS], F32, tag="rec")
                nc.vector.reciprocal(t_rec[:], t_den[:])
                if stage < 4:
                    continue

                # weighted message sum U = sum_l e * h
                tM = wpool.tile([P, L * HC], F32, tag="M")
                M3 = tM[:].rearrange("p (l j) -> p l j", j=HC)
                e_b = E3.unsqueeze(3).to_broadcast((P, L, HEADS, CH))
                nc.vector.tensor_tensor(
                    out=M3[:], in0=g3[:, :, 0:HC], in1=e_b,
                    op=mybir.AluOpType.mult)
                tU = wpool.tile([P, HC], F32, tag="U")
                nc.vector.tensor_reduce(
                    out=tU[:], in_=M3.transpose([0, 2, 1]),
                    axis=mybir.AxisListType.X, op=mybir.AluOpType.add)

                # out = relu(U / denom + bias)
                tV = wpool.tile([P, HC], F32, tag="V")
                rec_b = t_rec[:].unsqueeze(2).to_broadcast((P, HEADS, CH))
                nc.vector.tensor_tensor(
                    out=tV[:].rearrange("p (h c) -> p h c", c=CH),
                    in0=tU[:].rearrange("p (h c) -> p h c", c=CH),
                    in1=rec_b, op=mybir.AluOpType.mult)
                nc.vector.tensor_tensor(
                    out=tV[:], in0=tV[:], in1=t_bb[:], op=mybir.AluOpType.add)
                nc.scalar.activation(tV[:], tV[:], mybir.ActivationFunctionType.Relu)

                # rv = sum(V * W2); RV = [rv | cnt]
                tR = wpool.tile([P, HC], F32, tag="R")
                nc.vector.tensor_tensor(
                    out=tR[:], in0=tV[:], in1=t_w2[:], op=mybir.AluOpType.mult)
                tRV = wpool.tile([P, 2], F32, tag="RV")
                nc.vector.tensor_reduce(
                    out=tRV[:, 0:1], in_=tR[:], axis=mybir.AxisListType.X,
                    op=mybir.AluOpType.add)
                nc.scalar.copy(tRV[:, 1:2], t_bc[:, 4 * t + 1:4 * t + 2])

                # pool into batches: q += onehot(bid)^T @ RV
                t_oh = wpool.tile([P, P], F32, tag="oh")
                nc.vector.tensor_scalar(
                    out=t_oh[:], in0=t_io[:], scalar1=t_bc[:, 4 * t:4 * t + 1],
                    scalar2=None, op0=mybir.AluOpType.is_equal)
                ps_q = psB.tile([P, 2], F32, space="PSUM", tag="q")
                nc.tensor.matmul(ps_q[:], lhsT=t_oh[:], rhs=tRV[:],
                                 start=True, stop=True)
                nc.vector.tensor_tensor(
                    out=t_qacc[:], in0=t_qacc[:], in1=ps_q[:],
                    op=mybir.AluOpType.add)

            nc.sync.dma_start(d_q[:], t_qacc[:])
    nc.finalize()
    _nc_cache[key] = nc
    return nc


def kernel(**inputs):
    x_s = np.asarray(inputs["x_s"], np.float32)
    x_t = np.asarray(inputs["x_t"], np.float32)
    edge_index = np.asarray(inputs["edge_index"])
    x_s_batch = np.asarray(inputs["x_s_batch"]).astype(np.int64)
    W = np.asarray(inputs["W"], np.float32)
    att_src = np.asarray(inputs["att_src"], np.float32)
    att_dst = np.asarray(inputs["att_dst"], np.float32)
    bias = np.asarray(inputs["bias"], np.float32)
    fc1_w = np.asarray(inputs["fc1_w"], np.float32)
    fc1_b = np.asarray(inputs["fc1_b"], np.float32)
    fc3_w = np.asarray(inputs["fc3_w"], np.float32)
    fc3_b = np.asarray(inputs["fc3_b"], np.float32)

    n_nodes, in_dim = x_s.shape
    src = edge_index[0].astype(np.int64)
    dst = edge_index[1].astype(np.int64)

    # ---- host: edge bucketing by destination (index/layout prep only) ----
    deg = np.bincount(dst, minlength=n_nodes)
    order = np.argsort(-deg, kind="stable")          # nodes by degree desc
    inv_order = np.empty(n_nodes, np.int64)
    inv_order[order] = np.arange(n_nodes)
    nodes_per_core = (n_nodes + N_CORES - 1) // N_CORES
    n_dst_tiles = (nodes_per_core + P - 1) // P
    L_list = []
    for t in range(n_dst_tiles):
        r0 = t * P * N_CORES
        L = max(2, int(deg[order[min(r0, n_nodes - 1)]]))
        L_list.append(L + (L % 2))                   # even
    off_arr = np.concatenate([[0], np.cumsum(L_list)]).astype(np.int64)
    slot_tot = int(off_arr[-1])
    half_tot = slot_tot // 2
    n_xt_cols = n_dst_tiles * P

    # edges sorted by dst -> per-node contiguous src runs
    e_order = np.argsort(dst, kind="stable")
    dst_sorted = dst[e_order]
    src_sorted = src[e_order].astype(np.int64)
    starts = np.searchsorted(dst_sorted, np.arange(n_nodes))
    slot_within = np.arange(len(dst_sorted)) - starts[dst_sorted]

    k_global = inv_order[dst_sorted]
    core_of = (k_global % N_CORES).astype(np.int64)
    k_local = k_global // N_CORES
    t_of = k_local // P
    p_of = k_local % P
    col_of = off_arr[t_of] + slot_within

    # fold weights (host weight prep)
    wa_t = np.einsum("khc,hc->kh", W.reshape(in_dim, HEADS, CH), att_dst)
    wa_s = np.einsum("khc,hc->kh", W.reshape(in_dim, HEADS, CH), att_src)
    wfold = np.zeros((in_dim, ROW), np.float32)
    wfold[:, :HC] = W
    wfold[:, HC:HC + HEADS] = wa_s
    wfold2 = np.concatenate([wfold, wfold], axis=0).astype(ml_dtypes.bfloat16)
    wat = np.zeros((in_dim, 4), np.float32)
    wat[:, :HEADS] = wa_t
    w2 = (fc1_w @ fc3_w)[:, 0].astype(np.float32)    # [36]
    w2b = np.tile(w2[None, :], (P, 1))
    biasb = np.tile(bias[None, :], (P, 1)).astype(np.float32)
    iota = np.tile(np.arange(P, dtype=np.float32)[None, :], (P, 1))

    xsb_ext = np.zeros((n_nodes + 1, in_dim), ml_dtypes.bfloat16)
    xsb_ext[:n_nodes] = x_s.astype(ml_dtypes.bfloat16)
    SENT = n_nodes

    in_maps = []
    for c in range(N_CORES):
        node_ids = order[c::N_CORES]                 # this core's dst nodes
        ncnt = len(node_ids)
        m = core_of == c
        SRC = np.full((P, slot_tot), SENT, np.int64)
        SRC[p_of[m], col_of[m]] = src_sorted[m]

        # per-edge lhsT layout: rows 0:64 even slots, 64:128 odd slots
        xe = np.empty((2 * in_dim, half_tot * P), ml_dtypes.bfloat16)
        for par in range(2):
            S = SRC[:, par::2]                       # [P, half_tot]
            blk = xsb_ext[S]                         # [P, half_tot, in_dim]
            xe[par * in_dim:(par + 1) * in_dim] = (
                blk.transpose(2, 1, 0).reshape(in_dim, half_tot * P))

        bidcnt = np.zeros((P, n_dst_tiles * 4), np.float32)
        xt_t = np.zeros((in_dim, n_xt_cols), np.float32)
        kk = np.arange(n_dst_tiles * P)
        tt, pp = kk // P, kk % P
        present = kk < ncnt
        nid = np.where(present, node_ids[np.minimum(kk, ncnt - 1)], 0)
        Leff = np.asarray(L_list, np.float32)[tt]
        bidcnt[pp, 4 * tt] = np.where(present, x_s_batch[nid], 0.0)
        bidcnt[pp, 4 * tt + 1] = present.astype(np.float32)
        bidcnt[pp, 4 * tt + 2] = np.where(present, Leff - deg[nid], Leff)
        xt_t[:, :ncnt] = x_t[node_ids].T
        in_maps.append({
            "xe": xe, "xt_t": xt_t, "bidcnt": bidcnt, "wfold2": wfold2,
            "wat": wat, "w2b": w2b, "biasb": biasb, "iota": iota,
        })

    nc = _build_nc(in_dim, n_dst_tiles, L_list, half_tot, n_xt_cols)
    res = run_bass_kernel_spmd(nc, in_maps, core_ids=list(range(N_CORES)))

    q = np.zeros((P, 2), np.float64)
    for c in range(N_CORES):
        q += res.results[c]["q_out"]
    cnt = np.maximum(q[:, 1], 1.0)
    out = q[:, 0] / cnt
    const = float(fc1_b @ fc3_w[:, 0] + fc3_b[0])
    return (out + const).astype(np.float32)


# revision 20
# speedup vs baseline: 11.2476x; 1.7852x over previous
"""GAT (bipartite GATConv + mean-pool + 2 FC) on 8 Trainium2 NeuronCores.

Strategy: edges are sharded per destination node; destination nodes are
dealt round-robin (degree-sorted) across the 8 cores so the segment softmax
is fully local to a core.  Per the sharding hint each device holds its edge
shard with the source-node features replicated into matmul-ready per-edge
layout (host does only index manipulation / np.take layout; every model
FLOP runs on device):

  Phase A2: a_t = x_t @ (W att_dst) for this core's dst nodes (PE).
  Phase B: dst nodes are processed in tiles of 128 (one node per partition,
           nodes degree-sorted so tiles have uniform run lengths L).  The
           per-edge source features arrive as bf16 lhsT tiles [128, L/2*128]
           (slot pair 2j/2j+1 stacked as two K=64 halves); one K=128 matmul
           against a block-diagonal [wfold|0 / 0|wfold] rhs computes BOTH
           slots' rows [h_s (36, c-major) | a_s (3) | pad] into PSUM.  ACT
           casts them to bf16 SBUF; E = exp(leaky_relu(a_s+a_t)) =
           max(exp(z), exp(0.2 z)) via two ACT Exp passes over a DVE-added
           z.  The weighted message sum runs in bf16 on DVE's packed 2x
           path: W columns are stored c-major/h-minor so the E broadcast's
           innermost dim is the packed head dim, and the slot reduction is
           two packed tree-add levels plus a short tensor_reduce tail.
           Batch pooling is a PE matmul against host-shipped one-hot
           columns, accumulated over all tiles in PSUM; the final fc1@fc3
           contraction and count division happen on host partials.
           Pad slots carry x=0 => h=0, a_s=0; their exp(leaky_relu(a_t))
           denominator contribution is subtracted exactly via a
           host-precomputed pad-count correction.

Device-side per-edge gathers are avoided entirely: one [P,1]-offset
indirect-DMA gather costs ~1.1us of SWDGE descriptor generation on the Pool
engine (994ns fixed + 0.34ns/desc, 128 descriptors max per instruction) and
the batched-gather ucode (InstDMAGatherAnt etc.) is excluded from bedrock
images, so any gather-based design is floored at ~3.3ms/core.  Sequential
streaming of the pre-laid-out edge shard runs at DMA bandwidth instead.

HW pitfalls (probed): matmuls that switch PE row groups (partition-offset
lhsT/rhs) within one PSUM tile crash the device (the K=128 block-diagonal
formulation sidesteps row groups); Pool-engine TensorTensor is rejected by
this lowering; softmax denominators accumulated from bf16 exps lose ~6x
final accuracy (E stays f32; bf16 is cast only for the message multiply).
"""

import numpy as np
import ml_dtypes

import concourse.bacc as bacc
import concourse.tile as tile
from concourse import mybir
from concourse.bass_utils import run_bass_kernel_spmd

F32 = mybir.dt.float32
BF16 = mybir.dt.bfloat16

N_CORES = 8
P = 128
HEADS = 3
CH = 12
HC = HEADS * CH          # 36
ROW = HC + 4             # matmul output row: 36 h | 3 a_s | 1 pad = 40
ROW2 = 2 * ROW           # block-diagonal pair output
NEG_SLOPE = 0.2
GP = 6                   # slot pairs per PSUM tile (6*80 = 480 f32 <= 512)

_nc_cache = {}


def _build_nc(in_dim, n_dst_tiles, L_list, half_tot, n_xt_cols):
    key = (in_dim, n_dst_tiles, tuple(L_list), half_tot, n_xt_cols)
    if key in _nc_cache:
        return _nc_cache[key]

    nc = bacc.Bacc("TRN2", target_bir_lowering=False, debug=False)
    d_xe = nc.dram_tensor("xe", [2 * in_dim, half_tot * P], BF16, kind="ExternalInput")
    d_xt = nc.dram_tensor("xt_t", [in_dim, n_xt_cols], F32, kind="ExternalInput")
    d_pc = nc.dram_tensor("padc4", [P, n_dst_tiles * 4], F32, kind="ExternalInput")
    d_oh = nc.dram_tensor("oh", [P, n_dst_tiles * P], F32, kind="ExternalInput")
    d_wf = nc.dram_tensor("wfbd", [2 * in_dim, ROW2], BF16, kind="ExternalInput")
    d_wt = nc.dram_tensor("wat", [in_dim, 4], F32, kind="ExternalInput")
    d_bb = nc.dram_tensor("biasb", [P, HC], F32, kind="ExternalInput")
    d_q = nc.dram_tensor("q_out", [P, HC], F32, kind="ExternalOutput")

    with tile.TileContext(nc) as tc:
        with tc.tile_pool(name="const", bufs=1) as cpool, \
             tc.tile_pool(name="xload", bufs=3) as xpool, \
             tc.tile_pool(name="gat", bufs=3) as gpool, \
             tc.tile_pool(name="work", bufs=3) as wpool, \
             tc.tile_pool(name="psA", bufs=3, space="PSUM") as psA, \
             tc.tile_pool(name="psB", bufs=1, space="PSUM") as psB, \
             tc.tile_pool(name="psT", bufs=2, space="PSUM") as psT:

            # ---- constants into SBUF ----
            t_wf = cpool.tile([2 * in_dim, ROW2], BF16)
            nc.sync.dma_start(t_wf[:], d_wf[:])
            t_wt = cpool.tile([in_dim, 4], F32)
            nc.sync.dma_start(t_wt[:], d_wt[:])
            t_bb = cpool.tile([P, HC], F32)
            nc.sync.dma_start(t_bb[:], d_bb[:])
            t_pc = cpool.tile([P, n_dst_tiles * 4], F32)
            nc.sync.dma_start(t_pc[:], d_pc[:])
            t_oh = cpool.tile([P, n_dst_tiles * P], F32)
            nc.sync.dma_start(t_oh[:], d_oh[:])
            t_xt = cpool.tile([in_dim, n_xt_cols], F32)
            nc.sync.dma_start(t_xt[:], d_xt[:])

            # ---- phase A2: a_t per dst tile -> resident SBUF ----
            t_at = cpool.tile([P, n_dst_tiles * 4], F32)
            for t0 in range(0, n_dst_tiles, 4):
                nt = min(4, n_dst_tiles - t0)
                ps = psT.tile([P, 16], F32, space="PSUM", tag="psat")
                for j in range(nt):
                    nc.tensor.matmul(
                        ps[:, j * 4:(j + 1) * 4],
                        lhsT=t_xt[:, (t0 + j) * P:(t0 + j + 1) * P], rhs=t_wt[:],
                        start=True, stop=True)
                nc.scalar.copy(
                    t_at[:, t0 * 4:(t0 + nt) * 4], ps[:, :nt * 4])

            # exp(leaky_relu(z)) = max(exp(z), exp(0.2 z)) — exp monotonic.
            # Pad-slot denominator correction for all tiles at once:
            # corall = padc * max(exp(a_t), exp(0.2 a_t)).
            t_at2 = cpool.tile([P, n_dst_tiles * 4], F32)
            nc.vector.tensor_scalar_mul(t_at2[:], t_at[:], NEG_SLOPE)
            t_ep = cpool.tile([P, n_dst_tiles * 4], F32)
            t_ep2 = cpool.tile([P, n_dst_tiles * 4], F32)
            nc.scalar.activation(
                t_ep[:], t_at[:], mybir.ActivationFunctionType.Exp)
            nc.scalar.activation(
                t_ep2[:], t_at2[:], mybir.ActivationFunctionType.Exp)
            nc.vector.tensor_tensor(
                out=t_ep[:], in0=t_ep[:], in1=t_ep2[:], op=mybir.AluOpType.max)
            t_cor = cpool.tile([P, n_dst_tiles * 4], F32)
            nc.vector.tensor_tensor(
                out=t_cor[:], in0=t_ep[:], in1=t_pc[:], op=mybir.AluOpType.mult)

            # ---- phase B ----
            ps_q = psB.tile([P, HC], F32, space="PSUM", tag="q")
            off_h = 0
            for t in range(n_dst_tiles):
                L = L_list[t]          # multiple of 4
                Lh = L // 2
                xe_sb = xpool.tile([2 * in_dim, Lh * P], BF16, tag="xe")
                nc.sync.dma_start(
                    xe_sb[:], d_xe[:, off_h * P:(off_h + Lh) * P])
                off_h += Lh

                # per-edge rows via PE: one K=128 matmul per slot PAIR
                g = gpool.tile([P, L * ROW], BF16, tag="G")
                for h0 in range(0, Lh, GP):
                    nh = min(GP, Lh - h0)
                    ps = psA.tile([P, GP * ROW2], F32, space="PSUM", tag="psa")
                    for j in range(nh):
                        nc.tensor.matmul(
                            ps[:, j * ROW2:(j + 1) * ROW2],
                            lhsT=xe_sb[:, (h0 + j) * P:(h0 + j + 1) * P],
                            rhs=t_wf[:],
                            start=True, stop=True)
                    nc.scalar.copy(
                        g[:, h0 * ROW2:(h0 + nh) * ROW2], ps[:, :nh * ROW2])

                g3 = g[:].rearrange("p (l c) -> p l c", c=ROW)

                # z = a_s + a_t  (layout (l, h), broadcast a_t over slots)
                tZ = wpool.tile([P, L * HEADS], F32, tag="Z")
                Z3 = tZ[:].rearrange("p (l h) -> p l h", h=HEADS)
                at_b = (t_at[:, t * 4:t * 4 + HEADS].unsqueeze(1)
                        .to_broadcast((P, L, HEADS)))
                nc.vector.tensor_tensor(
                    out=Z3[:], in0=g3[:, :, HC:HC + HEADS], in1=at_b,
                    op=mybir.AluOpType.add)
                tE = wpool.tile([P, L * HEADS], F32, tag="E")
                tT = wpool.tile([P, L * HEADS], F32, tag="T")
                nc.scalar.activation(
                    tT[:], tZ[:], mybir.ActivationFunctionType.Exp)
                nc.scalar.activation(
                    tE[:], tZ[:], mybir.ActivationFunctionType.Exp,
                    scale=NEG_SLOPE)
                nc.vector.tensor_tensor(
                    out=tE[:], in0=tE[:], in1=tT[:], op=mybir.AluOpType.max)
                tEb = wpool.tile([P, L * HEADS], BF16, tag="Eb")
                nc.scalar.copy(tEb[:], tE[:])

                # denominator + pad correction + reciprocal
                t_den = wpool.tile([P, 4], F32, tag="den")
                nc.vector.tensor_reduce(
                    out=t_den[:, :HEADS],
                    in_=tE[:].rearrange("p (l h) -> p l h", h=HEADS)
                    .transpose([0, 2, 1]),
                    axis=mybir.AxisListType.X, op=mybir.AluOpType.add)
                nc.vector.tensor_tensor(
                    out=t_den[:, :HEADS], in0=t_den[:, :HEADS],
                    in1=t_cor[:, t * 4:t * 4 + HEADS],
                    op=mybir.AluOpType.subtract)
                nc.vector.tensor_scalar_max(
                    t_den[:, :HEADS], t_den[:, :HEADS], 1e-30)
                t_rec = wpool.tile([P, HEADS], F32, tag="rec")
                nc.vector.reciprocal(t_rec[:], t_den[:, :HEADS])

                # weighted messages M = e * h (bf16 2x: h block is c-major,
                # so the E broadcast's innermost dim is the packed head dim)
                tM = wpool.tile([P, L * HC], BF16, tag="M")
                M4 = tM[:].rearrange("p (l c h) -> p l c h", c=CH, h=HEADS)
                e_b = (tEb[:].rearrange("p (l h) -> p l h", h=HEADS)
                       .unsqueeze(2).to_broadcast((P, L, CH, HEADS)))
                g4 = g3[:, :, 0:HC].rearrange("p l (c h) -> p l c h", h=HEADS)
                nc.vector.tensor_tensor(
                    out=M4[:], in0=g4, in1=e_b, op=mybir.AluOpType.mult)

                # U = sum_l M: two packed tree-add levels + reduce tail
                Mv = tM[:]
                n = L // 2
                a = Mv[:, :2 * n * HC].rearrange(
                    "p (n two c) -> p n two c", two=2, c=HC)
                nc.vector.tensor_tensor(
                    out=Mv[:, :n * HC].rearrange("p (n c) -> p n c", c=HC),
                    in0=a[:, :, 0, :], in1=a[:, :, 1, :],
                    op=mybir.AluOpType.add)
                n = L // 4
                a = Mv[:, :2 * n * HC].rearrange(
                    "p (n two c) -> p n two c", two=2, c=HC)
                nc.vector.tensor_tensor(
                    out=Mv[:, :n * HC].rearrange("p (n c) -> p n c", c=HC),
                    in0=a[:, :, 0, :], in1=a[:, :, 1, :],
                    op=mybir.AluOpType.add)
                tU = wpool.tile([P, HC], F32, tag="U")
                nc.vector.tensor_reduce(
                    out=tU[:],
                    in_=Mv[:, :n * HC].rearrange("p (n c) -> p n c", c=HC)
                    .transpose([0, 2, 1]),
                    axis=mybir.AxisListType.X, op=mybir.AluOpType.add)

                # V = relu(U / denom + bias); relu on ACT
                tV = wpool.tile([P, HC], F32, tag="V")
                rec_b = (t_rec[:].unsqueeze(1)
                         .to_broadcast((P, CH, HEADS)))
                nc.vector.tensor_tensor(
                    out=tV[:].rearrange("p (c h) -> p c h", h=HEADS),
                    in0=tU[:].rearrange("p (c h) -> p c h", h=HEADS),
                    in1=rec_b, op=mybir.AluOpType.mult)
                nc.vector.tensor_tensor(
                    out=tV[:], in0=tV[:], in1=t_bb[:], op=mybir.AluOpType.add)
                nc.scalar.activation(
                    tV[:], tV[:], mybir.ActivationFunctionType.Relu)

                # pool into batches: q[b, 36] += onehot_t^T @ V, PSUM-accum
                nc.tensor.matmul(
                    ps_q[:], lhsT=t_oh[:, t * P:(t + 1) * P], rhs=tV[:],
                    start=(t == 0), stop=(t == n_dst_tiles - 1))

            t_q = cpool.tile([P, HC], F32)
            nc.vector.tensor_copy(t_q[:], ps_q[:])
            nc.sync.dma_start(d_q[:], t_q[:])
    nc.finalize()
    _nc_cache[key] = nc
    return nc


def kernel(**inputs):
    x_s = np.asarray(inputs["x_s"], np.float32)
    x_t = np.asarray(inputs["x_t"], np.float32)
    edge_index = np.asarray(inputs["edge_index"])
    x_s_batch = np.asarray(inputs["x_s_batch"]).astype(np.int64)
    W = np.asarray(inputs["W"], np.float32)
    att_src = np.asarray(inputs["att_src"], np.float32)
    att_dst = np.asarray(inputs["att_dst"], np.float32)
    bias = np.asarray(inputs["bias"], np.float32)
    fc1_w = np.asarray(inputs["fc1_w"], np.float32)
    fc1_b = np.asarray(inputs["fc1_b"], np.float32)
    fc3_w = np.asarray(inputs["fc3_w"], np.float32)
    fc3_b = np.asarray(inputs["fc3_b"], np.float32)

    n_nodes, in_dim = x_s.shape
    src = edge_index[0].astype(np.int64)
    dst = edge_index[1].astype(np.int64)

    # ---- host: edge bucketing by destination (index/layout prep only) ----
    deg = np.bincount(dst, minlength=n_nodes)
    order = np.argsort(-deg, kind="stable")          # nodes by degree desc
    inv_order = np.empty(n_nodes, np.int64)
    inv_order[order] = np.arange(n_nodes)
    nodes_per_core = (n_nodes + N_CORES - 1) // N_CORES
    n_dst_tiles = (nodes_per_core + P - 1) // P
    L_list = []
    for t in range(n_dst_tiles):
        r0 = t * P * N_CORES
        L = max(4, int(deg[order[min(r0, n_nodes - 1)]]))
        L_list.append((L + 3) // 4 * 4)              # multiple of 4
    off_arr = np.concatenate([[0], np.cumsum(L_list)]).astype(np.int64)
    slot_tot = int(off_arr[-1])
    half_tot = slot_tot // 2
    n_xt_cols = n_dst_tiles * P

    # edges sorted by dst -> per-node contiguous src runs
    e_order = np.argsort(dst, kind="stable")
    dst_sorted = dst[e_order]
    src_sorted = src[e_order].astype(np.int64)
    starts = np.searchsorted(dst_sorted, np.arange(n_nodes))
    slot_within = np.arange(len(dst_sorted)) - starts[dst_sorted]

    k_global = inv_order[dst_sorted]
    core_of = (k_global % N_CORES).astype(np.int64)
    k_local = k_global // N_CORES
    t_of = k_local // P
    p_of = k_local % P
    col_of = off_arr[t_of] + slot_within

    # fold weights (host weight prep).  W/bias/w2 columns permuted c-major:
    # folded col (c*HEADS + h) <- original col (h*CH + c).
    cm = np.array([h * CH + c for c in range(CH) for h in range(HEADS)])
    W_cm = W[:, cm]
    bias_cm = bias[cm]
    w2_cm = (fc1_w @ fc3_w)[:, 0].astype(np.float32)[cm]

    wa_t = np.einsum("khc,hc->kh", W.reshape(in_dim, HEADS, CH), att_dst)
    wa_s = np.einsum("khc,hc->kh", W.reshape(in_dim, HEADS, CH), att_src)
    wfold = np.zeros((in_dim, ROW), np.float32)
    wfold[:, :HC] = W_cm
    wfold[:, HC:HC + HEADS] = wa_s
    wfbd = np.zeros((2 * in_dim, ROW2), np.float32)
    wfbd[:in_dim, :ROW] = wfold
    wfbd[in_dim:, ROW:] = wfold
    wfbd = wfbd.astype(ml_dtypes.bfloat16)
    wat = np.zeros((in_dim, 4), np.float32)
    wat[:, :HEADS] = wa_t
    biasb = np.tile(bias_cm[None, :], (P, 1)).astype(np.float32)

    xsb_ext = np.zeros((n_nodes + 1, in_dim), ml_dtypes.bfloat16)
    xsb_ext[:n_nodes] = x_s.astype(ml_dtypes.bfloat16)
    SENT = n_nodes

    in_maps = []
    cnts = []
    for c in range(N_CORES):
        node_ids = order[c::N_CORES]                 # this core's dst nodes
        ncnt = len(node_ids)
        m = core_of == c
        SRC = np.full((P, slot_tot), SENT, np.int64)
        SRC[p_of[m], col_of[m]] = src_sorted[m]

        # per-edge lhsT layout: rows 0:64 even slots, 64:128 odd slots
        xe = np.empty((2 * in_dim, half_tot * P), ml_dtypes.bfloat16)
        for par in range(2):
            S = SRC[:, par::2]                       # [P, half_tot]
            blk = xsb_ext[S]                         # [P, half_tot, in_dim]
            xe[par * in_dim:(par + 1) * in_dim] = (
                blk.transpose(2, 1, 0).reshape(in_dim, half_tot * P))

        padc4 = np.zeros((P, n_dst_tiles * 4), np.float32)
        oh = np.zeros((P, n_dst_tiles * P), np.float32)
        xt_t = np.zeros((in_dim, n_xt_cols), np.float32)
        kk = np.arange(n_dst_tiles * P)
        tt, pp = kk // P, kk % P
        present = kk < ncnt
        nid = np.where(present, node_ids[np.minimum(kk, ncnt - 1)], 0)
        Leff = np.asarray(L_list, np.float32)[tt]
        pc = np.where(present, Leff - deg[nid], Leff)
        for j in range(4):
            padc4[pp, 4 * tt + j] = pc
        bid = x_s_batch[nid]
        oh[pp[present], tt[present] * P + bid[present]] = 1.0
        cnts.append(np.bincount(bid[present], minlength=P).astype(np.float64))
        xt_t[:, :ncnt] = x_t[node_ids].T
        in_maps.append({
            "xe": xe, "xt_t": xt_t, "padc4": padc4, "oh": oh, "wfbd": wfbd,
            "wat": wat, "biasb": biasb,
        })

    nc = _build_nc(in_dim, n_dst_tiles, L_list, half_tot, n_xt_cols)
    res = run_bass_kernel_spmd(nc, in_maps, core_ids=list(range(N_CORES)))

    q = np.zeros((P, HC), np.float64)
    cnt = np.zeros(P, np.float64)
    for c in range(N_CORES):
        q += res.results[c]["q_out"]
        cnt += cnts[c]
    num = q @ w2_cm.astype(np.float64)
    out = num / np.maximum(cnt, 1.0)
    const = float(fc1_b @ fc3_w[:, 0] + fc3_b[0])
    return (out + const).astype(np.float32)


# revision 29
# speedup vs baseline: 12.5464x; 1.1155x over previous
"""GAT (bipartite GATConv + mean-pool + 2 FC) on 8 Trainium2 NeuronCores.

Strategy: edges are sharded per destination node; destination nodes are
dealt round-robin (degree-sorted) across the 8 cores so the segment softmax
is fully local to a core.  Per the sharding hint each device holds its edge
shard with the source-node features replicated into matmul-ready per-edge
layout (host does only index manipulation / np.take layout; every model
FLOP runs on device):

  Phase A2: a_t = x_t @ (W att_dst) for this core's dst nodes (PE).
  Phase B: dst nodes are processed in tiles of 128 (one node per partition,
           nodes degree-sorted so tiles have uniform run lengths L).  The
           per-edge source features arrive as bf16 lhsT tiles [128, L/2*128]
           (slot pair 2j/2j+1 stacked as two K=64 halves); one K=128 matmul
           against a block-diagonal [wfold|0 / 0|wfold] rhs computes BOTH
           slots' rows [h_s (36, c-major) | a_s (3) | pad] into PSUM.  ACT
           casts them to bf16 SBUF; E = exp(leaky_relu(a_s+a_t)) =
           max(exp(z), exp(0.2 z)) via two ACT Exp passes over a DVE-added
           z.  The weighted message sum runs in bf16 on DVE's packed 2x
           path: W columns are stored c-major/h-minor so the E broadcast's
           innermost dim is the packed head dim, and the slot reduction is
           two packed tree-add levels plus a short tensor_reduce tail.
           Batch pooling is a PE matmul against host-shipped one-hot
           columns, accumulated over all tiles in PSUM; the final fc1@fc3
           contraction and count division happen on host partials.
           Pad slots carry x=0 => h=0, a_s=0; their exp(leaky_relu(a_t))
           denominator contribution is subtracted exactly via a
           host-precomputed pad-count correction.

Device-side per-edge gathers are avoided entirely: one [P,1]-offset
indirect-DMA gather costs ~1.1us of SWDGE descriptor generation on the Pool
engine (994ns fixed + 0.34ns/desc, 128 descriptors max per instruction) and
the batched-gather ucode (InstDMAGatherAnt etc.) is excluded from bedrock
images, so any gather-based design is floored at ~3.3ms/core.  Sequential
streaming of the pre-laid-out edge shard runs at DMA bandwidth instead.

HW pitfalls (probed): matmuls that switch PE row groups (partition-offset
lhsT/rhs) within one PSUM tile crash the device (the K=128 block-diagonal
formulation sidesteps row groups); Pool-engine TensorTensor is rejected by
this lowering; softmax denominators accumulated from bf16 exps lose ~6x
final accuracy (E stays f32; bf16 is cast only for the message multiply).
"""

import os

import numpy as np
import ml_dtypes

import concourse.bacc as bacc
import concourse.tile as tile
from concourse import mybir
from concourse.bass_utils import run_bass_kernel_spmd

F32 = mybir.dt.float32
BF16 = mybir.dt.bfloat16

N_CORES = 8
P = 128
HEADS = 3
CH = 12
HC = HEADS * CH          # 36
ROW = HC + 4             # matmul output row: 36 h | 3 a_s | 1 pad = 40
ROW2 = 2 * ROW           # block-diagonal pair output
NEG_SLOPE = 0.2
GP = 6                   # slot pairs per PSUM tile (6*80 = 480 f32 <= 512)

_nc_cache = {}


def _build_nc(in_dim, n_dst_tiles, L_list, half_tot, n_xt_cols, groups):
    key = (in_dim, n_dst_tiles, tuple(L_list), half_tot, n_xt_cols, tuple(groups))
    if key in _nc_cache:
        return _nc_cache[key]

    nc = bacc.Bacc("TRN2", target_bir_lowering=False, debug=False)
    d_xe = nc.dram_tensor("xe", [2 * in_dim, half_tot * P], BF16, kind="ExternalInput")
    d_xt = nc.dram_tensor("xt_t", [in_dim, n_xt_cols], F32, kind="ExternalInput")
    d_pc = nc.dram_tensor("padc4", [P, n_dst_tiles * 4], F32, kind="ExternalInput")
    d_oh = nc.dram_tensor("oh", [P, n_dst_tiles * P], F32, kind="ExternalInput")
    d_wf = nc.dram_tensor("wfbd", [2 * in_dim, ROW2], BF16, kind="ExternalInput")
    d_wt = nc.dram_tensor("wat", [in_dim, 4], F32, kind="ExternalInput")
    d_bb = nc.dram_tensor("biasb", [P, HC], F32, kind="ExternalInput")
    d_q = nc.dram_tensor("q_out", [P, HC], F32, kind="ExternalOutput")

    with tile.TileContext(nc) as tc:
        with tc.tile_pool(name="const", bufs=1) as cpool, \
             tc.tile_pool(name="xload", bufs=2) as xpool, \
             tc.tile_pool(name="gat", bufs=2) as gpool, \
             tc.tile_pool(name="work", bufs=3) as wpool, \
             tc.tile_pool(name="psA", bufs=3, space="PSUM") as psA, \
             tc.tile_pool(name="psB", bufs=1, space="PSUM") as psB, \
             tc.tile_pool(name="psT", bufs=2, space="PSUM") as psT:

            # ---- constants into SBUF ----
            t_wf = cpool.tile([2 * in_dim, ROW2], BF16)
            nc.sync.dma_start(t_wf[:], d_wf[:])
            t_wt = cpool.tile([in_dim, 4], F32)
            nc.sync.dma_start(t_wt[:], d_wt[:])
            t_bb = cpool.tile([P, HC], F32)
            nc.sync.dma_start(t_bb[:], d_bb[:])
            t_pc = cpool.tile([P, n_dst_tiles * 4], F32)
            nc.sync.dma_start(t_pc[:], d_pc[:])
            t_xt = cpool.tile([in_dim, n_xt_cols], F32)
            nc.sync.dma_start(t_xt[:], d_xt[:])

            # ---- phase A2: a_t per dst tile -> resident SBUF ----
            t_at = cpool.tile([P, n_dst_tiles * 4], F32)
            for t0 in range(0, n_dst_tiles, 4):
                nt = min(4, n_dst_tiles - t0)
                ps = psT.tile([P, 16], F32, space="PSUM", tag="psat")
                for j in range(nt):
                    nc.tensor.matmul(
                        ps[:, j * 4:(j + 1) * 4],
                        lhsT=t_xt[:, (t0 + j) * P:(t0 + j + 1) * P], rhs=t_wt[:],
                        start=True, stop=True)
                nc.scalar.copy(
                    t_at[:, t0 * 4:(t0 + nt) * 4], ps[:, :nt * 4])

            # exp(leaky_relu(z)) = max(exp(z), exp(0.2 z)) — exp monotonic.
            # Pad-slot denominator correction for all tiles at once:
            # corall = padc * max(exp(a_t), exp(0.2 a_t)).
            t_at2 = cpool.tile([P, n_dst_tiles * 4], F32)
            nc.vector.tensor_scalar_mul(t_at2[:], t_at[:], NEG_SLOPE)
            t_ep = cpool.tile([P, n_dst_tiles * 4], F32)
            t_ep2 = cpool.tile([P, n_dst_tiles * 4], F32)
            nc.scalar.activation(
                t_ep[:], t_at[:], mybir.ActivationFunctionType.Exp)
            nc.scalar.activation(
                t_ep2[:], t_at2[:], mybir.ActivationFunctionType.Exp)
            nc.vector.tensor_tensor(
                out=t_ep[:], in0=t_ep[:], in1=t_ep2[:], op=mybir.AluOpType.max)
            t_cor = cpool.tile([P, n_dst_tiles * 4], F32)
            nc.vector.tensor_tensor(
                out=t_cor[:], in0=t_ep[:], in1=t_pc[:], op=mybir.AluOpType.mult)

            # ---- phase B: tiles processed in groups sharing L (the few
            # high-degree tiles run solo; the rest in groups of 4) ----
            ps_q = psB.tile([P, HC], F32, space="PSUM", tag="q")
            off_h = 0
            for (t0, gs) in groups:
                L = L_list[t0]         # shared within group, multiple of 4
                Lh = L // 2
                GL = gs * L            # slots in group
                xe_sb = xpool.tile([2 * in_dim, gs * Lh * P], BF16, tag="xe")
                nc.sync.dma_start(
                    xe_sb[:], d_xe[:, off_h * P:(off_h + gs * Lh) * P])
                off_h += gs * Lh

                # per-edge rows via PE: one K=128 matmul per slot PAIR
                g = gpool.tile([P, GL * ROW], BF16, tag="G")
                for st in range(gs):
                    for h0 in range(0, Lh, GP):
                        nh = min(GP, Lh - h0)
                        ps = psA.tile([P, GP * ROW2], F32, space="PSUM", tag="psa")
                        for j in range(nh):
                            nc.tensor.matmul(
                                ps[:, j * ROW2:(j + 1) * ROW2],
                                lhsT=xe_sb[:, (st * Lh + h0 + j) * P:
                                           (st * Lh + h0 + j + 1) * P],
                                rhs=t_wf[:],
                                start=True, stop=True)
                        nc.scalar.copy(
                            g[:, (st * Lh + h0) * ROW2:
                              (st * Lh + h0 + nh) * ROW2],
                            ps[:, :nh * ROW2])

                g4 = g[:].rearrange("p (s l c) -> p s l c", s=gs, c=ROW)

                # z = a_s + a_t for the whole group  (layout (s, l, h))
                tZ = wpool.tile([P, GL * HEADS], F32, tag="Z")
                Z4 = tZ[:].rearrange("p (s l h) -> p s l h", s=gs, h=HEADS)
                at_b = (t_at[:, t0 * 4:(t0 + gs) * 4]
                        .rearrange("p (s h) -> p s h", h=4)[:, :, 0:HEADS]
                        .unsqueeze(2).to_broadcast((P, gs, L, HEADS)))
                nc.vector.tensor_tensor(
                    out=Z4[:], in0=g4[:, :, :, HC:HC + HEADS], in1=at_b,
                    op=mybir.AluOpType.add)
                tE = wpool.tile([P, GL * HEADS], F32, tag="E")
                tT = wpool.tile([P, GL * HEADS], F32, tag="T")
                nc.scalar.activation(
                    tT[:], tZ[:], mybir.ActivationFunctionType.Exp)
                nc.scalar.activation(
                    tE[:], tZ[:], mybir.ActivationFunctionType.Exp,
                    scale=NEG_SLOPE)
                nc.vector.tensor_tensor(
                    out=tE[:], in0=tE[:], in1=tT[:], op=mybir.AluOpType.max)
                tEb = wpool.tile([P, GL * HEADS], BF16, tag="Eb")
                nc.scalar.copy(tEb[:], tE[:])

                # denominators + pad correction + reciprocal
                # (contiguous per-sub-tile outputs; strided reduce outputs
                # misbehave on HW)
                t_den = wpool.tile([P, 4 * gs], F32, tag="den")
                t_rec = wpool.tile([P, 4 * gs], F32, tag="rec")
                nc.vector.memset(t_den[:], 1.0)
                for st in range(gs):
                    nc.vector.tensor_reduce(
                        out=t_den[:, st * 4:st * 4 + HEADS],
                        in_=tE[:, st * L * HEADS:(st + 1) * L * HEADS]
                        .rearrange("p (l h) -> p l h", h=HEADS)
                        .transpose([0, 2, 1]),
                        axis=mybir.AxisListType.X, op=mybir.AluOpType.add)
                dv = (t_den[:].rearrange("p (s h) -> p s h", h=4)
                      [:, :, 0:HEADS])
                cview = (t_cor[:, t0 * 4:(t0 + gs) * 4]
                         .rearrange("p (s h) -> p s h", h=4)[:, :, 0:HEADS])
                nc.vector.tensor_tensor(
                    out=dv, in0=dv, in1=cview, op=mybir.AluOpType.subtract)
                nc.vector.tensor_scalar_max(t_den[:], t_den[:], 1e-30)
                nc.vector.reciprocal(t_rec[:], t_den[:])

                # weighted messages M = e * h (bf16 2x; c-major h block);
                # slot-sum via two fold-by-half tree levels + reduce tail
                tM = wpool.tile([P, GL * HC], BF16, tag="M")
                tU = wpool.tile([P, gs * HC], F32, tag="U")
                for st in range(gs):
                    Ms = tM[:, st * L * HC:(st + 1) * L * HC]
                    M4v = Ms.rearrange("p (l c h) -> p l c h", c=CH, h=HEADS)
                    e_b = (tEb[:, st * L * HEADS:(st + 1) * L * HEADS]
                           .rearrange("p (l h) -> p l h", h=HEADS)
                           .unsqueeze(2).to_broadcast((P, L, CH, HEADS)))
                    g3 = g4[:, st, :, 0:HC].rearrange(
                        "p l (c h) -> p l c h", h=HEADS)
                    nc.vector.tensor_tensor(
                        out=M4v[:], in0=g3, in1=e_b, op=mybir.AluOpType.mult)
                    for n in (L // 2, L // 4):
                        a = Ms[:, :2 * n * HC].rearrange(
                            "p (n two c) -> p n two c", two=2, c=HC)
                        nc.vector.tensor_tensor(
                            out=Ms[:, :n * HC].rearrange(
                                "p (n c) -> p n c", c=HC),
                            in0=a[:, :, 0, :], in1=a[:, :, 1, :],
                            op=mybir.AluOpType.add)
                # combined tail reduce over the group: in (s, c, n)
                nc.vector.tensor_reduce(
                    out=tU[:].rearrange("p (s c) -> p s c", s=gs),
                    in_=tM[:].rearrange("p (s q) -> p s q", s=gs)
                    [:, :, :(L // 4) * HC]
                    .rearrange("p s (n c) -> p s n c", c=HC)
                    .transpose([0, 1, 3, 2]),
                    axis=mybir.AxisListType.X, op=mybir.AluOpType.add)

                # V = relu(U / denom + bias); relu on ACT casts to bf16
                tV = wpool.tile([P, gs * HC], F32, tag="V")
                for st in range(gs):
                    rec_b = (t_rec[:, st * 4:st * 4 + HEADS].unsqueeze(1)
                             .to_broadcast((P, CH, HEADS)))
                    nc.vector.tensor_tensor(
                        out=tV[:, st * HC:(st + 1) * HC].rearrange(
                            "p (c h) -> p c h", h=HEADS),
                        in0=tU[:, st * HC:(st + 1) * HC].rearrange(
                            "p (c h) -> p c h", h=HEADS),
                        in1=rec_b, op=mybir.AluOpType.mult)
                bb_b = t_bb[:].unsqueeze(1).to_broadcast((P, gs, HC))
                nc.vector.tensor_tensor(
                    out=tV[:].rearrange("p (s c) -> p s c", s=gs),
                    in0=tV[:].rearrange("p (s c) -> p s c", s=gs),
                    in1=bb_b, op=mybir.AluOpType.add)
                nc.scalar.activation(
                    tV[:], tV[:], mybir.ActivationFunctionType.Relu)

                # pool into batches: q[b, 36] += onehot_t^T @ V, PSUM-accum
                t_oh = wpool.tile([P, gs * P], F32, tag="oh")
                nc.sync.dma_start(t_oh[:], d_oh[:, t0 * P:(t0 + gs) * P])
                for st in range(gs):
                    t = t0 + st
                    nc.tensor.matmul(
                        ps_q[:], lhsT=t_oh[:, st * P:(st + 1) * P],
                        rhs=tV[:, st * HC:(st + 1) * HC],
                        start=(t == 0), stop=(t == n_dst_tiles - 1))

            t_q = cpool.tile([P, HC], F32)
            nc.vector.tensor_copy(t_q[:], ps_q[:])
            nc.sync.dma_start(d_q[:], t_q[:])
    nc.finalize()
    _nc_cache[key] = nc
    return nc


def kernel(**inputs):
    x_s = np.asarray(inputs["x_s"], np.float32)
    x_t = np.asarray(inputs["x_t"], np.float32)
    edge_index = np.asarray(inputs["edge_index"])
    x_s_batch = np.asarray(inputs["x_s_batch"]).astype(np.int64)
    W = np.asarray(inputs["W"], np.float32)
    att_src = np.asarray(inputs["att_src"], np.float32)
    att_dst = np.asarray(inputs["att_dst"], np.float32)
    bias = np.asarray(inputs["bias"], np.float32)
    fc1_w = np.asarray(inputs["fc1_w"], np.float32)
    fc1_b = np.asarray(inputs["fc1_b"], np.float32)
    fc3_w = np.asarray(inputs["fc3_w"], np.float32)
    fc3_b = np.asarray(inputs["fc3_b"], np.float32)

    n_nodes, in_dim = x_s.shape
    src = edge_index[0].astype(np.int64)
    dst = edge_index[1].astype(np.int64)

    # ---- host: edge bucketing by destination (index/layout prep only) ----
    deg = np.bincount(dst, minlength=n_nodes)
    order = np.argsort(-deg, kind="stable")          # nodes by degree desc
    inv_order = np.empty(n_nodes, np.int64)
    inv_order[order] = np.arange(n_nodes)
    nodes_per_core = (n_nodes + N_CORES - 1) // N_CORES
    n_dst_tiles = (nodes_per_core + P - 1) // P
    n_dst_tiles = (n_dst_tiles + 3) // 4 * 4         # whole groups of 4
    L_list = []
    for t in range(n_dst_tiles):
        r0 = t * P * N_CORES
        L = max(4, int(deg[order[min(r0, n_nodes - 1)]]))
        L_list.append((L + 3) // 4 * 4)              # multiple of 4
    k = 0                                            # solo tiles (big L)
    while k < n_dst_tiles and L_list[k] > 48:
        k += 1
    k = min((k + 3) // 4 * 4, n_dst_tiles)
    if int(os.environ.get("KB_NOGRP", 0)):
        k = n_dst_tiles
    groups = [(t, 1) for t in range(k)]
    for g in range(k, n_dst_tiles, 4):
        Lg = max(L_list[g:g + 4])                    # shared within group
        for t in range(g, g + 4):
            L_list[t] = Lg
        groups.append((g, 4))
    groups = tuple(groups)
    off_arr = np.concatenate([[0], np.cumsum(L_list)]).astype(np.int64)
    slot_tot = int(off_arr[-1])
    half_tot = slot_tot // 2
    n_xt_cols = n_dst_tiles * P

    # edges sorted by dst -> per-node contiguous src runs
    e_order = np.argsort(dst, kind="stable")
    dst_sorted = dst[e_order]
    src_sorted = src[e_order].astype(np.int64)
    starts = np.searchsorted(dst_sorted, np.arange(n_nodes))
    slot_within = np.arange(len(dst_sorted)) - starts[dst_sorted]

    k_global = inv_order[dst_sorted]
    core_of = (k_global % N_CORES).astype(np.int64)
    k_local = k_global // N_CORES
    t_of = k_local // P
    p_of = k_local % P
    col_of = off_arr[t_of] + slot_within

    # fold weights (host weight prep).  W/bias/w2 columns permuted c-major:
    # folded col (c*HEADS + h) <- original col (h*CH + c).
    cm = np.array([h * CH + c for c in range(CH) for h in range(HEADS)])
    W_cm = W[:, cm]
    bias_cm = bias[cm]
    w2_cm = (fc1_w @ fc3_w)[:, 0].astype(np.float32)[cm]

    wa_t = np.einsum("khc,hc->kh", W.reshape(in_dim, HEADS, CH), att_dst)
    wa_s = np.einsum("khc,hc->kh", W.reshape(in_dim, HEADS, CH), att_src)
    wfold = np.zeros((in_dim, ROW), np.float32)
    wfold[:, :HC] = W_cm
    wfold[:, HC:HC + HEADS] = wa_s
    wfbd = np.zeros((2 * in_dim, ROW2), np.float32)
    wfbd[:in_dim, :ROW] = wfold
    wfbd[in_dim:, ROW:] = wfold
    wfbd = wfbd.astype(ml_dtypes.bfloat16)
    wat = np.zeros((in_dim, 4), np.float32)
    wat[:, :HEADS] = wa_t
    biasb = np.tile(bias_cm[None, :], (P, 1)).astype(np.float32)

    xsb_ext = np.zeros((n_nodes + 1, in_dim), ml_dtypes.bfloat16)
    xsb_ext[:n_nodes] = x_s.astype(ml_dtypes.bfloat16)
    SENT = n_nodes

    in_maps = []
    cnts = []
    for c in range(N_CORES):
        node_ids = order[c::N_CORES]                 # this core's dst nodes
        ncnt = len(node_ids)
        m = core_of == c
        SRC = np.full((P, slot_tot), SENT, np.int64)
        SRC[p_of[m], col_of[m]] = src_sorted[m]

        # per-edge lhsT layout: rows 0:64 even slots, 64:128 odd slots
        xe = np.empty((2 * in_dim, half_tot * P), ml_dtypes.bfloat16)
        for par in range(2):
            S = SRC[:, par::2]                       # [P, half_tot]
            blk = xsb_ext[S]                         # [P, half_tot, in_dim]
            xe[par * in_dim:(par + 1) * in_dim] = (
                blk.transpose(2, 1, 0).reshape(in_dim, half_tot * P))

        padc4 = np.zeros((P, n_dst_tiles * 4), np.float32)
        oh = np.zeros((P, n_dst_tiles * P), np.float32)
        xt_t = np.zeros((in_dim, n_xt_cols), np.float32)
        kk = np.arange(n_dst_tiles * P)
        tt, pp = kk // P, kk % P
        present = kk < ncnt
        nid = np.where(present, node_ids[np.minimum(kk, ncnt - 1)], 0)
        Leff = np.asarray(L_list, np.float32)[tt]
        pc = np.where(present, Leff - deg[nid], Leff)
        for j in range(4):
            padc4[pp, 4 * tt + j] = pc
        bid = x_s_batch[nid]
        oh[pp[present], tt[present] * P + bid[present]] = 1.0
        cnts.append(np.bincount(bid[present], minlength=P).astype(np.float64))
        xt_t[:, :ncnt] = x_t[node_ids].T
        in_maps.append({
            "xe": xe, "xt_t": xt_t, "padc4": padc4, "oh": oh, "wfbd": wfbd,
            "wat": wat, "biasb": biasb,
        })

    nc = _build_nc(in_dim, n_dst_tiles, L_list, half_tot, n_xt_cols, groups)
    res = run_bass_kernel_spmd(nc, in_maps, core_ids=list(range(N_CORES)))

    q = np.zeros((P, HC), np.float64)
    cnt = np.zeros(P, np.float64)
    for c in range(N_CORES):
        q += res.results[c]["q_out"]
        cnt += cnts[c]
    num = q @ w2_cm.astype(np.float64)
    out = num / np.maximum(cnt, 1.0)
    const = float(fc1_b @ fc3_w[:, 0] + fc3_b[0])
    return (out + const).astype(np.float32)


# revision 34
# speedup vs baseline: 12.6932x; 1.0117x over previous
"""GAT (bipartite GATConv + mean-pool + 2 FC) on 8 Trainium2 NeuronCores.

Strategy: edges are sharded per destination node; destination nodes are
dealt round-robin (degree-sorted) across the 8 cores so the segment softmax
is fully local to a core.  Per the sharding hint each device holds its edge
shard with the source-node features replicated into matmul-ready per-edge
layout (host does only index manipulation / np.take layout; every model
FLOP runs on device):

  Phase A2: a_t = x_t @ (W att_dst) for this core's dst nodes (PE).
  Phase B: dst nodes are processed in tiles of 128 (one node per partition,
           nodes degree-sorted so tiles have uniform run lengths L).  The
           per-edge source features arrive as bf16 lhsT tiles [128, L/2*128]
           (slot pair 2j/2j+1 stacked as two K=64 halves); one K=128 matmul
           against a block-diagonal [wfold|0 / 0|wfold] rhs computes BOTH
           slots' rows [h_s (36, c-major) | a_s (3) | pad] into PSUM.  ACT
           casts them to bf16 SBUF; E = exp(leaky_relu(a_s+a_t)) =
           max(exp(z), exp(0.2 z)) via two ACT Exp passes over a DVE-added
           z.  The weighted message sum runs in bf16 on DVE's packed 2x
           path: W columns are stored c-major/h-minor so the E broadcast's
           innermost dim is the packed head dim, and the slot reduction is
           two packed tree-add levels plus a short tensor_reduce tail.
           Batch pooling is a PE matmul against host-shipped one-hot
           columns, accumulated over all tiles in PSUM; the final fc1@fc3
           contraction and count division happen on host partials.
           Pad slots carry x=0 => h=0, a_s=0; their exp(leaky_relu(a_t))
           denominator contribution is subtracted exactly via a
           host-precomputed pad-count correction.

Device-side per-edge gathers are avoided entirely: one [P,1]-offset
indirect-DMA gather costs ~1.1us of SWDGE descriptor generation on the Pool
engine (994ns fixed + 0.34ns/desc, 128 descriptors max per instruction) and
the batched-gather ucode (InstDMAGatherAnt etc.) is excluded from bedrock
images, so any gather-based design is floored at ~3.3ms/core.  Sequential
streaming of the pre-laid-out edge shard runs at DMA bandwidth instead.

HW pitfalls (probed): matmuls that switch PE row groups (partition-offset
lhsT/rhs) within one PSUM tile crash the device (the K=128 block-diagonal
formulation sidesteps row groups); Pool-engine TensorTensor is rejected by
this lowering; softmax denominators accumulated from bf16 exps lose ~6x
final accuracy (E stays f32; bf16 is cast only for the message multiply).
"""

import numpy as np
import ml_dtypes

import concourse.bacc as bacc
import concourse.tile as tile
from concourse import mybir
from concourse.bass_utils import run_bass_kernel_spmd

F32 = mybir.dt.float32
BF16 = mybir.dt.bfloat16

N_CORES = 8
P = 128
HEADS = 3
CH = 12
HC = HEADS * CH          # 36
ROW = HC + 4             # matmul output row: 36 h | 3 a_s | 1 pad = 40
ROW2 = 2 * ROW           # block-diagonal pair output
NEG_SLOPE = 0.2
GP = 6                   # slot pairs per PSUM tile (6*80 = 480 f32 <= 512)

_nc_cache = {}


def _build_nc(in_dim, n_dst_tiles, L_list, half_tot, n_xt_cols, groups):
    key = (in_dim, n_dst_tiles, tuple(L_list), half_tot, n_xt_cols, tuple(groups))
    if key in _nc_cache:
        return _nc_cache[key]

    nc = bacc.Bacc("TRN2", target_bir_lowering=False, debug=False)
    d_xe = nc.dram_tensor("xe", [2 * in_dim, half_tot * P], BF16, kind="ExternalInput")
    d_xt = nc.dram_tensor("xt_t", [in_dim, n_xt_cols], F32, kind="ExternalInput")
    d_pc = nc.dram_tensor("padc4", [P, n_dst_tiles * 4], F32, kind="ExternalInput")
    d_oh = nc.dram_tensor("oh", [P, n_dst_tiles * P], F32, kind="ExternalInput")
    d_wf = nc.dram_tensor("wfbd", [2 * in_dim, ROW2], BF16, kind="ExternalInput")
    d_wt = nc.dram_tensor("wat", [in_dim, 4], F32, kind="ExternalInput")
    d_bb = nc.dram_tensor("biasb", [P, HC], F32, kind="ExternalInput")
    d_q = nc.dram_tensor("q_out", [P, HC], F32, kind="ExternalOutput")

    with tile.TileContext(nc) as tc:
        with tc.tile_pool(name="const", bufs=1) as cpool, \
             tc.tile_pool(name="xload", bufs=2) as xpool, \
             tc.tile_pool(name="gat", bufs=3) as gpool, \
             tc.tile_pool(name="work", bufs=2) as wpool, \
             tc.tile_pool(name="psA", bufs=4, space="PSUM") as psA, \
             tc.tile_pool(name="psB", bufs=1, space="PSUM") as psB, \
             tc.tile_pool(name="psT", bufs=2, space="PSUM") as psT:

            # ---- constants into SBUF ----
            t_wf = cpool.tile([2 * in_dim, ROW2], BF16)
            nc.sync.dma_start(t_wf[:], d_wf[:])
            t_wt = cpool.tile([in_dim, 4], F32)
            nc.sync.dma_start(t_wt[:], d_wt[:])
            t_bb = cpool.tile([P, HC], F32)
            nc.sync.dma_start(t_bb[:], d_bb[:])
            t_pc = cpool.tile([P, n_dst_tiles * 4], F32)
            nc.sync.dma_start(t_pc[:], d_pc[:])
            t_xt = cpool.tile([in_dim, n_xt_cols], F32)
            nc.sync.dma_start(t_xt[:], d_xt[:])

            # ---- phase A2: a_t per dst tile -> resident SBUF ----
            t_at = cpool.tile([P, n_dst_tiles * 4], F32)
            for t0 in range(0, n_dst_tiles, 4):
                nt = min(4, n_dst_tiles - t0)
                ps = psT.tile([P, 16], F32, space="PSUM", tag="psat")
                for j in range(nt):
                    nc.tensor.matmul(
                        ps[:, j * 4:(j + 1) * 4],
                        lhsT=t_xt[:, (t0 + j) * P:(t0 + j + 1) * P], rhs=t_wt[:],
                        start=True, stop=True)
                nc.scalar.copy(
                    t_at[:, t0 * 4:(t0 + nt) * 4], ps[:, :nt * 4])

            # exp(leaky_relu(z)) = max(exp(z), exp(0.2 z)) — exp monotonic.
            # Pad-slot denominator correction for all tiles at once:
            # corall = padc * max(exp(a_t), exp(0.2 a_t)).
            t_at2 = cpool.tile([P, n_dst_tiles * 4], F32)
            nc.vector.tensor_scalar_mul(t_at2[:], t_at[:], NEG_SLOPE)
            t_ep = cpool.tile([P, n_dst_tiles * 4], F32)
            t_ep2 = cpool.tile([P, n_dst_tiles * 4], F32)
            nc.scalar.activation(
                t_ep[:], t_at[:], mybir.ActivationFunctionType.Exp)
            nc.scalar.activation(
                t_ep2[:], t_at2[:], mybir.ActivationFunctionType.Exp)
            nc.vector.tensor_tensor(
                out=t_ep[:], in0=t_ep[:], in1=t_ep2[:], op=mybir.AluOpType.max)
            t_cor = cpool.tile([P, n_dst_tiles * 4], F32)
            nc.vector.tensor_tensor(
                out=t_cor[:], in0=t_ep[:], in1=t_pc[:], op=mybir.AluOpType.mult)

            # ---- phase B: tiles processed in groups sharing L (the few
            # high-degree tiles run solo; the rest in groups of 4) ----
            ps_q = psB.tile([P, HC], F32, space="PSUM", tag="q")
            off_h = 0
            for (t0, gs) in groups:
                L = L_list[t0]         # shared within group, multiple of 4
                Lh = L // 2
                GL = gs * L            # slots in group
                xe_sb = xpool.tile([2 * in_dim, gs * Lh * P], BF16, tag="xe")
                nc.sync.dma_start(
                    xe_sb[:], d_xe[:, off_h * P:(off_h + gs * Lh) * P])
                off_h += gs * Lh

                # per-edge rows via PE: one K=128 matmul per slot PAIR
                g = gpool.tile([P, GL * ROW], BF16, tag="G")
                for st in range(gs):
                    for h0 in range(0, Lh, GP):
                        nh = min(GP, Lh - h0)
                        ps = psA.tile([P, GP * ROW2], F32, space="PSUM", tag="psa")
                        for j in range(nh):
                            nc.tensor.matmul(
                                ps[:, j * ROW2:(j + 1) * ROW2],
                                lhsT=xe_sb[:, (st * Lh + h0 + j) * P:
                                           (st * Lh + h0 + j + 1) * P],
                                rhs=t_wf[:],
                                start=True, stop=True)
                        nc.scalar.copy(
                            g[:, (st * Lh + h0) * ROW2:
                              (st * Lh + h0 + nh) * ROW2],
                            ps[:, :nh * ROW2])

                g4 = g[:].rearrange("p (s l c) -> p s l c", s=gs, c=ROW)

                # z = a_s + a_t for the whole group  (layout (s, l, h))
                tZ = wpool.tile([P, GL * HEADS], F32, tag="Z")
                Z4 = tZ[:].rearrange("p (s l h) -> p s l h", s=gs, h=HEADS)
                at_b = (t_at[:, t0 * 4:(t0 + gs) * 4]
                        .rearrange("p (s h) -> p s h", h=4)[:, :, 0:HEADS]
                        .unsqueeze(2).to_broadcast((P, gs, L, HEADS)))
                nc.vector.tensor_tensor(
                    out=Z4[:], in0=g4[:, :, :, HC:HC + HEADS], in1=at_b,
                    op=mybir.AluOpType.add)
                tE = wpool.tile([P, GL * HEADS], F32, tag="E")
                tT = wpool.tile([P, GL * HEADS], F32, tag="T")
                nc.scalar.activation(
                    tT[:], tZ[:], mybir.ActivationFunctionType.Exp)
                nc.scalar.activation(
                    tE[:], tZ[:], mybir.ActivationFunctionType.Exp,
                    scale=NEG_SLOPE)
                nc.vector.tensor_tensor(
                    out=tE[:], in0=tE[:], in1=tT[:], op=mybir.AluOpType.max)
                tEb = wpool.tile([P, GL * HEADS], BF16, tag="Eb")
                nc.scalar.copy(tEb[:], tE[:])

                # denominators + pad correction + reciprocal
                # (contiguous per-sub-tile outputs; strided reduce outputs
                # misbehave on HW)
                t_den = wpool.tile([P, 4 * gs], F32, tag="den")
                t_rec = wpool.tile([P, 4 * gs], F32, tag="rec")
                nc.vector.memset(t_den[:], 1.0)
                for st in range(gs):
                    nc.vector.tensor_reduce(
                        out=t_den[:, st * 4:st * 4 + HEADS],
                        in_=tE[:, st * L * HEADS:(st + 1) * L * HEADS]
                        .rearrange("p (l h) -> p l h", h=HEADS)
                        .transpose([0, 2, 1]),
                        axis=mybir.AxisListType.X, op=mybir.AluOpType.add)
                dv = (t_den[:].rearrange("p (s h) -> p s h", h=4)
                      [:, :, 0:HEADS])
                cview = (t_cor[:, t0 * 4:(t0 + gs) * 4]
                         .rearrange("p (s h) -> p s h", h=4)[:, :, 0:HEADS])
                nc.vector.tensor_tensor(
                    out=dv, in0=dv, in1=cview, op=mybir.AluOpType.subtract)
                nc.vector.tensor_scalar_max(t_den[:], t_den[:], 1e-30)
                nc.vector.reciprocal(t_rec[:], t_den[:])

                # weighted messages M = e * h (bf16 2x; c-major h block);
                # slot-sum via two fold-by-half tree levels + reduce tail
                tM = wpool.tile([P, GL * HC], BF16, tag="M")
                tU = wpool.tile([P, gs * HC], F32, tag="U")
                for st in range(gs):
                    Ms = tM[:, st * L * HC:(st + 1) * L * HC]
                    M4v = Ms.rearrange("p (l c h) -> p l c h", c=CH, h=HEADS)
                    e_b = (tEb[:, st * L * HEADS:(st + 1) * L * HEADS]
                           .rearrange("p (l h) -> p l h", h=HEADS)
                           .unsqueeze(2).to_broadcast((P, L, CH, HEADS)))
                    g3 = g4[:, st, :, 0:HC].rearrange(
                        "p l (c h) -> p l c h", h=HEADS)
                    nc.vector.tensor_tensor(
                        out=M4v[:], in0=g3, in1=e_b, op=mybir.AluOpType.mult)
                    for n in (L // 2, L // 4):
                        a = Ms[:, :2 * n * HC].rearrange(
                            "p (n two c) -> p n two c", two=2, c=HC)
                        nc.vector.tensor_tensor(
                            out=Ms[:, :n * HC].rearrange(
                                "p (n c) -> p n c", c=HC),
                            in0=a[:, :, 0, :], in1=a[:, :, 1, :],
                            op=mybir.AluOpType.add)
                # combined tail reduce over the group: in (s, c, n)
                nc.vector.tensor_reduce(
                    out=tU[:].rearrange("p (s c) -> p s c", s=gs),
                    in_=tM[:].rearrange("p (s q) -> p s q", s=gs)
                    [:, :, :(L // 4) * HC]
                    .rearrange("p s (n c) -> p s n c", c=HC)
                    .transpose([0, 1, 3, 2]),
                    axis=mybir.AxisListType.X, op=mybir.AluOpType.add)

                # V = relu(U / denom + bias); relu on ACT casts to bf16
                tV = wpool.tile([P, gs * HC], F32, tag="V")
                for st in range(gs):
                    rec_b = (t_rec[:, st * 4:st * 4 + HEADS].unsqueeze(1)
                             .to_broadcast((P, CH, HEADS)))
                    nc.vector.tensor_tensor(
                        out=tV[:, st * HC:(st + 1) * HC].rearrange(
                            "p (c h) -> p c h", h=HEADS),
                        in0=tU[:, st * HC:(st + 1) * HC].rearrange(
                            "p (c h) -> p c h", h=HEADS),
                        in1=rec_b, op=mybir.AluOpType.mult)
                bb_b = t_bb[:].unsqueeze(1).to_broadcast((P, gs, HC))
                nc.vector.tensor_tensor(
                    out=tV[:].rearrange("p (s c) -> p s c", s=gs),
                    in0=tV[:].rearrange("p (s c) -> p s c", s=gs),
                    in1=bb_b, op=mybir.AluOpType.add)
                nc.scalar.activation(
                    tV[:], tV[:], mybir.ActivationFunctionType.Relu)

                # pool into batches: q[b, 36] += onehot_t^T @ V, PSUM-accum
                t_oh = wpool.tile([P, gs * P], F32, tag="oh")
                nc.sync.dma_start(t_oh[:], d_oh[:, t0 * P:(t0 + gs) * P])
                for st in range(gs):
                    t = t0 + st
                    nc.tensor.matmul(
                        ps_q[:], lhsT=t_oh[:, st * P:(st + 1) * P],
                        rhs=tV[:, st * HC:(st + 1) * HC],
                        start=(t == 0), stop=(t == n_dst_tiles - 1))

            t_q = cpool.tile([P, HC], F32)
            nc.vector.tensor_copy(t_q[:], ps_q[:])
            nc.sync.dma_start(d_q[:], t_q[:])
    nc.finalize()
    _nc_cache[key] = nc
    return nc


def kernel(**inputs):
    x_s = np.asarray(inputs["x_s"], np.float32)
    x_t = np.asarray(inputs["x_t"], np.float32)
    edge_index = np.asarray(inputs["edge_index"])
    x_s_batch = np.asarray(inputs["x_s_batch"]).astype(np.int64)
    W = np.asarray(inputs["W"], np.float32)
    att_src = np.asarray(inputs["att_src"], np.float32)
    att_dst = np.asarray(inputs["att_dst"], np.float32)
    bias = np.asarray(inputs["bias"], np.float32)
    fc1_w = np.asarray(inputs["fc1_w"], np.float32)
    fc1_b = np.asarray(inputs["fc1_b"], np.float32)
    fc3_w = np.asarray(inputs["fc3_w"], np.float32)
    fc3_b = np.asarray(inputs["fc3_b"], np.float32)

    n_nodes, in_dim = x_s.shape
    src = edge_index[0].astype(np.int64)
    dst = edge_index[1].astype(np.int64)

    # ---- host: edge bucketing by destination (index/layout prep only) ----
    deg = np.bincount(dst, minlength=n_nodes)
    order = np.argsort(-deg, kind="stable")          # nodes by degree desc
    inv_order = np.empty(n_nodes, np.int64)
    inv_order[order] = np.arange(n_nodes)
    nodes_per_core = (n_nodes + N_CORES - 1) // N_CORES
    n_dst_tiles = (nodes_per_core + P - 1) // P
    n_dst_tiles = (n_dst_tiles + 3) // 4 * 4         # whole groups of 4
    L_list = []
    for t in range(n_dst_tiles):
        r0 = t * P * N_CORES
        L = max(4, int(deg[order[min(r0, n_nodes - 1)]]))
        L_list.append((L + 3) // 4 * 4)              # multiple of 4
    k = 0                                            # solo tiles (big L)
    while k < n_dst_tiles and L_list[k] > 44:
        k += 1
    k = min((k + 3) // 4 * 4, n_dst_tiles)
    groups = [(t, 1) for t in range(k)]
    for g in range(k, n_dst_tiles, 4):
        Lg = max(L_list[g:g + 4])                    # shared within group
        for t in range(g, g + 4):
            L_list[t] = Lg
        groups.append((g, 4))
    groups = tuple(groups)
    off_arr = np.concatenate([[0], np.cumsum(L_list)]).astype(np.int64)
    slot_tot = int(off_arr[-1])
    half_tot = slot_tot // 2
    n_xt_cols = n_dst_tiles * P

    # edges sorted by dst -> per-node contiguous src runs
    e_order = np.argsort(dst, kind="stable")
    dst_sorted = dst[e_order]
    src_sorted = src[e_order].astype(np.int64)
    starts = np.searchsorted(dst_sorted, np.arange(n_nodes))
    slot_within = np.arange(len(dst_sorted)) - starts[dst_sorted]

    k_global = inv_order[dst_sorted]
    core_of = (k_global % N_CORES).astype(np.int64)
    k_local = k_global // N_CORES
    t_of = k_local // P
    p_of = k_local % P
    col_of = off_arr[t_of] + slot_within

    # fold weights (host weight prep).  W/bias/w2 columns permuted c-major:
    # folded col (c*HEADS + h) <- original col (h*CH + c).
    cm = np.array([h * CH + c for c in range(CH) for h in range(HEADS)])
    W_cm = W[:, cm]
    bias_cm = bias[cm]
    w2_cm = (fc1_w @ fc3_w)[:, 0].astype(np.float32)[cm]

    wa_t = np.einsum("khc,hc->kh", W.reshape(in_dim, HEADS, CH), att_dst)
    wa_s = np.einsum("khc,hc->kh", W.reshape(in_dim, HEADS, CH), att_src)
    wfold = np.zeros((in_dim, ROW), np.float32)
    wfold[:, :HC] = W_cm
    wfold[:, HC:HC + HEADS] = wa_s
    wfbd = np.zeros((2 * in_dim, ROW2), np.float32)
    wfbd[:in_dim, :ROW] = wfold
    wfbd[in_dim:, ROW:] = wfold
    wfbd = wfbd.astype(ml_dtypes.bfloat16)
    wat = np.zeros((in_dim, 4), np.float32)
    wat[:, :HEADS] = wa_t
    biasb = np.tile(bias_cm[None, :], (P, 1)).astype(np.float32)

    xsb_ext = np.zeros((n_nodes + 1, in_dim), ml_dtypes.bfloat16)
    xsb_ext[:n_nodes] = x_s.astype(ml_dtypes.bfloat16)
    SENT = n_nodes

    in_maps = []
    cnts = []
    for c in range(N_CORES):
        node_ids = order[c::N_CORES]                 # this core's dst nodes
        ncnt = len(node_ids)
        m = core_of == c
        SRC = np.full((P, slot_tot), SENT, np.int64)
        SRC[p_of[m], col_of[m]] = src_sorted[m]

        # per-edge lhsT layout: rows 0:64 even slots, 64:128 odd slots
        xe = np.empty((2 * in_dim, half_tot * P), ml_dtypes.bfloat16)
        for par in range(2):
            S = SRC[:, par::2]                       # [P, half_tot]
            blk = xsb_ext[S]                         # [P, half_tot, in_dim]
            xe[par * in_dim:(par + 1) * in_dim] = (
                blk.transpose(2, 1, 0).reshape(in_dim, half_tot * P))

        padc4 = np.zeros((P, n_dst_tiles * 4), np.float32)
        oh = np.zeros((P, n_dst_tiles * P), np.float32)
        xt_t = np.zeros((in_dim, n_xt_cols), np.float32)
        kk = np.arange(n_dst_tiles * P)
        tt, pp = kk // P, kk % P
        present = kk < ncnt
        nid = np.where(present, node_ids[np.minimum(kk, ncnt - 1)], 0)
        Leff = np.asarray(L_list, np.float32)[tt]
        pc = np.where(present, Leff - deg[nid], Leff)
        for j in range(4):
            padc4[pp, 4 * tt + j] = pc
        bid = x_s_batch[nid]
        oh[pp[present], tt[present] * P + bid[present]] = 1.0
        cnts.append(np.bincount(bid[present], minlength=P).astype(np.float64))
        xt_t[:, :ncnt] = x_t[node_ids].T
        in_maps.append({
            "xe": xe, "xt_t": xt_t, "padc4": padc4, "oh": oh, "wfbd": wfbd,
            "wat": wat, "biasb": biasb,
        })

    nc = _build_nc(in_dim, n_dst_tiles, L_list, half_tot, n_xt_cols, groups)
    res = run_bass_kernel_spmd(nc, in_maps, core_ids=list(range(N_CORES)))

    q = np.zeros((P, HC), np.float64)
    cnt = np.zeros(P, np.float64)
    for c in range(N_CORES):
        q += res.results[c]["q_out"]
        cnt += cnts[c]
    num = q @ w2_cm.astype(np.float64)
    out = num / np.maximum(cnt, 1.0)
    const = float(fc1_b @ fc3_w[:, 0] + fc3_b[0])
    return (out + const).astype(np.float32)


# revision 35
# speedup vs baseline: 12.8171x; 1.0098x over previous
"""GAT (bipartite GATConv + mean-pool + 2 FC) on 8 Trainium2 NeuronCores.

Strategy: edges are sharded per destination node; destination nodes are
dealt round-robin (degree-sorted) across the 8 cores so the segment softmax
is fully local to a core.  Per the sharding hint each device holds its edge
shard with the source-node features replicated into matmul-ready per-edge
layout (host does only index manipulation / np.take layout; every model
FLOP runs on device):

  Phase A2: a_t = x_t @ (W att_dst) for this core's dst nodes (PE).
  Phase B: dst nodes are processed in tiles of 128 (one node per partition,
           nodes degree-sorted so tiles have uniform run lengths L).  The
           per-edge source features arrive as bf16 lhsT tiles [128, L/2*128]
           (slot pair 2j/2j+1 stacked as two K=64 halves); one K=128 matmul
           against a block-diagonal [wfold|0 / 0|wfold] rhs computes BOTH
           slots' rows [h_s (36, c-major) | a_s (3) | pad] into PSUM.  ACT
           casts them to bf16 SBUF; E = exp(leaky_relu(a_s+a_t)) =
           max(exp(z), exp(0.2 z)) via two ACT Exp passes over a DVE-added
           z.  The weighted message sum runs in bf16 on DVE's packed 2x
           path: W columns are stored c-major/h-minor so the E broadcast's
           innermost dim is the packed head dim, and the slot reduction is
           two packed tree-add levels plus a short tensor_reduce tail.
           Batch pooling is a PE matmul against host-shipped one-hot
           columns, accumulated over all tiles in PSUM; the final fc1@fc3
           contraction and count division happen on host partials.
           Pad slots carry x=0 => h=0, a_s=0; their exp(leaky_relu(a_t))
           denominator contribution is subtracted exactly via a
           host-precomputed pad-count correction.

Device-side per-edge gathers are avoided entirely: one [P,1]-offset
indirect-DMA gather costs ~1.1us of SWDGE descriptor generation on the Pool
engine (994ns fixed + 0.34ns/desc, 128 descriptors max per instruction) and
the batched-gather ucode (InstDMAGatherAnt etc.) is excluded from bedrock
images, so any gather-based design is floored at ~3.3ms/core.  Sequential
streaming of the pre-laid-out edge shard runs at DMA bandwidth instead.

HW pitfalls (probed): matmuls that switch PE row groups (partition-offset
lhsT/rhs) within one PSUM tile crash the device (the K=128 block-diagonal
formulation sidesteps row groups); Pool-engine TensorTensor is rejected by
this lowering; softmax denominators accumulated from bf16 exps lose ~6x
final accuracy (E stays f32; bf16 is cast only for the message multiply).
"""

import numpy as np
import ml_dtypes

import concourse.bacc as bacc
import concourse.tile as tile
from concourse import mybir
from concourse.bass_utils import run_bass_kernel_spmd

F32 = mybir.dt.float32
BF16 = mybir.dt.bfloat16

N_CORES = 8
P = 128
HEADS = 3
CH = 12
HC = HEADS * CH          # 36
ROW = HC + 4             # matmul output row: 36 h | 3 a_s | 1 pad = 40
ROW2 = 2 * ROW           # block-diagonal pair output
NEG_SLOPE = 0.2
GP = 6                   # slot pairs per PSUM tile (6*80 = 480 f32 <= 512)

_nc_cache = {}


def _build_nc(in_dim, n_dst_tiles, L_list, half_tot, n_xt_cols, groups):
    key = (in_dim, n_dst_tiles, tuple(L_list), half_tot, n_xt_cols, tuple(groups))
    if key in _nc_cache:
        return _nc_cache[key]

    nc = bacc.Bacc("TRN2", target_bir_lowering=False, debug=False)
    d_xe = nc.dram_tensor("xe", [2 * in_dim, half_tot * P], BF16, kind="ExternalInput")
    d_xt = nc.dram_tensor("xt_t", [in_dim, n_xt_cols], F32, kind="ExternalInput")
    d_pc = nc.dram_tensor("padc4", [P, n_dst_tiles * 4], F32, kind="ExternalInput")
    d_oh = nc.dram_tensor("oh", [P, n_dst_tiles * P], F32, kind="ExternalInput")
    d_wf = nc.dram_tensor("wfbd", [2 * in_dim, ROW2], BF16, kind="ExternalInput")
    d_wt = nc.dram_tensor("wat", [in_dim, 4], F32, kind="ExternalInput")
    d_bb = nc.dram_tensor("biasb", [P, HC], F32, kind="ExternalInput")
    d_q = nc.dram_tensor("q_out", [P, HC], F32, kind="ExternalOutput")

    with tile.TileContext(nc) as tc:
        with tc.tile_pool(name="const", bufs=1) as cpool, \
             tc.tile_pool(name="xload", bufs=2) as xpool, \
             tc.tile_pool(name="gat", bufs=3) as gpool, \
             tc.tile_pool(name="work", bufs=2) as wpool, \
             tc.tile_pool(name="psA", bufs=4, space="PSUM") as psA, \
             tc.tile_pool(name="psB", bufs=1, space="PSUM") as psB, \
             tc.tile_pool(name="psT", bufs=2, space="PSUM") as psT:

            # ---- constants into SBUF ----
            t_wf = cpool.tile([2 * in_dim, ROW2], BF16)
            nc.sync.dma_start(t_wf[:], d_wf[:])
            t_wt = cpool.tile([in_dim, 4], F32)
            nc.sync.dma_start(t_wt[:], d_wt[:])
            t_bb = cpool.tile([P, HC], F32)
            nc.sync.dma_start(t_bb[:], d_bb[:])
            t_pc = cpool.tile([P, n_dst_tiles * 4], F32)
            nc.sync.dma_start(t_pc[:], d_pc[:])
            t_xt = cpool.tile([in_dim, n_xt_cols], F32)
            nc.sync.dma_start(t_xt[:], d_xt[:])

            # ---- phase A2: a_t per dst tile -> resident SBUF ----
            t_at = cpool.tile([P, n_dst_tiles * 4], F32)
            for t0 in range(0, n_dst_tiles, 4):
                nt = min(4, n_dst_tiles - t0)
                ps = psT.tile([P, 16], F32, space="PSUM", tag="psat")
                for j in range(nt):
                    nc.tensor.matmul(
                        ps[:, j * 4:(j + 1) * 4],
                        lhsT=t_xt[:, (t0 + j) * P:(t0 + j + 1) * P], rhs=t_wt[:],
                        start=True, stop=True)
                nc.scalar.copy(
                    t_at[:, t0 * 4:(t0 + nt) * 4], ps[:, :nt * 4])

            # exp(leaky_relu(z)) = max(exp(z), exp(0.2 z)) — exp monotonic.
            # Pad-slot denominator correction for all tiles at once:
            # corall = padc * max(exp(a_t), exp(0.2 a_t)).
            t_at2 = cpool.tile([P, n_dst_tiles * 4], F32)
            nc.vector.tensor_scalar_mul(t_at2[:], t_at[:], NEG_SLOPE)
            t_ep = cpool.tile([P, n_dst_tiles * 4], F32)
            t_ep2 = cpool.tile([P, n_dst_tiles * 4], F32)
            nc.scalar.activation(
                t_ep[:], t_at[:], mybir.ActivationFunctionType.Exp)
            nc.scalar.activation(
                t_ep2[:], t_at2[:], mybir.ActivationFunctionType.Exp)
            nc.vector.tensor_tensor(
                out=t_ep[:], in0=t_ep[:], in1=t_ep2[:], op=mybir.AluOpType.max)
            t_cor = cpool.tile([P, n_dst_tiles * 4], F32)
            nc.vector.tensor_tensor(
                out=t_cor[:], in0=t_ep[:], in1=t_pc[:], op=mybir.AluOpType.mult)

            # ---- phase B: tiles processed in groups sharing L (the few
            # high-degree tiles run solo; the rest in groups of 4) ----
            ps_q = psB.tile([P, HC], F32, space="PSUM", tag="q")
            off_h = 0
            for (t0, gs) in groups:
                L = L_list[t0]         # shared within group, multiple of 4
                Lh = L // 2
                GL = gs * L            # slots in group
                xe_sb = xpool.tile([2 * in_dim, gs * Lh * P], BF16, tag="xe")
                nc.sync.dma_start(
                    xe_sb[:], d_xe[:, off_h * P:(off_h + gs * Lh) * P])
                off_h += gs * Lh

                # per-edge rows via PE: one K=128 matmul per slot PAIR
                g = gpool.tile([P, GL * ROW], BF16, tag="G")
                for st in range(gs):
                    for h0 in range(0, Lh, GP):
                        nh = min(GP, Lh - h0)
                        ps = psA.tile([P, GP * ROW2], F32, space="PSUM", tag="psa")
                        for j in range(nh):
                            nc.tensor.matmul(
                                ps[:, j * ROW2:(j + 1) * ROW2],
                                lhsT=xe_sb[:, (st * Lh + h0 + j) * P:
                                           (st * Lh + h0 + j + 1) * P],
                                rhs=t_wf[:],
                                start=True, stop=True)
                        nc.scalar.copy(
                            g[:, (st * Lh + h0) * ROW2:
                              (st * Lh + h0 + nh) * ROW2],
                            ps[:, :nh * ROW2])

                g4 = g[:].rearrange("p (s l c) -> p s l c", s=gs, c=ROW)

                # z = a_s + a_t for the whole group  (layout (s, l, h))
                tZ = wpool.tile([P, GL * HEADS], F32, tag="Z")
                Z4 = tZ[:].rearrange("p (s l h) -> p s l h", s=gs, h=HEADS)
                at_b = (t_at[:, t0 * 4:(t0 + gs) * 4]
                        .rearrange("p (s h) -> p s h", h=4)[:, :, 0:HEADS]
                        .unsqueeze(2).to_broadcast((P, gs, L, HEADS)))
                nc.vector.tensor_tensor(
                    out=Z4[:], in0=g4[:, :, :, HC:HC + HEADS], in1=at_b,
                    op=mybir.AluOpType.add)
                tE = wpool.tile([P, GL * HEADS], F32, tag="E")
                tT = wpool.tile([P, GL * HEADS], F32, tag="T")
                nc.scalar.activation(
                    tT[:], tZ[:], mybir.ActivationFunctionType.Exp)
                nc.scalar.activation(
                    tE[:], tZ[:], mybir.ActivationFunctionType.Exp,
                    scale=NEG_SLOPE)
                nc.vector.tensor_tensor(
                    out=tE[:], in0=tE[:], in1=tT[:], op=mybir.AluOpType.max)
                tEb = wpool.tile([P, GL * HEADS], BF16, tag="Eb")
                nc.scalar.copy(tEb[:], tE[:])

                # denominators + pad correction + reciprocal
                # (contiguous per-sub-tile outputs; strided reduce outputs
                # misbehave on HW)
                t_den = wpool.tile([P, 4 * gs], F32, tag="den")
                t_rec = wpool.tile([P, 4 * gs], F32, tag="rec")
                nc.vector.memset(t_den[:], 1.0)
                nc.vector.tensor_reduce(
                    out=t_den[:].rearrange("p (s h) -> p s h", h=4)
                    [:, :, 0:HEADS],
                    in_=tE[:].rearrange("p (s l h) -> p s l h", s=gs, h=HEADS)
                    .transpose([0, 1, 3, 2]),
                    axis=mybir.AxisListType.X, op=mybir.AluOpType.add)
                dv = (t_den[:].rearrange("p (s h) -> p s h", h=4)
                      [:, :, 0:HEADS])
                cview = (t_cor[:, t0 * 4:(t0 + gs) * 4]
                         .rearrange("p (s h) -> p s h", h=4)[:, :, 0:HEADS])
                nc.vector.tensor_tensor(
                    out=dv, in0=dv, in1=cview, op=mybir.AluOpType.subtract)
                nc.vector.tensor_scalar_max(t_den[:], t_den[:], 1e-30)
                nc.vector.reciprocal(t_rec[:], t_den[:])

                # weighted messages M = e * h (bf16 2x; c-major h block);
                # slot-sum via two fold-by-half tree levels + reduce tail
                tM = wpool.tile([P, GL * HC], BF16, tag="M")
                tU = wpool.tile([P, gs * HC], F32, tag="U")
                for st in range(gs):
                    Ms = tM[:, st * L * HC:(st + 1) * L * HC]
                    M4v = Ms.rearrange("p (l c h) -> p l c h", c=CH, h=HEADS)
                    e_b = (tEb[:, st * L * HEADS:(st + 1) * L * HEADS]
                           .rearrange("p (l h) -> p l h", h=HEADS)
                           .unsqueeze(2).to_broadcast((P, L, CH, HEADS)))
                    g3 = g4[:, st, :, 0:HC].rearrange(
                        "p l (c h) -> p l c h", h=HEADS)
                    nc.vector.tensor_tensor(
                        out=M4v[:], in0=g3, in1=e_b, op=mybir.AluOpType.mult)
                    for n in (L // 2, L // 4):
                        a = Ms[:, :2 * n * HC].rearrange(
                            "p (n two c) -> p n two c", two=2, c=HC)
                        nc.vector.tensor_tensor(
                            out=Ms[:, :n * HC].rearrange(
                                "p (n c) -> p n c", c=HC),
                            in0=a[:, :, 0, :], in1=a[:, :, 1, :],
                            op=mybir.AluOpType.add)
                # combined tail reduce over the group: in (s, c, n)
                nc.vector.tensor_reduce(
                    out=tU[:].rearrange("p (s c) -> p s c", s=gs),
                    in_=tM[:].rearrange("p (s q) -> p s q", s=gs)
                    [:, :, :(L // 4) * HC]
                    .rearrange("p s (n c) -> p s n c", c=HC)
                    .transpose([0, 1, 3, 2]),
                    axis=mybir.AxisListType.X, op=mybir.AluOpType.add)

                # V = relu(U / denom + bias); relu on ACT casts to bf16
                tV = wpool.tile([P, gs * HC], F32, tag="V")
                for st in range(gs):
                    rec_b = (t_rec[:, st * 4:st * 4 + HEADS].unsqueeze(1)
                             .to_broadcast((P, CH, HEADS)))
                    nc.vector.tensor_tensor(
                        out=tV[:, st * HC:(st + 1) * HC].rearrange(
                            "p (c h) -> p c h", h=HEADS),
                        in0=tU[:, st * HC:(st + 1) * HC].rearrange(
                            "p (c h) -> p c h", h=HEADS),
                        in1=rec_b, op=mybir.AluOpType.mult)
                bb_b = t_bb[:].unsqueeze(1).to_broadcast((P, gs, HC))
                nc.vector.tensor_tensor(
                    out=tV[:].rearrange("p (s c) -> p s c", s=gs),
                    in0=tV[:].rearrange("p (s c) -> p s c", s=gs),
                    in1=bb_b, op=mybir.AluOpType.add)
                nc.scalar.activation(
                    tV[:], tV[:], mybir.ActivationFunctionType.Relu)

                # pool into batches: q[b, 36] += onehot_t^T @ V, PSUM-accum
                t_oh = wpool.tile([P, gs * P], F32, tag="oh")
                nc.sync.dma_start(t_oh[:], d_oh[:, t0 * P:(t0 + gs) * P])
                for st in range(gs):
                    t = t0 + st
                    nc.tensor.matmul(
                        ps_q[:], lhsT=t_oh[:, st * P:(st + 1) * P],
                        rhs=tV[:, st * HC:(st + 1) * HC],
                        start=(t == 0), stop=(t == n_dst_tiles - 1))

            t_q = cpool.tile([P, HC], F32)
            nc.vector.tensor_copy(t_q[:], ps_q[:])
            nc.sync.dma_start(d_q[:], t_q[:])
    nc.finalize()
    _nc_cache[key] = nc
    return nc


def kernel(**inputs):
    x_s = np.asarray(inputs["x_s"], np.float32)
    x_t = np.asarray(inputs["x_t"], np.float32)
    edge_index = np.asarray(inputs["edge_index"])
    x_s_batch = np.asarray(inputs["x_s_batch"]).astype(np.int64)
    W = np.asarray(inputs["W"], np.float32)
    att_src = np.asarray(inputs["att_src"], np.float32)
    att_dst = np.asarray(inputs["att_dst"], np.float32)
    bias = np.asarray(inputs["bias"], np.float32)
    fc1_w = np.asarray(inputs["fc1_w"], np.float32)
    fc1_b = np.asarray(inputs["fc1_b"], np.float32)
    fc3_w = np.asarray(inputs["fc3_w"], np.float32)
    fc3_b = np.asarray(inputs["fc3_b"], np.float32)

    n_nodes, in_dim = x_s.shape
    src = edge_index[0].astype(np.int64)
    dst = edge_index[1].astype(np.int64)

    # ---- host: edge bucketing by destination (index/layout prep only) ----
    deg = np.bincount(dst, minlength=n_nodes)
    order = np.argsort(-deg, kind="stable")          # nodes by degree desc
    inv_order = np.empty(n_nodes, np.int64)
    inv_order[order] = np.arange(n_nodes)
    nodes_per_core = (n_nodes + N_CORES - 1) // N_CORES
    n_dst_tiles = (nodes_per_core + P - 1) // P
    n_dst_tiles = (n_dst_tiles + 3) // 4 * 4         # whole groups of 4
    L_list = []
    for t in range(n_dst_tiles):
        r0 = t * P * N_CORES
        L = max(4, int(deg[order[min(r0, n_nodes - 1)]]))
        L_list.append((L + 3) // 4 * 4)              # multiple of 4
    k = 0                                            # solo tiles (big L)
    while k < n_dst_tiles and L_list[k] > 44:
        k += 1
    k = min((k + 3) // 4 * 4, n_dst_tiles)
    groups = [(t, 1) for t in range(k)]
    for g in range(k, n_dst_tiles, 4):
        Lg = max(L_list[g:g + 4])                    # shared within group
        for t in range(g, g + 4):
            L_list[t] = Lg
        groups.append((g, 4))
    groups = tuple(groups)
    off_arr = np.concatenate([[0], np.cumsum(L_list)]).astype(np.int64)
    slot_tot = int(off_arr[-1])
    half_tot = slot_tot // 2
    n_xt_cols = n_dst_tiles * P

    # edges sorted by dst -> per-node contiguous src runs
    e_order = np.argsort(dst, kind="stable")
    dst_sorted = dst[e_order]
    src_sorted = src[e_order].astype(np.int64)
    starts = np.searchsorted(dst_sorted, np.arange(n_nodes))
    slot_within = np.arange(len(dst_sorted)) - starts[dst_sorted]

    k_global = inv_order[dst_sorted]
    core_of = (k_global % N_CORES).astype(np.int64)
    k_local = k_global // N_CORES
    t_of = k_local // P
    p_of = k_local % P
    col_of = off_arr[t_of] + slot_within

    # fold weights (host weight prep).  W/bias/w2 columns permuted c-major:
    # folded col (c*HEADS + h) <- original col (h*CH + c).
    cm = np.array([h * CH + c for c in range(CH) for h in range(HEADS)])
    W_cm = W[:, cm]
    bias_cm = bias[cm]
    w2_cm = (fc1_w @ fc3_w)[:, 0].astype(np.float32)[cm]

    wa_t = np.einsum("khc,hc->kh", W.reshape(in_dim, HEADS, CH), att_dst)
    wa_s = np.einsum("khc,hc->kh", W.reshape(in_dim, HEADS, CH), att_src)
    wfold = np.zeros((in_dim, ROW), np.float32)
    wfold[:, :HC] = W_cm
    wfold[:, HC:HC + HEADS] = wa_s
    wfbd = np.zeros((2 * in_dim, ROW2), np.float32)
    wfbd[:in_dim, :ROW] = wfold
    wfbd[in_dim:, ROW:] = wfold
    wfbd = wfbd.astype(ml_dtypes.bfloat16)
    wat = np.zeros((in_dim, 4), np.float32)
    wat[:, :HEADS] = wa_t
    biasb = np.tile(bias_cm[None, :], (P, 1)).astype(np.float32)

    xsb_ext = np.zeros((n_nodes + 1, in_dim), ml_dtypes.bfloat16)
    xsb_ext[:n_nodes] = x_s.astype(ml_dtypes.bfloat16)
    SENT = n_nodes

    in_maps = []
    cnts = []
    for c in range(N_CORES):
        node_ids = order[c::N_CORES]                 # this core's dst nodes
        ncnt = len(node_ids)
        m = core_of == c
        SRC = np.full((P, slot_tot), SENT, np.int64)
        SRC[p_of[m], col_of[m]] = src_sorted[m]

        # per-edge lhsT layout: rows 0:64 even slots, 64:128 odd slots
        xe = np.empty((2 * in_dim, half_tot * P), ml_dtypes.bfloat16)
        for par in range(2):
            S = SRC[:, par::2]                       # [P, half_tot]
            blk = xsb_ext[S]                         # [P, half_tot, in_dim]
            xe[par * in_dim:(par + 1) * in_dim] = (
                blk.transpose(2, 1, 0).reshape(in_dim, half_tot * P))

        padc4 = np.zeros((P, n_dst_tiles * 4), np.float32)
        oh = np.zeros((P, n_dst_tiles * P), np.float32)
        xt_t = np.zeros((in_dim, n_xt_cols), np.float32)
        kk = np.arange(n_dst_tiles * P)
        tt, pp = kk // P, kk % P
        present = kk < ncnt
        nid = np.where(present, node_ids[np.minimum(kk, ncnt - 1)], 0)
        Leff = np.asarray(L_list, np.float32)[tt]
        pc = np.where(present, Leff - deg[nid], Leff)
        for j in range(4):
            padc4[pp, 4 * tt + j] = pc
        bid = x_s_batch[nid]
        oh[pp[present], tt[present] * P + bid[present]] = 1.0
        cnts.append(np.bincount(bid[present], minlength=P).astype(np.float64))
        xt_t[:, :ncnt] = x_t[node_ids].T
        in_maps.append({
            "xe": xe, "xt_t": xt_t, "padc4": padc4, "oh": oh, "wfbd": wfbd,
            "wat": wat, "biasb": biasb,
        })

    nc = _build_nc(in_dim, n_dst_tiles, L_list, half_tot, n_xt_cols, groups)
    res = run_bass_kernel_spmd(nc, in_maps, core_ids=list(range(N_CORES)))

    q = np.zeros((P, HC), np.float64)
    cnt = np.zeros(P, np.float64)
    for c in range(N_CORES):
        q += res.results[c]["q_out"]
        cnt += cnts[c]
    num = q @ w2_cm.astype(np.float64)
    out = num / np.maximum(cnt, 1.0)
    const = float(fc1_b @ fc3_w[:, 0] + fc3_b[0])
    return (out + const).astype(np.float32)


# revision 36
# speedup vs baseline: 13.0803x; 1.0205x over previous
"""GAT (bipartite GATConv + mean-pool + 2 FC) on 8 Trainium2 NeuronCores.

Strategy: edges are sharded per destination node; destination nodes are
dealt round-robin (degree-sorted) across the 8 cores so the segment softmax
is fully local to a core.  Per the sharding hint each device holds its edge
shard with the source-node features replicated into matmul-ready per-edge
layout (host does only index manipulation / np.take layout; every model
FLOP runs on device):

  Phase A2: a_t = x_t @ (W att_dst) for this core's dst nodes (PE).
  Phase B: dst nodes are processed in tiles of 128 (one node per partition,
           nodes degree-sorted so tiles have uniform run lengths L).  The
           per-edge source features arrive as bf16 lhsT tiles [128, L/2*128]
           (slot pair 2j/2j+1 stacked as two K=64 halves); one K=128 matmul
           against a block-diagonal [wfold|0 / 0|wfold] rhs computes BOTH
           slots' rows [h_s (36, c-major) | a_s (3) | pad] into PSUM.  ACT
           casts them to bf16 SBUF; E = exp(leaky_relu(a_s+a_t)) =
           max(exp(z), exp(0.2 z)) via two ACT Exp passes over a DVE-added
           z.  The weighted message sum runs in bf16 on DVE's packed 2x
           path: W columns are stored c-major/h-minor so the E broadcast's
           innermost dim is the packed head dim, and the slot reduction is
           two packed tree-add levels plus a short tensor_reduce tail.
           Batch pooling is a PE matmul against host-shipped one-hot
           columns, accumulated over all tiles in PSUM; the final fc1@fc3
           contraction and count division happen on host partials.
           Pad slots carry x=0 => h=0, a_s=0; their exp(leaky_relu(a_t))
           denominator contribution is subtracted exactly via a
           host-precomputed pad-count correction.

Device-side per-edge gathers are avoided entirely: one [P,1]-offset
indirect-DMA gather costs ~1.1us of SWDGE descriptor generation on the Pool
engine (994ns fixed + 0.34ns/desc, 128 descriptors max per instruction) and
the batched-gather ucode (InstDMAGatherAnt etc.) is excluded from bedrock
images, so any gather-based design is floored at ~3.3ms/core.  Sequential
streaming of the pre-laid-out edge shard runs at DMA bandwidth instead.

HW pitfalls (probed): matmuls that switch PE row groups (partition-offset
lhsT/rhs) within one PSUM tile crash the device (the K=128 block-diagonal
formulation sidesteps row groups); Pool-engine TensorTensor is rejected by
this lowering; softmax denominators accumulated from bf16 exps lose ~6x
final accuracy (E stays f32; bf16 is cast only for the message multiply).
"""

import numpy as np
import ml_dtypes

import concourse.bacc as bacc
import concourse.tile as tile
from concourse import mybir
from concourse.bass_utils import run_bass_kernel_spmd

F32 = mybir.dt.float32
BF16 = mybir.dt.bfloat16

N_CORES = 8
P = 128
HEADS = 3
CH = 12
HC = HEADS * CH          # 36
ROW = HC + 4             # matmul output row: 36 h | 3 a_s | 1 pad = 40
ROW2 = 2 * ROW           # block-diagonal pair output
NEG_SLOPE = 0.2
GP = 6                   # slot pairs per PSUM tile (6*80 = 480 f32 <= 512)

_nc_cache = {}


def _build_nc(in_dim, n_dst_tiles, L_list, half_tot, n_xt_cols, groups):
    key = (in_dim, n_dst_tiles, tuple(L_list), half_tot, n_xt_cols, tuple(groups))
    if key in _nc_cache:
        return _nc_cache[key]

    nc = bacc.Bacc("TRN2", target_bir_lowering=False, debug=False)
    d_xe = nc.dram_tensor("xe", [2 * in_dim, half_tot * P], BF16, kind="ExternalInput")
    d_xt = nc.dram_tensor("xt_t", [in_dim, n_xt_cols], F32, kind="ExternalInput")
    d_pc = nc.dram_tensor("padc4", [P, n_dst_tiles * 4], F32, kind="ExternalInput")
    d_oh = nc.dram_tensor("oh", [P, n_dst_tiles * P], F32, kind="ExternalInput")
    d_wf = nc.dram_tensor("wfbd", [2 * in_dim, ROW2], BF16, kind="ExternalInput")
    d_wt = nc.dram_tensor("wat", [in_dim, 4], F32, kind="ExternalInput")
    d_bb = nc.dram_tensor("biasb", [P, HC], F32, kind="ExternalInput")
    d_q = nc.dram_tensor("q_out", [P, HC], F32, kind="ExternalOutput")

    with tile.TileContext(nc) as tc:
        with tc.tile_pool(name="const", bufs=1) as cpool, \
             tc.tile_pool(name="xload", bufs=2) as xpool, \
             tc.tile_pool(name="gat", bufs=3) as gpool, \
             tc.tile_pool(name="work", bufs=2) as wpool, \
             tc.tile_pool(name="psA", bufs=4, space="PSUM") as psA, \
             tc.tile_pool(name="psB", bufs=1, space="PSUM") as psB, \
             tc.tile_pool(name="psT", bufs=2, space="PSUM") as psT:

            # ---- constants into SBUF ----
            t_wf = cpool.tile([2 * in_dim, ROW2], BF16)
            nc.sync.dma_start(t_wf[:], d_wf[:])
            t_wt = cpool.tile([in_dim, 4], F32)
            nc.sync.dma_start(t_wt[:], d_wt[:])
            t_bb = cpool.tile([P, HC], F32)
            nc.sync.dma_start(t_bb[:], d_bb[:])
            t_pc = cpool.tile([P, n_dst_tiles * 4], F32)
            nc.sync.dma_start(t_pc[:], d_pc[:])
            t_xt = cpool.tile([in_dim, n_xt_cols], F32)
            nc.sync.dma_start(t_xt[:], d_xt[:])

            # ---- phase A2: a_t per dst tile -> resident SBUF ----
            t_at = cpool.tile([P, n_dst_tiles * 4], F32)
            for t0 in range(0, n_dst_tiles, 4):
                nt = min(4, n_dst_tiles - t0)
                ps = psT.tile([P, 16], F32, space="PSUM", tag="psat")
                for j in range(nt):
                    nc.tensor.matmul(
                        ps[:, j * 4:(j + 1) * 4],
                        lhsT=t_xt[:, (t0 + j) * P:(t0 + j + 1) * P], rhs=t_wt[:],
                        start=True, stop=True)
                nc.scalar.copy(
                    t_at[:, t0 * 4:(t0 + nt) * 4], ps[:, :nt * 4])

            # exp(leaky_relu(z)) = max(exp(z), exp(0.2 z)) — exp monotonic.
            # Pad-slot denominator correction for all tiles at once:
            # corall = padc * max(exp(a_t), exp(0.2 a_t)).
            t_at2 = cpool.tile([P, n_dst_tiles * 4], F32)
            nc.vector.tensor_scalar_mul(t_at2[:], t_at[:], NEG_SLOPE)
            t_ep = cpool.tile([P, n_dst_tiles * 4], F32)
            t_ep2 = cpool.tile([P, n_dst_tiles * 4], F32)
            nc.scalar.activation(
                t_ep[:], t_at[:], mybir.ActivationFunctionType.Exp)
            nc.scalar.activation(
                t_ep2[:], t_at2[:], mybir.ActivationFunctionType.Exp)
            nc.vector.tensor_tensor(
                out=t_ep[:], in0=t_ep[:], in1=t_ep2[:], op=mybir.AluOpType.max)
            t_cor = cpool.tile([P, n_dst_tiles * 4], F32)
            nc.vector.tensor_tensor(
                out=t_cor[:], in0=t_ep[:], in1=t_pc[:], op=mybir.AluOpType.mult)

            # ---- phase B: tiles processed in groups sharing L (the few
            # high-degree tiles run solo; the rest in groups of 4) ----
            ps_q = psB.tile([P, HC], F32, space="PSUM", tag="q")
            off_h = 0
            for (t0, gs) in groups:
                L = L_list[t0]         # shared within group, multiple of 4
                Lh = L // 2
                GL = gs * L            # slots in group
                xe_sb = xpool.tile([2 * in_dim, gs * Lh * P], BF16, tag="xe")
                nc.sync.dma_start(
                    xe_sb[:], d_xe[:, off_h * P:(off_h + gs * Lh) * P])
                off_h += gs * Lh

                # per-edge rows via PE: one K=128 matmul per slot PAIR
                g = gpool.tile([P, GL * ROW], BF16, tag="G")
                for st in range(gs):
                    for h0 in range(0, Lh, GP):
                        nh = min(GP, Lh - h0)
                        ps = psA.tile([P, GP * ROW2], F32, space="PSUM", tag="psa")
                        for j in range(nh):
                            nc.tensor.matmul(
                                ps[:, j * ROW2:(j + 1) * ROW2],
                                lhsT=xe_sb[:, (st * Lh + h0 + j) * P:
                                           (st * Lh + h0 + j + 1) * P],
                                rhs=t_wf[:],
                                start=True, stop=True)
                        nc.scalar.copy(
                            g[:, (st * Lh + h0) * ROW2:
                              (st * Lh + h0 + nh) * ROW2],
                            ps[:, :nh * ROW2])

                g4 = g[:].rearrange("p (s l c) -> p s l c", s=gs, c=ROW)

                # z = a_s + a_t for the whole group  (layout (s, l, h))
                tZ = wpool.tile([P, GL * HEADS], F32, tag="Z")
                Z4 = tZ[:].rearrange("p (s l h) -> p s l h", s=gs, h=HEADS)
                at_b = (t_at[:, t0 * 4:(t0 + gs) * 4]
                        .rearrange("p (s h) -> p s h", h=4)[:, :, 0:HEADS]
                        .unsqueeze(2).to_broadcast((P, gs, L, HEADS)))
                nc.vector.tensor_tensor(
                    out=Z4[:], in0=g4[:, :, :, HC:HC + HEADS], in1=at_b,
                    op=mybir.AluOpType.add)
                tE = wpool.tile([P, GL * HEADS], F32, tag="E")
                tT = wpool.tile([P, GL * HEADS], F32, tag="T")
                nc.scalar.activation(
                    tT[:], tZ[:], mybir.ActivationFunctionType.Exp)
                nc.scalar.activation(
                    tE[:], tZ[:], mybir.ActivationFunctionType.Exp,
                    scale=NEG_SLOPE)
                nc.vector.tensor_tensor(
                    out=tE[:], in0=tE[:], in1=tT[:], op=mybir.AluOpType.max)
                tEb = wpool.tile([P, GL * HEADS], BF16, tag="Eb")
                nc.scalar.copy(tEb[:], tE[:])

                # denominators + pad correction + reciprocal
                # (contiguous per-sub-tile outputs; strided reduce outputs
                # misbehave on HW)
                t_den = wpool.tile([P, 4 * gs], F32, tag="den")
                t_rec = wpool.tile([P, 4 * gs], F32, tag="rec")
                nc.vector.memset(t_den[:], 1.0)
                nc.vector.tensor_reduce(
                    out=t_den[:].rearrange("p (s h) -> p s h", h=4)
                    [:, :, 0:HEADS],
                    in_=tE[:].rearrange("p (s l h) -> p s l h", s=gs, h=HEADS)
                    .transpose([0, 1, 3, 2]),
                    axis=mybir.AxisListType.X, op=mybir.AluOpType.add)
                dv = (t_den[:].rearrange("p (s h) -> p s h", h=4)
                      [:, :, 0:HEADS])
                cview = (t_cor[:, t0 * 4:(t0 + gs) * 4]
                         .rearrange("p (s h) -> p s h", h=4)[:, :, 0:HEADS])
                nc.vector.tensor_tensor(
                    out=dv, in0=dv, in1=cview, op=mybir.AluOpType.subtract)
                nc.vector.tensor_scalar_max(t_den[:], t_den[:], 1e-30)
                nc.vector.reciprocal(t_rec[:], t_den[:])

                # weighted messages M = e * h (bf16; c-major h block),
                # one multiply for the whole group ((s,l) merged), then two
                # group-wide pairwise tree-add levels + one reduce tail
                tM = wpool.tile([P, GL * HC], BF16, tag="M")
                tU = wpool.tile([P, gs * HC], F32, tag="U")
                M4v = tM[:].rearrange("p (q c h) -> p q c h", c=CH, h=HEADS)
                e_b = (tEb[:].rearrange("p (q h) -> p q h", h=HEADS)
                       .unsqueeze(2).to_broadcast((P, GL, CH, HEADS)))
                gh = (g[:].rearrange("p (q c) -> p q c", c=ROW)[:, :, 0:HC]
                      .rearrange("p q (c h) -> p q c h", h=HEADS))
                nc.vector.tensor_tensor(
                    out=M4v[:], in0=gh, in1=e_b, op=mybir.AluOpType.mult)
                for n in (L // 2, L // 4):
                    sv = tM[:].rearrange("p (s q) -> p s q", s=gs)
                    src = (sv[:, :, :2 * n * HC]
                           .rearrange("p s (n two c) -> p s n two c",
                                      two=2, c=HC))
                    dstv = (sv[:, :, :n * HC]
                            .rearrange("p s (n c) -> p s n c", c=HC))
                    nc.vector.tensor_tensor(
                        out=dstv, in0=src[:, :, :, 0, :],
                        in1=src[:, :, :, 1, :], op=mybir.AluOpType.add)
                nc.vector.tensor_reduce(
                    out=tU[:].rearrange("p (s c) -> p s c", s=gs),
                    in_=tM[:].rearrange("p (s q) -> p s q", s=gs)
                    [:, :, :(L // 4) * HC]
                    .rearrange("p s (n c) -> p s n c", c=HC)
                    .transpose([0, 1, 3, 2]),
                    axis=mybir.AxisListType.X, op=mybir.AluOpType.add)

                # V = relu(U / denom + bias); relu on ACT casts to bf16
                tV = wpool.tile([P, gs * HC], F32, tag="V")
                rec_b = (t_rec[:].rearrange("p (s h) -> p s h", h=4)
                         [:, :, 0:HEADS].unsqueeze(2)
                         .to_broadcast((P, gs, CH, HEADS)))
                nc.vector.tensor_tensor(
                    out=tV[:].rearrange("p (s c h) -> p s c h", c=CH, h=HEADS),
                    in0=tU[:].rearrange("p (s c h) -> p s c h", c=CH, h=HEADS),
                    in1=rec_b, op=mybir.AluOpType.mult)
                bb_b = t_bb[:].unsqueeze(1).to_broadcast((P, gs, HC))
                nc.vector.tensor_tensor(
                    out=tV[:].rearrange("p (s c) -> p s c", s=gs),
                    in0=tV[:].rearrange("p (s c) -> p s c", s=gs),
                    in1=bb_b, op=mybir.AluOpType.add)
                nc.scalar.activation(
                    tV[:], tV[:], mybir.ActivationFunctionType.Relu)

                # pool into batches: q[b, 36] += onehot_t^T @ V, PSUM-accum
                t_oh = wpool.tile([P, gs * P], F32, tag="oh")
                nc.sync.dma_start(t_oh[:], d_oh[:, t0 * P:(t0 + gs) * P])
                for st in range(gs):
                    t = t0 + st
                    nc.tensor.matmul(
                        ps_q[:], lhsT=t_oh[:, st * P:(st + 1) * P],
                        rhs=tV[:, st * HC:(st + 1) * HC],
                        start=(t == 0), stop=(t == n_dst_tiles - 1))

            t_q = cpool.tile([P, HC], F32)
            nc.vector.tensor_copy(t_q[:], ps_q[:])
            nc.sync.dma_start(d_q[:], t_q[:])
    nc.finalize()
    _nc_cache[key] = nc
    return nc


def kernel(**inputs):
    x_s = np.asarray(inputs["x_s"], np.float32)
    x_t = np.asarray(inputs["x_t"], np.float32)
    edge_index = np.asarray(inputs["edge_index"])
    x_s_batch = np.asarray(inputs["x_s_batch"]).astype(np.int64)
    W = np.asarray(inputs["W"], np.float32)
    att_src = np.asarray(inputs["att_src"], np.float32)
    att_dst = np.asarray(inputs["att_dst"], np.float32)
    bias = np.asarray(inputs["bias"], np.float32)
    fc1_w = np.asarray(inputs["fc1_w"], np.float32)
    fc1_b = np.asarray(inputs["fc1_b"], np.float32)
    fc3_w = np.asarray(inputs["fc3_w"], np.float32)
    fc3_b = np.asarray(inputs["fc3_b"], np.float32)

    n_nodes, in_dim = x_s.shape
    src = edge_index[0].astype(np.int64)
    dst = edge_index[1].astype(np.int64)

    # ---- host: edge bucketing by destination (index/layout prep only) ----
    deg = np.bincount(dst, minlength=n_nodes)
    order = np.argsort(-deg, kind="stable")          # nodes by degree desc
    inv_order = np.empty(n_nodes, np.int64)
    inv_order[order] = np.arange(n_nodes)
    nodes_per_core = (n_nodes + N_CORES - 1) // N_CORES
    n_dst_tiles = (nodes_per_core + P - 1) // P
    n_dst_tiles = (n_dst_tiles + 3) // 4 * 4         # whole groups of 4
    L_list = []
    for t in range(n_dst_tiles):
        r0 = t * P * N_CORES
        L = max(4, int(deg[order[min(r0, n_nodes - 1)]]))
        L_list.append((L + 3) // 4 * 4)              # multiple of 4
    k = 0                                            # solo tiles (big L)
    while k < n_dst_tiles and L_list[k] > 44:
        k += 1
    k = min((k + 3) // 4 * 4, n_dst_tiles)
    groups = [(t, 1) for t in range(k)]
    for g in range(k, n_dst_tiles, 4):
        Lg = max(L_list[g:g + 4])                    # shared within group
        for t in range(g, g + 4):
            L_list[t] = Lg
        groups.append((g, 4))
    groups = tuple(groups)
    off_arr = np.concatenate([[0], np.cumsum(L_list)]).astype(np.int64)
    slot_tot = int(off_arr[-1])
    half_tot = slot_tot // 2
    n_xt_cols = n_dst_tiles * P

    # edges sorted by dst -> per-node contiguous src runs
    e_order = np.argsort(dst, kind="stable")
    dst_sorted = dst[e_order]
    src_sorted = src[e_order].astype(np.int64)
    starts = np.searchsorted(dst_sorted, np.arange(n_nodes))
    slot_within = np.arange(len(dst_sorted)) - starts[dst_sorted]

    k_global = inv_order[dst_sorted]
    core_of = (k_global % N_CORES).astype(np.int64)
    k_local = k_global // N_CORES
    t_of = k_local // P
    p_of = k_local % P
    col_of = off_arr[t_of] + slot_within

    # fold weights (host weight prep).  W/bias/w2 columns permuted c-major:
    # folded col (c*HEADS + h) <- original col (h*CH + c).
    cm = np.array([h * CH + c for c in range(CH) for h in range(HEADS)])
    W_cm = W[:, cm]
    bias_cm = bias[cm]
    w2_cm = (fc1_w @ fc3_w)[:, 0].astype(np.float32)[cm]

    wa_t = np.einsum("khc,hc->kh", W.reshape(in_dim, HEADS, CH), att_dst)
    wa_s = np.einsum("khc,hc->kh", W.reshape(in_dim, HEADS, CH), att_src)
    wfold = np.zeros((in_dim, ROW), np.float32)
    wfold[:, :HC] = W_cm
    wfold[:, HC:HC + HEADS] = wa_s
    wfbd = np.zeros((2 * in_dim, ROW2), np.float32)
    wfbd[:in_dim, :ROW] = wfold
    wfbd[in_dim:, ROW:] = wfold
    wfbd = wfbd.astype(ml_dtypes.bfloat16)
    wat = np.zeros((in_dim, 4), np.float32)
    wat[:, :HEADS] = wa_t
    biasb = np.tile(bias_cm[None, :], (P, 1)).astype(np.float32)

    xsb_ext = np.zeros((n_nodes + 1, in_dim), ml_dtypes.bfloat16)
    xsb_ext[:n_nodes] = x_s.astype(ml_dtypes.bfloat16)
    SENT = n_nodes

    in_maps = []
    cnts = []
    for c in range(N_CORES):
        node_ids = order[c::N_CORES]                 # this core's dst nodes
        ncnt = len(node_ids)
        m = core_of == c
        SRC = np.full((P, slot_tot), SENT, np.int64)
        SRC[p_of[m], col_of[m]] = src_sorted[m]

        # per-edge lhsT layout: rows 0:64 even slots, 64:128 odd slots
        xe = np.empty((2 * in_dim, half_tot * P), ml_dtypes.bfloat16)
        for par in range(2):
            S = SRC[:, par::2]                       # [P, half_tot]
            blk = xsb_ext[S]                         # [P, half_tot, in_dim]
            xe[par * in_dim:(par + 1) * in_dim] = (
                blk.transpose(2, 1, 0).reshape(in_dim, half_tot * P))

        padc4 = np.zeros((P, n_dst_tiles * 4), np.float32)
        oh = np.zeros((P, n_dst_tiles * P), np.float32)
        xt_t = np.zeros((in_dim, n_xt_cols), np.float32)
        kk = np.arange(n_dst_tiles * P)
        tt, pp = kk // P, kk % P
        present = kk < ncnt
        nid = np.where(present, node_ids[np.minimum(kk, ncnt - 1)], 0)
        Leff = np.asarray(L_list, np.float32)[tt]
        pc = np.where(present, Leff - deg[nid], Leff)
        for j in range(4):
            padc4[pp, 4 * tt + j] = pc
        bid = x_s_batch[nid]
        oh[pp[present], tt[present] * P + bid[present]] = 1.0
        cnts.append(np.bincount(bid[present], minlength=P).astype(np.float64))
        xt_t[:, :ncnt] = x_t[node_ids].T
        in_maps.append({
            "xe": xe, "xt_t": xt_t, "padc4": padc4, "oh": oh, "wfbd": wfbd,
            "wat": wat, "biasb": biasb,
        })

    nc = _build_nc(in_dim, n_dst_tiles, L_list, half_tot, n_xt_cols, groups)
    res = run_bass_kernel_spmd(nc, in_maps, core_ids=list(range(N_CORES)))

    q = np.zeros((P, HC), np.float64)
    cnt = np.zeros(P, np.float64)
    for c in range(N_CORES):
        q += res.results[c]["q_out"]
        cnt += cnts[c]
    num = q @ w2_cm.astype(np.float64)
    out = num / np.maximum(cnt, 1.0)
    const = float(fc1_b @ fc3_w[:, 0] + fc3_b[0])
    return (out + const).astype(np.float32)


# revision 37
# speedup vs baseline: 13.1242x; 1.0034x over previous
"""GAT (bipartite GATConv + mean-pool + 2 FC) on 8 Trainium2 NeuronCores.

Strategy: edges are sharded per destination node; destination nodes are
dealt round-robin (degree-sorted) across the 8 cores so the segment softmax
is fully local to a core.  Per the sharding hint each device holds its edge
shard with the source-node features replicated into matmul-ready per-edge
layout (host does only index manipulation / np.take layout; every model
FLOP runs on device):

  Phase A2: a_t = x_t @ (W att_dst) for this core's dst nodes (PE).
  Phase B: dst nodes are processed in tiles of 128 (one node per partition,
           nodes degree-sorted so tiles have uniform run lengths L).  The
           per-edge source features arrive as bf16 lhsT tiles [128, L/2*128]
           (slot pair 2j/2j+1 stacked as two K=64 halves); one K=128 matmul
           against a block-diagonal [wfold|0 / 0|wfold] rhs computes BOTH
           slots' rows [h_s (36, c-major) | a_s (3) | pad] into PSUM.  ACT
           casts them to bf16 SBUF; E = exp(leaky_relu(a_s+a_t)) =
           max(exp(z), exp(0.2 z)) via two ACT Exp passes over a DVE-added
           z.  The weighted message sum runs in bf16 on DVE's packed 2x
           path: W columns are stored c-major/h-minor so the E broadcast's
           innermost dim is the packed head dim, and the slot reduction is
           two packed tree-add levels plus a short tensor_reduce tail.
           Batch pooling is a PE matmul against host-shipped one-hot
           columns, accumulated over all tiles in PSUM; the final fc1@fc3
           contraction and count division happen on host partials.
           Pad slots carry x=0 => h=0, a_s=0; their exp(leaky_relu(a_t))
           denominator contribution is subtracted exactly via a
           host-precomputed pad-count correction.

Device-side per-edge gathers are avoided entirely: one [P,1]-offset
indirect-DMA gather costs ~1.1us of SWDGE descriptor generation on the Pool
engine (994ns fixed + 0.34ns/desc, 128 descriptors max per instruction) and
the batched-gather ucode (InstDMAGatherAnt etc.) is excluded from bedrock
images, so any gather-based design is floored at ~3.3ms/core.  Sequential
streaming of the pre-laid-out edge shard runs at DMA bandwidth instead.

HW pitfalls (probed): matmuls that switch PE row groups (partition-offset
lhsT/rhs) within one PSUM tile crash the device (the K=128 block-diagonal
formulation sidesteps row groups); Pool-engine TensorTensor is rejected by
this lowering; softmax denominators accumulated from bf16 exps lose ~6x
final accuracy (E stays f32; bf16 is cast only for the message multiply).
"""

import numpy as np
import ml_dtypes

import concourse.bacc as bacc
import concourse.tile as tile
from concourse import mybir
from concourse.bass_utils import run_bass_kernel_spmd

F32 = mybir.dt.float32
BF16 = mybir.dt.bfloat16

N_CORES = 8
P = 128
HEADS = 3
CH = 12
HC = HEADS * CH          # 36
ROW = HC + 4             # matmul output row: 36 h | 3 a_s | 1 pad = 40
ROW2 = 2 * ROW           # block-diagonal pair output
NEG_SLOPE = 0.2
GP = 6                   # slot pairs per PSUM tile (6*80 = 480 f32 <= 512)

_nc_cache = {}


def _build_nc(in_dim, n_dst_tiles, L_list, half_tot, n_xt_cols, groups):
    key = (in_dim, n_dst_tiles, tuple(L_list), half_tot, n_xt_cols, tuple(groups))
    if key in _nc_cache:
        return _nc_cache[key]

    nc = bacc.Bacc("TRN2", target_bir_lowering=False, debug=False)
    d_xe = nc.dram_tensor("xe", [2 * in_dim, half_tot * P], BF16, kind="ExternalInput")
    d_xt = nc.dram_tensor("xt_t", [in_dim, n_xt_cols], F32, kind="ExternalInput")
    d_pc = nc.dram_tensor("padc4", [P, n_dst_tiles * 4], F32, kind="ExternalInput")
    d_oh = nc.dram_tensor("oh", [P, n_dst_tiles * P], F32, kind="ExternalInput")
    d_wf = nc.dram_tensor("wfbd", [2 * in_dim, ROW2], BF16, kind="ExternalInput")
    d_wt = nc.dram_tensor("wat", [in_dim, 4], F32, kind="ExternalInput")
    d_bb = nc.dram_tensor("biasb", [P, HC], F32, kind="ExternalInput")
    d_q = nc.dram_tensor("q_out", [P, HC], F32, kind="ExternalOutput")

    with tile.TileContext(nc) as tc:
        with tc.tile_pool(name="const", bufs=1) as cpool, \
             tc.tile_pool(name="xload", bufs=2) as xpool, \
             tc.tile_pool(name="gat", bufs=3) as gpool, \
             tc.tile_pool(name="work", bufs=3) as wpool, \
             tc.tile_pool(name="msg", bufs=2) as mpool, \
             tc.tile_pool(name="psA", bufs=4, space="PSUM") as psA, \
             tc.tile_pool(name="psB", bufs=1, space="PSUM") as psB, \
             tc.tile_pool(name="psT", bufs=2, space="PSUM") as psT:

            # ---- constants into SBUF ----
            t_wf = cpool.tile([2 * in_dim, ROW2], BF16)
            nc.sync.dma_start(t_wf[:], d_wf[:])
            t_wt = cpool.tile([in_dim, 4], F32)
            nc.sync.dma_start(t_wt[:], d_wt[:])
            t_bb = cpool.tile([P, HC], F32)
            nc.sync.dma_start(t_bb[:], d_bb[:])
            t_pc = cpool.tile([P, n_dst_tiles * 4], F32)
            nc.sync.dma_start(t_pc[:], d_pc[:])
            t_xt = cpool.tile([in_dim, n_xt_cols], F32)
            nc.sync.dma_start(t_xt[:], d_xt[:])

            # ---- phase A2: a_t per dst tile -> resident SBUF ----
            t_at = cpool.tile([P, n_dst_tiles * 4], F32)
            for t0 in range(0, n_dst_tiles, 4):
                nt = min(4, n_dst_tiles - t0)
                ps = psT.tile([P, 16], F32, space="PSUM", tag="psat")
                for j in range(nt):
                    nc.tensor.matmul(
                        ps[:, j * 4:(j + 1) * 4],
                        lhsT=t_xt[:, (t0 + j) * P:(t0 + j + 1) * P], rhs=t_wt[:],
                        start=True, stop=True)
                nc.scalar.copy(
                    t_at[:, t0 * 4:(t0 + nt) * 4], ps[:, :nt * 4])

            # exp(leaky_relu(z)) = max(exp(z), exp(0.2 z)) — exp monotonic.
            # Pad-slot denominator correction for all tiles at once:
            # corall = padc * max(exp(a_t), exp(0.2 a_t)).
            t_at2 = cpool.tile([P, n_dst_tiles * 4], F32)
            nc.vector.tensor_scalar_mul(t_at2[:], t_at[:], NEG_SLOPE)
            t_ep = cpool.tile([P, n_dst_tiles * 4], F32)
            t_ep2 = cpool.tile([P, n_dst_tiles * 4], F32)
            nc.scalar.activation(
                t_ep[:], t_at[:], mybir.ActivationFunctionType.Exp)
            nc.scalar.activation(
                t_ep2[:], t_at2[:], mybir.ActivationFunctionType.Exp)
            nc.vector.tensor_tensor(
                out=t_ep[:], in0=t_ep[:], in1=t_ep2[:], op=mybir.AluOpType.max)
            t_cor = cpool.tile([P, n_dst_tiles * 4], F32)
            nc.vector.tensor_tensor(
                out=t_cor[:], in0=t_ep[:], in1=t_pc[:], op=mybir.AluOpType.mult)

            # ---- phase B: tiles processed in groups sharing L (the few
            # high-degree tiles run solo; the rest in groups of 4) ----
            ps_q = psB.tile([P, HC], F32, space="PSUM", tag="q")
            off_h = 0
            for (t0, gs) in groups:
                L = L_list[t0]         # shared within group, multiple of 4
                Lh = L // 2
                GL = gs * L            # slots in group
                xe_sb = xpool.tile([2 * in_dim, gs * Lh * P], BF16, tag="xe")
                nc.sync.dma_start(
                    xe_sb[:], d_xe[:, off_h * P:(off_h + gs * Lh) * P])
                off_h += gs * Lh

                # per-edge rows via PE: one K=128 matmul per slot PAIR
                g = gpool.tile([P, GL * ROW], BF16, tag="G")
                for st in range(gs):
                    for h0 in range(0, Lh, GP):
                        nh = min(GP, Lh - h0)
                        ps = psA.tile([P, GP * ROW2], F32, space="PSUM", tag="psa")
                        for j in range(nh):
                            nc.tensor.matmul(
                                ps[:, j * ROW2:(j + 1) * ROW2],
                                lhsT=xe_sb[:, (st * Lh + h0 + j) * P:
                                           (st * Lh + h0 + j + 1) * P],
                                rhs=t_wf[:],
                                start=True, stop=True)
                        nc.scalar.copy(
                            g[:, (st * Lh + h0) * ROW2:
                              (st * Lh + h0 + nh) * ROW2],
                            ps[:, :nh * ROW2])

                g4 = g[:].rearrange("p (s l c) -> p s l c", s=gs, c=ROW)

                # z = a_s + a_t for the whole group  (layout (s, l, h))
                tZ = wpool.tile([P, GL * HEADS], F32, tag="Z")
                Z4 = tZ[:].rearrange("p (s l h) -> p s l h", s=gs, h=HEADS)
                at_b = (t_at[:, t0 * 4:(t0 + gs) * 4]
                        .rearrange("p (s h) -> p s h", h=4)[:, :, 0:HEADS]
                        .unsqueeze(2).to_broadcast((P, gs, L, HEADS)))
                nc.vector.tensor_tensor(
                    out=Z4[:], in0=g4[:, :, :, HC:HC + HEADS], in1=at_b,
                    op=mybir.AluOpType.add)
                tE = wpool.tile([P, GL * HEADS], F32, tag="E")
                tT = wpool.tile([P, GL * HEADS], F32, tag="T")
                nc.scalar.activation(
                    tT[:], tZ[:], mybir.ActivationFunctionType.Exp)
                nc.scalar.activation(
                    tE[:], tZ[:], mybir.ActivationFunctionType.Exp,
                    scale=NEG_SLOPE)
                nc.vector.tensor_tensor(
                    out=tE[:], in0=tE[:], in1=tT[:], op=mybir.AluOpType.max)
                tEb = wpool.tile([P, GL * HEADS], BF16, tag="Eb")
                nc.scalar.copy(tEb[:], tE[:])

                # denominators + pad correction + reciprocal
                # (contiguous per-sub-tile outputs; strided reduce outputs
                # misbehave on HW)
                t_den = wpool.tile([P, 4 * gs], F32, tag="den")
                t_rec = wpool.tile([P, 4 * gs], F32, tag="rec")
                nc.vector.memset(t_den[:], 1.0)
                nc.vector.tensor_reduce(
                    out=t_den[:].rearrange("p (s h) -> p s h", h=4)
                    [:, :, 0:HEADS],
                    in_=tE[:].rearrange("p (s l h) -> p s l h", s=gs, h=HEADS)
                    .transpose([0, 1, 3, 2]),
                    axis=mybir.AxisListType.X, op=mybir.AluOpType.add)
                dv = (t_den[:].rearrange("p (s h) -> p s h", h=4)
                      [:, :, 0:HEADS])
                cview = (t_cor[:, t0 * 4:(t0 + gs) * 4]
                         .rearrange("p (s h) -> p s h", h=4)[:, :, 0:HEADS])
                nc.vector.tensor_tensor(
                    out=dv, in0=dv, in1=cview, op=mybir.AluOpType.subtract)
                nc.vector.tensor_scalar_max(t_den[:], t_den[:], 1e-30)
                nc.vector.reciprocal(t_rec[:], t_den[:])

                # weighted messages M = e * h (bf16; c-major h block),
                # one multiply for the whole group ((s,l) merged), then two
                # group-wide pairwise tree-add levels + one reduce tail
                tM = mpool.tile([P, GL * HC], BF16, tag="M")
                tU = wpool.tile([P, gs * HC], F32, tag="U")
                M4v = tM[:].rearrange("p (q c h) -> p q c h", c=CH, h=HEADS)
                e_b = (tEb[:].rearrange("p (q h) -> p q h", h=HEADS)
                       .unsqueeze(2).to_broadcast((P, GL, CH, HEADS)))
                gh = (g[:].rearrange("p (q c) -> p q c", c=ROW)[:, :, 0:HC]
                      .rearrange("p q (c h) -> p q c h", h=HEADS))
                nc.vector.tensor_tensor(
                    out=M4v[:], in0=gh, in1=e_b, op=mybir.AluOpType.mult)
                for n in (L // 2, L // 4):
                    sv = tM[:].rearrange("p (s q) -> p s q", s=gs)
                    src = (sv[:, :, :2 * n * HC]
                           .rearrange("p s (n two c) -> p s n two c",
                                      two=2, c=HC))
                    dstv = (sv[:, :, :n * HC]
                            .rearrange("p s (n c) -> p s n c", c=HC))
                    nc.vector.tensor_tensor(
                        out=dstv, in0=src[:, :, :, 0, :],
                        in1=src[:, :, :, 1, :], op=mybir.AluOpType.add)
                nc.vector.tensor_reduce(
                    out=tU[:].rearrange("p (s c) -> p s c", s=gs),
                    in_=tM[:].rearrange("p (s q) -> p s q", s=gs)
                    [:, :, :(L // 4) * HC]
                    .rearrange("p s (n c) -> p s n c", c=HC)
                    .transpose([0, 1, 3, 2]),
                    axis=mybir.AxisListType.X, op=mybir.AluOpType.add)

                # V = relu(U / denom + bias); relu on ACT casts to bf16
                tV = wpool.tile([P, gs * HC], F32, tag="V")
                rec_b = (t_rec[:].rearrange("p (s h) -> p s h", h=4)
                         [:, :, 0:HEADS].unsqueeze(2)
                         .to_broadcast((P, gs, CH, HEADS)))
                nc.vector.tensor_tensor(
                    out=tV[:].rearrange("p (s c h) -> p s c h", c=CH, h=HEADS),
                    in0=tU[:].rearrange("p (s c h) -> p s c h", c=CH, h=HEADS),
                    in1=rec_b, op=mybir.AluOpType.mult)
                bb_b = t_bb[:].unsqueeze(1).to_broadcast((P, gs, HC))
                nc.vector.tensor_tensor(
                    out=tV[:].rearrange("p (s c) -> p s c", s=gs),
                    in0=tV[:].rearrange("p (s c) -> p s c", s=gs),
                    in1=bb_b, op=mybir.AluOpType.add)
                nc.scalar.activation(
                    tV[:], tV[:], mybir.ActivationFunctionType.Relu)

                # pool into batches: q[b, 36] += onehot_t^T @ V, PSUM-accum
                t_oh = wpool.tile([P, gs * P], F32, tag="oh")
                nc.sync.dma_start(t_oh[:], d_oh[:, t0 * P:(t0 + gs) * P])
                for st in range(gs):
                    t = t0 + st
                    nc.tensor.matmul(
                        ps_q[:], lhsT=t_oh[:, st * P:(st + 1) * P],
                        rhs=tV[:, st * HC:(st + 1) * HC],
                        start=(t == 0), stop=(t == n_dst_tiles - 1))

            t_q = cpool.tile([P, HC], F32)
            nc.vector.tensor_copy(t_q[:], ps_q[:])
            nc.sync.dma_start(d_q[:], t_q[:])
    nc.finalize()
    _nc_cache[key] = nc
    return nc


def kernel(**inputs):
    x_s = np.asarray(inputs["x_s"], np.float32)
    x_t = np.asarray(inputs["x_t"], np.float32)
    edge_index = np.asarray(inputs["edge_index"])
    x_s_batch = np.asarray(inputs["x_s_batch"]).astype(np.int64)
    W = np.asarray(inputs["W"], np.float32)
    att_src = np.asarray(inputs["att_src"], np.float32)
    att_dst = np.asarray(inputs["att_dst"], np.float32)
    bias = np.asarray(inputs["bias"], np.float32)
    fc1_w = np.asarray(inputs["fc1_w"], np.float32)
    fc1_b = np.asarray(inputs["fc1_b"], np.float32)
    fc3_w = np.asarray(inputs["fc3_w"], np.float32)
    fc3_b = np.asarray(inputs["fc3_b"], np.float32)

    n_nodes, in_dim = x_s.shape
    src = edge_index[0].astype(np.int64)
    dst = edge_index[1].astype(np.int64)

    # ---- host: edge bucketing by destination (index/layout prep only) ----
    deg = np.bincount(dst, minlength=n_nodes)
    order = np.argsort(-deg, kind="stable")          # nodes by degree desc
    inv_order = np.empty(n_nodes, np.int64)
    inv_order[order] = np.arange(n_nodes)
    nodes_per_core = (n_nodes + N_CORES - 1) // N_CORES
    n_dst_tiles = (nodes_per_core + P - 1) // P
    n_dst_tiles = (n_dst_tiles + 3) // 4 * 4         # whole groups of 4
    L_list = []
    for t in range(n_dst_tiles):
        r0 = t * P * N_CORES
        L = max(4, int(deg[order[min(r0, n_nodes - 1)]]))
        L_list.append((L + 3) // 4 * 4)              # multiple of 4
    k = 0                                            # solo tiles (big L)
    while k < n_dst_tiles and L_list[k] > 44:
        k += 1
    k = min((k + 3) // 4 * 4, n_dst_tiles)
    groups = [(t, 1) for t in range(k)]
    for g in range(k, n_dst_tiles, 4):
        Lg = max(L_list[g:g + 4])                    # shared within group
        for t in range(g, g + 4):
            L_list[t] = Lg
        groups.append((g, 4))
    groups = tuple(groups)
    off_arr = np.concatenate([[0], np.cumsum(L_list)]).astype(np.int64)
    slot_tot = int(off_arr[-1])
    half_tot = slot_tot // 2
    n_xt_cols = n_dst_tiles * P

    # edges sorted by dst -> per-node contiguous src runs
    e_order = np.argsort(dst, kind="stable")
    dst_sorted = dst[e_order]
    src_sorted = src[e_order].astype(np.int64)
    starts = np.searchsorted(dst_sorted, np.arange(n_nodes))
    slot_within = np.arange(len(dst_sorted)) - starts[dst_sorted]

    k_global = inv_order[dst_sorted]
    core_of = (k_global % N_CORES).astype(np.int64)
    k_local = k_global // N_CORES
    t_of = k_local // P
    p_of = k_local % P
    col_of = off_arr[t_of] + slot_within

    # fold weights (host weight prep).  W/bias/w2 columns permuted c-major:
    # folded col (c*HEADS + h) <- original col (h*CH + c).
    cm = np.array([h * CH + c for c in range(CH) for h in range(HEADS)])
    W_cm = W[:, cm]
    bias_cm = bias[cm]
    w2_cm = (fc1_w @ fc3_w)[:, 0].astype(np.float32)[cm]

    wa_t = np.einsum("khc,hc->kh", W.reshape(in_dim, HEADS, CH), att_dst)
    wa_s = np.einsum("khc,hc->kh", W.reshape(in_dim, HEADS, CH), att_src)
    wfold = np.zeros((in_dim, ROW), np.float32)
    wfold[:, :HC] = W_cm
    wfold[:, HC:HC + HEADS] = wa_s
    wfbd = np.zeros((2 * in_dim, ROW2), np.float32)
    wfbd[:in_dim, :ROW] = wfold
    wfbd[in_dim:, ROW:] = wfold
    wfbd = wfbd.astype(ml_dtypes.bfloat16)
    wat = np.zeros((in_dim, 4), np.float32)
    wat[:, :HEADS] = wa_t
    biasb = np.tile(bias_cm[None, :], (P, 1)).astype(np.float32)

    xsb_ext = np.zeros((n_nodes + 1, in_dim), ml_dtypes.bfloat16)
    xsb_ext[:n_nodes] = x_s.astype(ml_dtypes.bfloat16)
    SENT = n_nodes

    in_maps = []
    cnts = []
    for c in range(N_CORES):
        node_ids = order[c::N_CORES]                 # this core's dst nodes
        ncnt = len(node_ids)
        m = core_of == c
        SRC = np.full((P, slot_tot), SENT, np.int64)
        SRC[p_of[m], col_of[m]] = src_sorted[m]

        # per-edge lhsT layout: rows 0:64 even slots, 64:128 odd slots
        xe = np.empty((2 * in_dim, half_tot * P), ml_dtypes.bfloat16)
        for par in range(2):
            S = SRC[:, par::2]                       # [P, half_tot]
            blk = xsb_ext[S]                         # [P, half_tot, in_dim]
            xe[par * in_dim:(par + 1) * in_dim] = (
                blk.transpose(2, 1, 0).reshape(in_dim, half_tot * P))

        padc4 = np.zeros((P, n_dst_tiles * 4), np.float32)
        oh = np.zeros((P, n_dst_tiles * P), np.float32)
        xt_t = np.zeros((in_dim, n_xt_cols), np.float32)
        kk = np.arange(n_dst_tiles * P)
        tt, pp = kk // P, kk % P
        present = kk < ncnt
        nid = np.where(present, node_ids[np.minimum(kk, ncnt - 1)], 0)
        Leff = np.asarray(L_list, np.float32)[tt]
        pc = np.where(present, Leff - deg[nid], Leff)
        for j in range(4):
            padc4[pp, 4 * tt + j] = pc
        bid = x_s_batch[nid]
        oh[pp[present], tt[present] * P + bid[present]] = 1.0
        cnts.append(np.bincount(bid[present], minlength=P).astype(np.float64))
        xt_t[:, :ncnt] = x_t[node_ids].T
        in_maps.append({
            "xe": xe, "xt_t": xt_t, "padc4": padc4, "oh": oh, "wfbd": wfbd,
            "wat": wat, "biasb": biasb,
        })

    nc = _build_nc(in_dim, n_dst_tiles, L_list, half_tot, n_xt_cols, groups)
    res = run_bass_kernel_spmd(nc, in_maps, core_ids=list(range(N_CORES)))

    q = np.zeros((P, HC), np.float64)
    cnt = np.zeros(P, np.float64)
    for c in range(N_CORES):
        q += res.results[c]["q_out"]
        cnt += cnts[c]
    num = q @ w2_cm.astype(np.float64)
    out = num / np.maximum(cnt, 1.0)
    const = float(fc1_b @ fc3_w[:, 0] + fc3_b[0])
    return (out + const).astype(np.float32)


# revision 38
# speedup vs baseline: 13.1379x; 1.0010x over previous
"""GAT (bipartite GATConv + mean-pool + 2 FC) on 8 Trainium2 NeuronCores.

Strategy: edges are sharded per destination node; destination nodes are
dealt round-robin (degree-sorted) across the 8 cores so the segment softmax
is fully local to a core.  Per the sharding hint each device holds its edge
shard with the source-node features replicated into matmul-ready per-edge
layout (host does only index manipulation / np.take layout; every model
FLOP runs on device):

  Phase A2: a_t = x_t @ (W att_dst) for this core's dst nodes (PE).
  Phase B: dst nodes are processed in tiles of 128 (one node per partition,
           nodes degree-sorted so tiles have uniform run lengths L).  The
           per-edge source features arrive as bf16 lhsT tiles [128, L/2*128]
           (slot pair 2j/2j+1 stacked as two K=64 halves); one K=128 matmul
           against a block-diagonal [wfold|0 / 0|wfold] rhs computes BOTH
           slots' rows [h_s (36, c-major) | a_s (3) | pad] into PSUM.  ACT
           casts them to bf16 SBUF; E = exp(leaky_relu(a_s+a_t)) =
           max(exp(z), exp(0.2 z)) via two ACT Exp passes over a DVE-added
           z.  The weighted message sum runs in bf16 on DVE's packed 2x
           path: W columns are stored c-major/h-minor so the E broadcast's
           innermost dim is the packed head dim, and the slot reduction is
           two packed tree-add levels plus a short tensor_reduce tail.
           Batch pooling is a PE matmul against host-shipped one-hot
           columns, accumulated over all tiles in PSUM; the final fc1@fc3
           contraction and count division happen on host partials.
           Pad slots carry x=0 => h=0, a_s=0; their exp(leaky_relu(a_t))
           denominator contribution is subtracted exactly via a
           host-precomputed pad-count correction.

Device-side per-edge gathers are avoided entirely: one [P,1]-offset
indirect-DMA gather costs ~1.1us of SWDGE descriptor generation on the Pool
engine (994ns fixed + 0.34ns/desc, 128 descriptors max per instruction) and
the batched-gather ucode (InstDMAGatherAnt etc.) is excluded from bedrock
images, so any gather-based design is floored at ~3.3ms/core.  Sequential
streaming of the pre-laid-out edge shard runs at DMA bandwidth instead.

HW pitfalls (probed): matmuls that switch PE row groups (partition-offset
lhsT/rhs) within one PSUM tile crash the device (the K=128 block-diagonal
formulation sidesteps row groups); Pool-engine TensorTensor is rejected by
this lowering; softmax denominators accumulated from bf16 exps lose ~6x
final accuracy (E stays f32; bf16 is cast only for the message multiply).
"""

import numpy as np
import ml_dtypes

import concourse.bacc as bacc
import concourse.tile as tile
from concourse import mybir
from concourse.bass_utils import run_bass_kernel_spmd

F32 = mybir.dt.float32
BF16 = mybir.dt.bfloat16

N_CORES = 8
P = 128
HEADS = 3
CH = 12
HC = HEADS * CH          # 36
ROW = HC + 4             # matmul output row: 36 h | 3 a_s | 1 pad = 40
ROW2 = 2 * ROW           # block-diagonal pair output
NEG_SLOPE = 0.2
GP = 6                   # slot pairs per PSUM tile (6*80 = 480 f32 <= 512)

_nc_cache = {}


def _build_nc(in_dim, n_dst_tiles, L_list, half_tot, n_xt_cols, groups):
    key = (in_dim, n_dst_tiles, tuple(L_list), half_tot, n_xt_cols, tuple(groups))
    if key in _nc_cache:
        return _nc_cache[key]

    nc = bacc.Bacc("TRN2", target_bir_lowering=False, debug=False)
    d_xe = nc.dram_tensor("xe", [2 * in_dim, half_tot * P], BF16, kind="ExternalInput")
    d_xt = nc.dram_tensor("xt_t", [in_dim, n_xt_cols], F32, kind="ExternalInput")
    d_pc = nc.dram_tensor("padc4", [P, n_dst_tiles * 4], F32, kind="ExternalInput")
    d_oh = nc.dram_tensor("oh", [P, n_dst_tiles * P], F32, kind="ExternalInput")
    d_wf = nc.dram_tensor("wfbd", [2 * in_dim, ROW2], BF16, kind="ExternalInput")
    d_wt = nc.dram_tensor("wat", [in_dim, 4], F32, kind="ExternalInput")
    d_bb = nc.dram_tensor("biasb", [P, HC], F32, kind="ExternalInput")
    d_q = nc.dram_tensor("q_out", [P, HC], F32, kind="ExternalOutput")

    with tile.TileContext(nc) as tc:
        with tc.tile_pool(name="const", bufs=1) as cpool, \
             tc.tile_pool(name="xload", bufs=2) as xpool, \
             tc.tile_pool(name="gat", bufs=3) as gpool, \
             tc.tile_pool(name="work", bufs=3) as wpool, \
             tc.tile_pool(name="msg", bufs=2) as mpool, \
             tc.tile_pool(name="psA", bufs=4, space="PSUM") as psA, \
             tc.tile_pool(name="psB", bufs=1, space="PSUM") as psB, \
             tc.tile_pool(name="psT", bufs=2, space="PSUM") as psT:

            # ---- constants into SBUF ----
            t_wf = cpool.tile([2 * in_dim, ROW2], BF16)
            nc.sync.dma_start(t_wf[:], d_wf[:])
            t_wt = cpool.tile([in_dim, 4], F32)
            nc.sync.dma_start(t_wt[:], d_wt[:])
            t_bb = cpool.tile([P, HC], F32)
            nc.sync.dma_start(t_bb[:], d_bb[:])
            t_pc = cpool.tile([P, n_dst_tiles * 4], F32)
            nc.sync.dma_start(t_pc[:], d_pc[:])
            t_xt = cpool.tile([in_dim, n_xt_cols], F32)
            nc.sync.dma_start(t_xt[:], d_xt[:])

            # ---- phase A2: a_t per dst tile -> resident SBUF ----
            t_at = cpool.tile([P, n_dst_tiles * 4], F32)
            for t0 in range(0, n_dst_tiles, 4):
                nt = min(4, n_dst_tiles - t0)
                ps = psT.tile([P, 16], F32, space="PSUM", tag="psat")
                for j in range(nt):
                    nc.tensor.matmul(
                        ps[:, j * 4:(j + 1) * 4],
                        lhsT=t_xt[:, (t0 + j) * P:(t0 + j + 1) * P], rhs=t_wt[:],
                        start=True, stop=True)
                nc.scalar.copy(
                    t_at[:, t0 * 4:(t0 + nt) * 4], ps[:, :nt * 4])

            # exp(leaky_relu(z)) = max(exp(z), exp(0.2 z)) — exp monotonic.
            # Pad-slot denominator correction for all tiles at once:
            # corall = padc * max(exp(a_t), exp(0.2 a_t)).
            t_at2 = cpool.tile([P, n_dst_tiles * 4], F32)
            nc.vector.tensor_scalar_mul(t_at2[:], t_at[:], NEG_SLOPE)
            t_ep = cpool.tile([P, n_dst_tiles * 4], F32)
            t_ep2 = cpool.tile([P, n_dst_tiles * 4], F32)
            nc.scalar.activation(
                t_ep[:], t_at[:], mybir.ActivationFunctionType.Exp)
            nc.scalar.activation(
                t_ep2[:], t_at2[:], mybir.ActivationFunctionType.Exp)
            nc.vector.tensor_tensor(
                out=t_ep[:], in0=t_ep[:], in1=t_ep2[:], op=mybir.AluOpType.max)
            t_cor = cpool.tile([P, n_dst_tiles * 4], F32)
            nc.vector.tensor_tensor(
                out=t_cor[:], in0=t_ep[:], in1=t_pc[:], op=mybir.AluOpType.mult)

            # ---- phase B: tiles processed in groups sharing L (the few
            # high-degree tiles run solo; the rest in groups of 4).
            # Software-pipelined: group i's DMA/matmul/copy/z/exp stage (A)
            # is emitted before group i-1's softmax/message stage (B), so
            # the in-order DVE stream has group i-1's heavy message work to
            # run while ACT computes group i's exponentials. ----
            ps_q = psB.tile([P, HC], F32, space="PSUM", tag="q")
            state = {}
            off_h = [0]

            def emit_A(idx):
                t0, gs = groups[idx]
                L = L_list[t0]        # shared within group, multiple of 4
                Lh = L // 2
                GL = gs * L
                xe_sb = xpool.tile([2 * in_dim, gs * Lh * P], BF16, tag="xe")
                nc.sync.dma_start(
                    xe_sb[:], d_xe[:, off_h[0] * P:(off_h[0] + gs * Lh) * P])
                off_h[0] += gs * Lh

                # per-edge rows via PE: one K=128 matmul per slot PAIR
                g = gpool.tile([P, GL * ROW], BF16, tag="G")
                for st in range(gs):
                    for h0 in range(0, Lh, GP):
                        nh = min(GP, Lh - h0)
                        ps = psA.tile([P, GP * ROW2], F32, space="PSUM", tag="psa")
                        for j in range(nh):
                            nc.tensor.matmul(
                                ps[:, j * ROW2:(j + 1) * ROW2],
                                lhsT=xe_sb[:, (st * Lh + h0 + j) * P:
                                           (st * Lh + h0 + j + 1) * P],
                                rhs=t_wf[:],
                                start=True, stop=True)
                        nc.scalar.copy(
                            g[:, (st * Lh + h0) * ROW2:
                              (st * Lh + h0 + nh) * ROW2],
                            ps[:, :nh * ROW2])

                g4 = g[:].rearrange("p (s l c) -> p s l c", s=gs, c=ROW)

                # z = a_s + a_t  (layout (s, l, h)); exps on ACT
                tZ = wpool.tile([P, GL * HEADS], F32, tag="Z")
                Z4 = tZ[:].rearrange("p (s l h) -> p s l h", s=gs, h=HEADS)
                at_b = (t_at[:, t0 * 4:(t0 + gs) * 4]
                        .rearrange("p (s h) -> p s h", h=4)[:, :, 0:HEADS]
                        .unsqueeze(2).to_broadcast((P, gs, L, HEADS)))
                nc.vector.tensor_tensor(
                    out=Z4[:], in0=g4[:, :, :, HC:HC + HEADS], in1=at_b,
                    op=mybir.AluOpType.add)
                tE = wpool.tile([P, GL * HEADS], F32, tag="E")
                tT = wpool.tile([P, GL * HEADS], F32, tag="T")
                nc.scalar.activation(
                    tT[:], tZ[:], mybir.ActivationFunctionType.Exp)
                nc.scalar.activation(
                    tE[:], tZ[:], mybir.ActivationFunctionType.Exp,
                    scale=NEG_SLOPE)
                state[idx] = (t0, gs, L, GL, g, tE, tT)

            def emit_B(idx):
                t0, gs, L, GL, g, tE, tT = state.pop(idx)
                nc.vector.tensor_tensor(
                    out=tE[:], in0=tE[:], in1=tT[:], op=mybir.AluOpType.max)
                tEb = wpool.tile([P, GL * HEADS], BF16, tag="Eb")
                nc.scalar.copy(tEb[:], tE[:])

                # denominators + pad correction + reciprocal
                t_den = wpool.tile([P, 4 * gs], F32, tag="den")
                t_rec = wpool.tile([P, 4 * gs], F32, tag="rec")
                nc.vector.memset(t_den[:], 1.0)
                nc.vector.tensor_reduce(
                    out=t_den[:].rearrange("p (s h) -> p s h", h=4)
                    [:, :, 0:HEADS],
                    in_=tE[:].rearrange("p (s l h) -> p s l h", s=gs, h=HEADS)
                    .transpose([0, 1, 3, 2]),
                    axis=mybir.AxisListType.X, op=mybir.AluOpType.add)
                dv = (t_den[:].rearrange("p (s h) -> p s h", h=4)
                      [:, :, 0:HEADS])
                cview = (t_cor[:, t0 * 4:(t0 + gs) * 4]
                         .rearrange("p (s h) -> p s h", h=4)[:, :, 0:HEADS])
                nc.vector.tensor_tensor(
                    out=dv, in0=dv, in1=cview, op=mybir.AluOpType.subtract)
                nc.vector.tensor_scalar_max(t_den[:], t_den[:], 1e-30)
                nc.vector.reciprocal(t_rec[:], t_den[:])

                # weighted messages M = e * h (bf16; c-major h block),
                # one multiply for the whole group ((s,l) merged), then two
                # group-wide pairwise tree-add levels + one reduce tail
                tM = mpool.tile([P, GL * HC], BF16, tag="M")
                tU = wpool.tile([P, gs * HC], F32, tag="U")
                M4v = tM[:].rearrange("p (q c h) -> p q c h", c=CH, h=HEADS)
                e_b = (tEb[:].rearrange("p (q h) -> p q h", h=HEADS)
                       .unsqueeze(2).to_broadcast((P, GL, CH, HEADS)))
                gh = (g[:].rearrange("p (q c) -> p q c", c=ROW)[:, :, 0:HC]
                      .rearrange("p q (c h) -> p q c h", h=HEADS))
                nc.vector.tensor_tensor(
                    out=M4v[:], in0=gh, in1=e_b, op=mybir.AluOpType.mult)
                for n in (L // 2, L // 4):
                    sv = tM[:].rearrange("p (s q) -> p s q", s=gs)
                    src = (sv[:, :, :2 * n * HC]
                           .rearrange("p s (n two c) -> p s n two c",
                                      two=2, c=HC))
                    dstv = (sv[:, :, :n * HC]
                            .rearrange("p s (n c) -> p s n c", c=HC))
                    nc.vector.tensor_tensor(
                        out=dstv, in0=src[:, :, :, 0, :],
                        in1=src[:, :, :, 1, :], op=mybir.AluOpType.add)
                nc.vector.tensor_reduce(
                    out=tU[:].rearrange("p (s c) -> p s c", s=gs),
                    in_=tM[:].rearrange("p (s q) -> p s q", s=gs)
                    [:, :, :(L // 4) * HC]
                    .rearrange("p s (n c) -> p s n c", c=HC)
                    .transpose([0, 1, 3, 2]),
                    axis=mybir.AxisListType.X, op=mybir.AluOpType.add)

                # V = relu(U / denom + bias); relu on ACT
                tV = wpool.tile([P, gs * HC], F32, tag="V")
                rec_b = (t_rec[:].rearrange("p (s h) -> p s h", h=4)
                         [:, :, 0:HEADS].unsqueeze(2)
                         .to_broadcast((P, gs, CH, HEADS)))
                nc.vector.tensor_tensor(
                    out=tV[:].rearrange("p (s c h) -> p s c h", c=CH, h=HEADS),
                    in0=tU[:].rearrange("p (s c h) -> p s c h", c=CH, h=HEADS),
                    in1=rec_b, op=mybir.AluOpType.mult)
                bb_b = t_bb[:].unsqueeze(1).to_broadcast((P, gs, HC))
                nc.vector.tensor_tensor(
                    out=tV[:].rearrange("p (s c) -> p s c", s=gs),
                    in0=tV[:].rearrange("p (s c) -> p s c", s=gs),
                    in1=bb_b, op=mybir.AluOpType.add)
                nc.scalar.activation(
                    tV[:], tV[:], mybir.ActivationFunctionType.Relu)

                # pool into batches: q[b, 36] += onehot_t^T @ V, PSUM-accum
                t_oh = wpool.tile([P, gs * P], F32, tag="oh")
                nc.sync.dma_start(t_oh[:], d_oh[:, t0 * P:(t0 + gs) * P])
                for st in range(gs):
                    t = t0 + st
                    nc.tensor.matmul(
                        ps_q[:], lhsT=t_oh[:, st * P:(st + 1) * P],
                        rhs=tV[:, st * HC:(st + 1) * HC],
                        start=(t == 0), stop=(t == n_dst_tiles - 1))

            for idx in range(len(groups)):
                emit_A(idx)
                if idx > 0:
                    emit_B(idx - 1)
            emit_B(len(groups) - 1)

            t_q = cpool.tile([P, HC], F32)
            nc.vector.tensor_copy(t_q[:], ps_q[:])
            nc.sync.dma_start(d_q[:], t_q[:])
    nc.finalize()
    _nc_cache[key] = nc
    return nc


def kernel(**inputs):
    x_s = np.asarray(inputs["x_s"], np.float32)
    x_t = np.asarray(inputs["x_t"], np.float32)
    edge_index = np.asarray(inputs["edge_index"])
    x_s_batch = np.asarray(inputs["x_s_batch"]).astype(np.int64)
    W = np.asarray(inputs["W"], np.float32)
    att_src = np.asarray(inputs["att_src"], np.float32)
    att_dst = np.asarray(inputs["att_dst"], np.float32)
    bias = np.asarray(inputs["bias"], np.float32)
    fc1_w = np.asarray(inputs["fc1_w"], np.float32)
    fc1_b = np.asarray(inputs["fc1_b"], np.float32)
    fc3_w = np.asarray(inputs["fc3_w"], np.float32)
    fc3_b = np.asarray(inputs["fc3_b"], np.float32)

    n_nodes, in_dim = x_s.shape
    src = edge_index[0].astype(np.int64)
    dst = edge_index[1].astype(np.int64)

    # ---- host: edge bucketing by destination (index/layout prep only) ----
    deg = np.bincount(dst, minlength=n_nodes)
    order = np.argsort(-deg, kind="stable")          # nodes by degree desc
    inv_order = np.empty(n_nodes, np.int64)
    inv_order[order] = np.arange(n_nodes)
    nodes_per_core = (n_nodes + N_CORES - 1) // N_CORES
    n_dst_tiles = (nodes_per_core + P - 1) // P
    n_dst_tiles = (n_dst_tiles + 3) // 4 * 4         # whole groups of 4
    L_list = []
    for t in range(n_dst_tiles):
        r0 = t * P * N_CORES
        L = max(4, int(deg[order[min(r0, n_nodes - 1)]]))
        L_list.append((L + 3) // 4 * 4)              # multiple of 4
    k = 0                                            # solo tiles (big L)
    while k < n_dst_tiles and L_list[k] > 44:
        k += 1
    k = min((k + 3) // 4 * 4, n_dst_tiles)
    groups = [(t, 1) for t in range(k)]
    for g in range(k, n_dst_tiles, 4):
        Lg = max(L_list[g:g + 4])                    # shared within group
        for t in range(g, g + 4):
            L_list[t] = Lg
        groups.append((g, 4))
    groups = tuple(groups)
    off_arr = np.concatenate([[0], np.cumsum(L_list)]).astype(np.int64)
    slot_tot = int(off_arr[-1])
    half_tot = slot_tot // 2
    n_xt_cols = n_dst_tiles * P

    # edges sorted by dst -> per-node contiguous src runs
    e_order = np.argsort(dst, kind="stable")
    dst_sorted = dst[e_order]
    src_sorted = src[e_order].astype(np.int64)
    starts = np.searchsorted(dst_sorted, np.arange(n_nodes))
    slot_within = np.arange(len(dst_sorted)) - starts[dst_sorted]

    k_global = inv_order[dst_sorted]
    core_of = (k_global % N_CORES).astype(np.int64)
    k_local = k_global // N_CORES
    t_of = k_local // P
    p_of = k_local % P
    col_of = off_arr[t_of] + slot_within

    # fold weights (host weight prep).  W/bias/w2 columns permuted c-major:
    # folded col (c*HEADS + h) <- original col (h*CH + c).
    cm = np.array([h * CH + c for c in range(CH) for h in range(HEADS)])
    W_cm = W[:, cm]
    bias_cm = bias[cm]
    w2_cm = (fc1_w @ fc3_w)[:, 0].astype(np.float32)[cm]

    wa_t = np.einsum("khc,hc->kh", W.reshape(in_dim, HEADS, CH), att_dst)
    wa_s = np.einsum("khc,hc->kh", W.reshape(in_dim, HEADS, CH), att_src)
    wfold = np.zeros((in_dim, ROW), np.float32)
    wfold[:, :HC] = W_cm
    wfold[:, HC:HC + HEADS] = wa_s
    wfbd = np.zeros((2 * in_dim, ROW2), np.float32)
    wfbd[:in_dim, :ROW] = wfold
    wfbd[in_dim:, ROW:] = wfold
    wfbd = wfbd.astype(ml_dtypes.bfloat16)
    wat = np.zeros((in_dim, 4), np.float32)
    wat[:, :HEADS] = wa_t
    biasb = np.tile(bias_cm[None, :], (P, 1)).astype(np.float32)

    xsb_ext = np.zeros((n_nodes + 1, in_dim), ml_dtypes.bfloat16)
    xsb_ext[:n_nodes] = x_s.astype(ml_dtypes.bfloat16)
    SENT = n_nodes

    in_maps = []
    cnts = []
    for c in range(N_CORES):
        node_ids = order[c::N_CORES]                 # this core's dst nodes
        ncnt = len(node_ids)
        m = core_of == c
        SRC = np.full((P, slot_tot), SENT, np.int64)
        SRC[p_of[m], col_of[m]] = src_sorted[m]

        # per-edge lhsT layout: rows 0:64 even slots, 64:128 odd slots
        xe = np.empty((2 * in_dim, half_tot * P), ml_dtypes.bfloat16)
        for par in range(2):
            S = SRC[:, par::2]                       # [P, half_tot]
            blk = xsb_ext[S]                         # [P, half_tot, in_dim]
            xe[par * in_dim:(par + 1) * in_dim] = (
                blk.transpose(2, 1, 0).reshape(in_dim, half_tot * P))

        padc4 = np.zeros((P, n_dst_tiles * 4), np.float32)
        oh = np.zeros((P, n_dst_tiles * P), np.float32)
        xt_t = np.zeros((in_dim, n_xt_cols), np.float32)
        kk = np.arange(n_dst_tiles * P)
        tt, pp = kk // P, kk % P
        present = kk < ncnt
        nid = np.where(present, node_ids[np.minimum(kk, ncnt - 1)], 0)
        Leff = np.asarray(L_list, np.float32)[tt]
        pc = np.where(present, Leff - deg[nid], Leff)
        for j in range(4):
            padc4[pp, 4 * tt + j] = pc
        bid = x_s_batch[nid]
        oh[pp[present], tt[present] * P + bid[present]] = 1.0
        cnts.append(np.bincount(bid[present], minlength=P).astype(np.float64))
        xt_t[:, :ncnt] = x_t[node_ids].T
        in_maps.append({
            "xe": xe, "xt_t": xt_t, "padc4": padc4, "oh": oh, "wfbd": wfbd,
            "wat": wat, "biasb": biasb,
        })

    nc = _build_nc(in_dim, n_dst_tiles, L_list, half_tot, n_xt_cols, groups)
    res = run_bass_kernel_spmd(nc, in_maps, core_ids=list(range(N_CORES)))

    q = np.zeros((P, HC), np.float64)
    cnt = np.zeros(P, np.float64)
    for c in range(N_CORES):
        q += res.results[c]["q_out"]
        cnt += cnts[c]
    num = q @ w2_cm.astype(np.float64)
    out = num / np.maximum(cnt, 1.0)
    const = float(fc1_b @ fc3_w[:, 0] + fc3_b[0])
    return (out + const).astype(np.float32)


# revision 39
# speedup vs baseline: 14.1004x; 1.0733x over previous
"""GAT (bipartite GATConv + mean-pool + 2 FC) on 8 Trainium2 NeuronCores.

Strategy: edges are sharded per destination node; destination nodes are
dealt round-robin (degree-sorted) across the 8 cores so the segment softmax
is fully local to a core.  Per the sharding hint each device holds its edge
shard with the source-node features replicated into matmul-ready per-edge
layout (host does only index manipulation / np.take layout; every model
FLOP runs on device):

  Phase A2: a_t = x_t @ (W att_dst) for this core's dst nodes (PE).
  Phase B: dst nodes are processed in tiles of 128 (one node per partition,
           nodes degree-sorted so tiles have uniform run lengths L).  The
           per-edge source features arrive as bf16 lhsT tiles [128, L/2*128]
           (slot pair 2j/2j+1 stacked as two K=64 halves); one K=128 matmul
           against a block-diagonal [wfold|0 / 0|wfold] rhs computes BOTH
           slots' rows [h_s (36, c-major) | a_s (3) | pad] into PSUM.  ACT
           casts them to bf16 SBUF; E = exp(leaky_relu(a_s+a_t)) =
           max(exp(z), exp(0.2 z)) via two ACT Exp passes over a DVE-added
           z.  The weighted message sum runs in bf16 on DVE's packed 2x
           path: W columns are stored c-major/h-minor so the E broadcast's
           innermost dim is the packed head dim, and the slot reduction is
           two packed tree-add levels plus a short tensor_reduce tail.
           Batch pooling is a PE matmul against host-shipped one-hot
           columns, accumulated over all tiles in PSUM; the final fc1@fc3
           contraction and count division happen on host partials.
           Pad slots carry x=0 => h=0, a_s=0; their exp(leaky_relu(a_t))
           denominator contribution is subtracted exactly via a
           host-precomputed pad-count correction.

Device-side per-edge gathers are avoided entirely: one [P,1]-offset
indirect-DMA gather costs ~1.1us of SWDGE descriptor generation on the Pool
engine (994ns fixed + 0.34ns/desc, 128 descriptors max per instruction) and
the batched-gather ucode (InstDMAGatherAnt etc.) is excluded from bedrock
images, so any gather-based design is floored at ~3.3ms/core.  Sequential
streaming of the pre-laid-out edge shard runs at DMA bandwidth instead.

HW pitfalls (probed): matmuls that switch PE row groups (partition-offset
lhsT/rhs) within one PSUM tile crash the device (the K=128 block-diagonal
formulation sidesteps row groups); Pool-engine TensorTensor is rejected by
this lowering; softmax denominators accumulated from bf16 exps lose ~6x
final accuracy (E stays f32; bf16 is cast only for the message multiply).
"""

import numpy as np
import ml_dtypes

import concourse.bacc as bacc
import concourse.tile as tile
from concourse import mybir
from concourse.bass_utils import run_bass_kernel_spmd

F32 = mybir.dt.float32
BF16 = mybir.dt.bfloat16

N_CORES = 8
P = 128
HEADS = 3
CH = 12
HC = HEADS * CH          # 36
ROW = HC + 4             # matmul output row: 36 h | 3 a_s | 1 pad = 40
ROW2 = 2 * ROW           # block-diagonal pair output
NEG_SLOPE = 0.2
GP = 6                   # slot pairs per PSUM tile (6*80 = 480 f32 <= 512)

_nc_cache = {}


def _build_nc(in_dim, n_dst_tiles, L_list, half_tot, n_xt_cols, groups):
    key = (in_dim, n_dst_tiles, tuple(L_list), half_tot, n_xt_cols, tuple(groups))
    if key in _nc_cache:
        return _nc_cache[key]

    nc = bacc.Bacc("TRN2", target_bir_lowering=False, debug=False)
    d_xe = nc.dram_tensor("xe", [2 * in_dim, half_tot * P], BF16, kind="ExternalInput")
    d_xt = nc.dram_tensor("xt_t", [in_dim, n_xt_cols], F32, kind="ExternalInput")
    d_pc = nc.dram_tensor("padc4", [P, n_dst_tiles * 4], F32, kind="ExternalInput")
    d_oh = nc.dram_tensor("oh", [P, n_dst_tiles * P], F32, kind="ExternalInput")
    d_wf = nc.dram_tensor("wfbd", [2 * in_dim, ROW2], BF16, kind="ExternalInput")
    d_wt = nc.dram_tensor("wat", [in_dim, 4], F32, kind="ExternalInput")
    d_bb = nc.dram_tensor("biasb", [P, HC], F32, kind="ExternalInput")
    d_q = nc.dram_tensor("q_out", [P, HC], F32, kind="ExternalOutput")

    with tile.TileContext(nc) as tc:
        with tc.tile_pool(name="const", bufs=1) as cpool, \
             tc.tile_pool(name="xload", bufs=2) as xpool, \
             tc.tile_pool(name="gat", bufs=3) as gpool, \
             tc.tile_pool(name="work", bufs=3) as wpool, \
             tc.tile_pool(name="msg", bufs=2) as mpool, \
             tc.tile_pool(name="psA", bufs=4, space="PSUM") as psA, \
             tc.tile_pool(name="psB", bufs=1, space="PSUM") as psB, \
             tc.tile_pool(name="psT", bufs=2, space="PSUM") as psT:

            # ---- constants into SBUF ----
            t_wf = cpool.tile([2 * in_dim, ROW2], BF16)
            nc.sync.dma_start(t_wf[:], d_wf[:])
            t_wt = cpool.tile([in_dim, 4], F32)
            nc.sync.dma_start(t_wt[:], d_wt[:])
            t_bb = cpool.tile([P, HC], F32)
            nc.sync.dma_start(t_bb[:], d_bb[:])
            t_pc = cpool.tile([P, n_dst_tiles * 4], F32)
            nc.sync.dma_start(t_pc[:], d_pc[:])
            t_xt = cpool.tile([in_dim, n_xt_cols], F32)
            nc.sync.dma_start(t_xt[:], d_xt[:])

            # ---- phase B: tiles processed in groups sharing L (the few
            # high-degree tiles run solo; the rest in groups of 4).
            # Software-pipelined: group i's DMA/matmul/copy/z/exp stage (A)
            # is emitted before group i-1's softmax/message stage (B), so
            # the in-order DVE stream has group i-1's heavy message work to
            # run while ACT computes group i's exponentials. ----
            ps_q = psB.tile([P, HC], F32, space="PSUM", tag="q")
            state = {}
            off_h = [0]

            def emit_A(idx):
                t0, gs = groups[idx]
                L = L_list[t0]        # shared within group, multiple of 4
                Lh = L // 2
                GL = gs * L
                xe_sb = xpool.tile([2 * in_dim, gs * Lh * P], BF16, tag="xe")
                nc.sync.dma_start(
                    xe_sb[:], d_xe[:, off_h[0] * P:(off_h[0] + gs * Lh) * P])
                off_h[0] += gs * Lh

                # a_t for this group's tiles (interleaved A2)
                t_atg = wpool.tile([P, 4 * gs], F32, tag="at")
                psa2 = psT.tile([P, 4 * gs], F32, space="PSUM", tag="psat")
                for j in range(gs):
                    nc.tensor.matmul(
                        psa2[:, j * 4:(j + 1) * 4],
                        lhsT=t_xt[:, (t0 + j) * P:(t0 + j + 1) * P],
                        rhs=t_wt[:], start=True, stop=True)
                nc.scalar.copy(t_atg[:], psa2[:])

                # per-edge rows via PE: one K=128 matmul per slot PAIR
                g = gpool.tile([P, GL * ROW], BF16, tag="G")
                for st in range(gs):
                    for h0 in range(0, Lh, GP):
                        nh = min(GP, Lh - h0)
                        ps = psA.tile([P, GP * ROW2], F32, space="PSUM", tag="psa")
                        for j in range(nh):
                            nc.tensor.matmul(
                                ps[:, j * ROW2:(j + 1) * ROW2],
                                lhsT=xe_sb[:, (st * Lh + h0 + j) * P:
                                           (st * Lh + h0 + j + 1) * P],
                                rhs=t_wf[:],
                                start=True, stop=True)
                        nc.scalar.copy(
                            g[:, (st * Lh + h0) * ROW2:
                              (st * Lh + h0 + nh) * ROW2],
                            ps[:, :nh * ROW2])

                g4 = g[:].rearrange("p (s l c) -> p s l c", s=gs, c=ROW)

                # z = a_s + a_t  (layout (s, l, h)); exps on ACT
                tZ = wpool.tile([P, GL * HEADS], F32, tag="Z")
                Z4 = tZ[:].rearrange("p (s l h) -> p s l h", s=gs, h=HEADS)
                at_b = (t_atg[:].rearrange("p (s h) -> p s h", h=4)
                        [:, :, 0:HEADS]
                        .unsqueeze(2).to_broadcast((P, gs, L, HEADS)))
                nc.vector.tensor_tensor(
                    out=Z4[:], in0=g4[:, :, :, HC:HC + HEADS], in1=at_b,
                    op=mybir.AluOpType.add)
                tE = wpool.tile([P, GL * HEADS], F32, tag="E")
                tT = wpool.tile([P, GL * HEADS], F32, tag="T")
                nc.scalar.activation(
                    tT[:], tZ[:], mybir.ActivationFunctionType.Exp)
                nc.scalar.activation(
                    tE[:], tZ[:], mybir.ActivationFunctionType.Exp,
                    scale=NEG_SLOPE)
                state[idx] = (t0, gs, L, GL, g, tE, tT, t_atg)

            def emit_B(idx):
                t0, gs, L, GL, g, tE, tT, t_atg = state.pop(idx)
                nc.vector.tensor_tensor(
                    out=tE[:], in0=tE[:], in1=tT[:], op=mybir.AluOpType.max)
                tEb = wpool.tile([P, GL * HEADS], BF16, tag="Eb")
                nc.scalar.copy(tEb[:], tE[:])

                # denominators + pad correction + reciprocal
                t_den = wpool.tile([P, 4 * gs], F32, tag="den")
                t_rec = wpool.tile([P, 4 * gs], F32, tag="rec")
                nc.vector.memset(t_den[:], 1.0)
                nc.vector.tensor_reduce(
                    out=t_den[:].rearrange("p (s h) -> p s h", h=4)
                    [:, :, 0:HEADS],
                    in_=tE[:].rearrange("p (s l h) -> p s l h", s=gs, h=HEADS)
                    .transpose([0, 1, 3, 2]),
                    axis=mybir.AxisListType.X, op=mybir.AluOpType.add)
                # pad correction: cor = padc * max(exp(a_t), exp(0.2 a_t))
                t_c2 = wpool.tile([P, 4 * gs], F32, tag="c2")
                t_c3 = wpool.tile([P, 4 * gs], F32, tag="c3")
                nc.vector.tensor_scalar_mul(t_c2[:], t_atg[:], NEG_SLOPE)
                nc.scalar.activation(
                    t_c3[:], t_c2[:], mybir.ActivationFunctionType.Exp)
                nc.scalar.activation(
                    t_c2[:], t_atg[:], mybir.ActivationFunctionType.Exp)
                nc.vector.tensor_tensor(
                    out=t_c2[:], in0=t_c2[:], in1=t_c3[:],
                    op=mybir.AluOpType.max)
                nc.vector.tensor_tensor(
                    out=t_c2[:], in0=t_c2[:],
                    in1=t_pc[:, t0 * 4:(t0 + gs) * 4],
                    op=mybir.AluOpType.mult)
                dv = (t_den[:].rearrange("p (s h) -> p s h", h=4)
                      [:, :, 0:HEADS])
                cview = (t_c2[:].rearrange("p (s h) -> p s h", h=4)
                         [:, :, 0:HEADS])
                nc.vector.tensor_tensor(
                    out=dv, in0=dv, in1=cview, op=mybir.AluOpType.subtract)
                nc.vector.tensor_scalar_max(t_den[:], t_den[:], 1e-30)
                nc.vector.reciprocal(t_rec[:], t_den[:])

                # weighted messages M = e * h (bf16; c-major h block),
                # one multiply for the whole group ((s,l) merged), then two
                # group-wide pairwise tree-add levels + one reduce tail
                tM = mpool.tile([P, GL * HC], BF16, tag="M")
                tU = wpool.tile([P, gs * HC], F32, tag="U")
                M4v = tM[:].rearrange("p (q c h) -> p q c h", c=CH, h=HEADS)
                e_b = (tEb[:].rearrange("p (q h) -> p q h", h=HEADS)
                       .unsqueeze(2).to_broadcast((P, GL, CH, HEADS)))
                gh = (g[:].rearrange("p (q c) -> p q c", c=ROW)[:, :, 0:HC]
                      .rearrange("p q (c h) -> p q c h", h=HEADS))
                nc.vector.tensor_tensor(
                    out=M4v[:], in0=gh, in1=e_b, op=mybir.AluOpType.mult)
                for n in (L // 2, L // 4):
                    sv = tM[:].rearrange("p (s q) -> p s q", s=gs)
                    src = (sv[:, :, :2 * n * HC]
                           .rearrange("p s (n two c) -> p s n two c",
                                      two=2, c=HC))
                    dstv = (sv[:, :, :n * HC]
                            .rearrange("p s (n c) -> p s n c", c=HC))
                    nc.vector.tensor_tensor(
                        out=dstv, in0=src[:, :, :, 0, :],
                        in1=src[:, :, :, 1, :], op=mybir.AluOpType.add)
                nc.vector.tensor_reduce(
                    out=tU[:].rearrange("p (s c) -> p s c", s=gs),
                    in_=tM[:].rearrange("p (s q) -> p s q", s=gs)
                    [:, :, :(L // 4) * HC]
                    .rearrange("p s (n c) -> p s n c", c=HC)
                    .transpose([0, 1, 3, 2]),
                    axis=mybir.AxisListType.X, op=mybir.AluOpType.add)

                # V = relu(U / denom + bias); relu on ACT
                tV = wpool.tile([P, gs * HC], F32, tag="V")
                rec_b = (t_rec[:].rearrange("p (s h) -> p s h", h=4)
                         [:, :, 0:HEADS].unsqueeze(2)
                         .to_broadcast((P, gs, CH, HEADS)))
                nc.vector.tensor_tensor(
                    out=tV[:].rearrange("p (s c h) -> p s c h", c=CH, h=HEADS),
                    in0=tU[:].rearrange("p (s c h) -> p s c h", c=CH, h=HEADS),
                    in1=rec_b, op=mybir.AluOpType.mult)
                bb_b = t_bb[:].unsqueeze(1).to_broadcast((P, gs, HC))
                nc.vector.tensor_tensor(
                    out=tV[:].rearrange("p (s c) -> p s c", s=gs),
                    in0=tV[:].rearrange("p (s c) -> p s c", s=gs),
                    in1=bb_b, op=mybir.AluOpType.add)
                nc.scalar.activation(
                    tV[:], tV[:], mybir.ActivationFunctionType.Relu)

                # pool into batches: q[b, 36] += onehot_t^T @ V, PSUM-accum
                t_oh = wpool.tile([P, gs * P], F32, tag="oh")
                nc.sync.dma_start(t_oh[:], d_oh[:, t0 * P:(t0 + gs) * P])
                for st in range(gs):
                    t = t0 + st
                    nc.tensor.matmul(
                        ps_q[:], lhsT=t_oh[:, st * P:(st + 1) * P],
                        rhs=tV[:, st * HC:(st + 1) * HC],
                        start=(t == 0), stop=(t == n_dst_tiles - 1))

            for idx in range(len(groups)):
                emit_A(idx)
                if idx > 0:
                    emit_B(idx - 1)
            emit_B(len(groups) - 1)

            t_q = cpool.tile([P, HC], F32)
            nc.vector.tensor_copy(t_q[:], ps_q[:])
            nc.sync.dma_start(d_q[:], t_q[:])
    nc.finalize()
    _nc_cache[key] = nc
    return nc


def kernel(**inputs):
    x_s = np.asarray(inputs["x_s"], np.float32)
    x_t = np.asarray(inputs["x_t"], np.float32)
    edge_index = np.asarray(inputs["edge_index"])
    x_s_batch = np.asarray(inputs["x_s_batch"]).astype(np.int64)
    W = np.asarray(inputs["W"], np.float32)
    att_src = np.asarray(inputs["att_src"], np.float32)
    att_dst = np.asarray(inputs["att_dst"], np.float32)
    bias = np.asarray(inputs["bias"], np.float32)
    fc1_w = np.asarray(inputs["fc1_w"], np.float32)
    fc1_b = np.asarray(inputs["fc1_b"], np.float32)
    fc3_w = np.asarray(inputs["fc3_w"], np.float32)
    fc3_b = np.asarray(inputs["fc3_b"], np.float32)

    n_nodes, in_dim = x_s.shape
    src = edge_index[0].astype(np.int64)
    dst = edge_index[1].astype(np.int64)

    # ---- host: edge bucketing by destination (index/layout prep only) ----
    deg = np.bincount(dst, minlength=n_nodes)
    order = np.argsort(-deg, kind="stable")          # nodes by degree desc
    inv_order = np.empty(n_nodes, np.int64)
    inv_order[order] = np.arange(n_nodes)
    nodes_per_core = (n_nodes + N_CORES - 1) // N_CORES
    n_dst_tiles = (nodes_per_core + P - 1) // P
    n_dst_tiles = (n_dst_tiles + 3) // 4 * 4         # whole groups of 4
    L_list = []
    for t in range(n_dst_tiles):
        r0 = t * P * N_CORES
        L = max(4, int(deg[order[min(r0, n_nodes - 1)]]))
        L_list.append((L + 3) // 4 * 4)              # multiple of 4
    k = 0                                            # solo tiles (big L)
    while k < n_dst_tiles and L_list[k] > 44:
        k += 1
    k = min((k + 3) // 4 * 4, n_dst_tiles)
    groups = [(t, 1) for t in range(k)]
    for g in range(k, n_dst_tiles, 4):
        Lg = max(L_list[g:g + 4])                    # shared within group
        for t in range(g, g + 4):
            L_list[t] = Lg
        groups.append((g, 4))
    groups = tuple(groups)
    off_arr = np.concatenate([[0], np.cumsum(L_list)]).astype(np.int64)
    slot_tot = int(off_arr[-1])
    half_tot = slot_tot // 2
    n_xt_cols = n_dst_tiles * P

    # edges sorted by dst -> per-node contiguous src runs
    e_order = np.argsort(dst, kind="stable")
    dst_sorted = dst[e_order]
    src_sorted = src[e_order].astype(np.int64)
    starts = np.searchsorted(dst_sorted, np.arange(n_nodes))
    slot_within = np.arange(len(dst_sorted)) - starts[dst_sorted]

    k_global = inv_order[dst_sorted]
    core_of = (k_global % N_CORES).astype(np.int64)
    k_local = k_global // N_CORES
    t_of = k_local // P
    p_of = k_local % P
    col_of = off_arr[t_of] + slot_within

    # fold weights (host weight prep).  W/bias/w2 columns permuted c-major:
    # folded col (c*HEADS + h) <- original col (h*CH + c).
    cm = np.array([h * CH + c for c in range(CH) for h in range(HEADS)])
    W_cm = W[:, cm]
    bias_cm = bias[cm]
    w2_cm = (fc1_w @ fc3_w)[:, 0].astype(np.float32)[cm]

    wa_t = np.einsum("khc,hc->kh", W.reshape(in_dim, HEADS, CH), att_dst)
    wa_s = np.einsum("khc,hc->kh", W.reshape(in_dim, HEADS, CH), att_src)
    wfold = np.zeros((in_dim, ROW), np.float32)
    wfold[:, :HC] = W_cm
    wfold[:, HC:HC + HEADS] = wa_s
    wfbd = np.zeros((2 * in_dim, ROW2), np.float32)
    wfbd[:in_dim, :ROW] = wfold
    wfbd[in_dim:, ROW:] = wfold
    wfbd = wfbd.astype(ml_dtypes.bfloat16)
    wat = np.zeros((in_dim, 4), np.float32)
    wat[:, :HEADS] = wa_t
    biasb = np.tile(bias_cm[None, :], (P, 1)).astype(np.float32)

    xsb_ext = np.zeros((n_nodes + 1, in_dim), ml_dtypes.bfloat16)
    xsb_ext[:n_nodes] = x_s.astype(ml_dtypes.bfloat16)
    SENT = n_nodes

    in_maps = []
    cnts = []
    for c in range(N_CORES):
        node_ids = order[c::N_CORES]                 # this core's dst nodes
        ncnt = len(node_ids)
        m = core_of == c
        SRC = np.full((P, slot_tot), SENT, np.int64)
        SRC[p_of[m], col_of[m]] = src_sorted[m]

        # per-edge lhsT layout: rows 0:64 even slots, 64:128 odd slots
        xe = np.empty((2 * in_dim, half_tot * P), ml_dtypes.bfloat16)
        for par in range(2):
            S = SRC[:, par::2]                       # [P, half_tot]
            blk = xsb_ext[S]                         # [P, half_tot, in_dim]
            xe[par * in_dim:(par + 1) * in_dim] = (
                blk.transpose(2, 1, 0).reshape(in_dim, half_tot * P))

        padc4 = np.zeros((P, n_dst_tiles * 4), np.float32)
        oh = np.zeros((P, n_dst_tiles * P), np.float32)
        xt_t = np.zeros((in_dim, n_xt_cols), np.float32)
        kk = np.arange(n_dst_tiles * P)
        tt, pp = kk // P, kk % P
        present = kk < ncnt
        nid = np.where(present, node_ids[np.minimum(kk, ncnt - 1)], 0)
        Leff = np.asarray(L_list, np.float32)[tt]
        pc = np.where(present, Leff - deg[nid], Leff)
        for j in range(4):
            padc4[pp, 4 * tt + j] = pc
        bid = x_s_batch[nid]
        oh[pp[present], tt[present] * P + bid[present]] = 1.0
        cnts.append(np.bincount(bid[present], minlength=P).astype(np.float64))
        xt_t[:, :ncnt] = x_t[node_ids].T
        in_maps.append({
            "xe": xe, "xt_t": xt_t, "padc4": padc4, "oh": oh, "wfbd": wfbd,
            "wat": wat, "biasb": biasb,
        })

    nc = _build_nc(in_dim, n_dst_tiles, L_list, half_tot, n_xt_cols, groups)
    res = run_bass_kernel_spmd(nc, in_maps, core_ids=list(range(N_CORES)))

    q = np.zeros((P, HC), np.float64)
    cnt = np.zeros(P, np.float64)
    for c in range(N_CORES):
        q += res.results[c]["q_out"]
        cnt += cnts[c]
    num = q @ w2_cm.astype(np.float64)
    out = num / np.maximum(cnt, 1.0)
    const = float(fc1_b @ fc3_w[:, 0] + fc3_b[0])
    return (out + const).astype(np.float32)


# revision 40
# speedup vs baseline: 14.8705x; 1.0546x over previous
"""GAT (bipartite GATConv + mean-pool + 2 FC) on 8 Trainium2 NeuronCores.

Strategy: edges are sharded per destination node; destination nodes are
dealt round-robin (degree-sorted) across the 8 cores so the segment softmax
is fully local to a core.  Per the sharding hint each device holds its edge
shard with the source-node features replicated into matmul-ready per-edge
layout (host does only index manipulation / np.take layout; every model
FLOP runs on device):

  Phase A2: a_t = x_t @ (W att_dst) for this core's dst nodes (PE).
  Phase B: dst nodes are processed in tiles of 128 (one node per partition,
           nodes degree-sorted so tiles have uniform run lengths L).  The
           per-edge source features arrive as bf16 lhsT tiles [128, L/2*128]
           (slot pair 2j/2j+1 stacked as two K=64 halves); one K=128 matmul
           against a block-diagonal [wfold|0 / 0|wfold] rhs computes BOTH
           slots' rows [h_s (36, c-major) | a_s (3) | pad] into PSUM.  ACT
           casts them to bf16 SBUF; E = exp(leaky_relu(a_s+a_t)) =
           max(exp(z), exp(0.2 z)) via two ACT Exp passes over a DVE-added
           z.  The weighted message sum runs in bf16 on DVE's packed 2x
           path: W columns are stored c-major/h-minor so the E broadcast's
           innermost dim is the packed head dim, and the slot reduction is
           two packed tree-add levels plus a short tensor_reduce tail.
           Batch pooling is a PE matmul against host-shipped one-hot
           columns, accumulated over all tiles in PSUM; the final fc1@fc3
           contraction and count division happen on host partials.
           Pad slots carry x=0 => h=0, a_s=0; their exp(leaky_relu(a_t))
           denominator contribution is subtracted exactly via a
           host-precomputed pad-count correction.

Device-side per-edge gathers are avoided entirely: one [P,1]-offset
indirect-DMA gather costs ~1.1us of SWDGE descriptor generation on the Pool
engine (994ns fixed + 0.34ns/desc, 128 descriptors max per instruction) and
the batched-gather ucode (InstDMAGatherAnt etc.) is excluded from bedrock
images, so any gather-based design is floored at ~3.3ms/core.  Sequential
streaming of the pre-laid-out edge shard runs at DMA bandwidth instead.

HW pitfalls (probed): matmuls that switch PE row groups (partition-offset
lhsT/rhs) within one PSUM tile crash the device (the K=128 block-diagonal
formulation sidesteps row groups); Pool-engine TensorTensor is rejected by
this lowering; softmax denominators accumulated from bf16 exps lose ~6x
final accuracy (E stays f32; bf16 is cast only for the message multiply).
"""

import numpy as np
import ml_dtypes

import concourse.bacc as bacc
import concourse.tile as tile
from concourse import mybir
from concourse.bass_utils import run_bass_kernel_spmd

F32 = mybir.dt.float32
BF16 = mybir.dt.bfloat16

N_CORES = 8
P = 128
HEADS = 3
CH = 12
HC = HEADS * CH          # 36
ROW = HC + 4             # matmul output row: 36 h | 3 a_s | 1 pad = 40
ROW2 = 2 * ROW           # block-diagonal pair output
NEG_SLOPE = 0.2
GP = 6                   # slot pairs per PSUM tile (6*80 = 480 f32 <= 512)

_nc_cache = {}


def _build_nc(in_dim, n_dst_tiles, L_list, half_tot, n_xt_cols, groups):
    key = (in_dim, n_dst_tiles, tuple(L_list), half_tot, n_xt_cols, tuple(groups))
    if key in _nc_cache:
        return _nc_cache[key]

    nc = bacc.Bacc("TRN2", target_bir_lowering=False, debug=False)
    d_xe = nc.dram_tensor("xe", [2 * in_dim, half_tot * P], BF16, kind="ExternalInput")
    d_xt = nc.dram_tensor("xt_t", [in_dim, n_xt_cols], F32, kind="ExternalInput")
    d_pc = nc.dram_tensor("padc4", [P, n_dst_tiles * 4], F32, kind="ExternalInput")
    d_oh = nc.dram_tensor("oh", [P, n_dst_tiles * P], F32, kind="ExternalInput")
    d_wf = nc.dram_tensor("wfbd", [2 * in_dim, ROW2], BF16, kind="ExternalInput")
    d_wt = nc.dram_tensor("wat", [in_dim, 4], F32, kind="ExternalInput")
    d_bb = nc.dram_tensor("biasb", [P, HC], F32, kind="ExternalInput")
    d_q = nc.dram_tensor("q_out", [P, HC], F32, kind="ExternalOutput")

    with tile.TileContext(nc) as tc:
        with tc.tile_pool(name="const", bufs=1) as cpool, \
             tc.tile_pool(name="xload", bufs=2) as xpool, \
             tc.tile_pool(name="gat", bufs=3) as gpool, \
             tc.tile_pool(name="work", bufs=3) as wpool, \
             tc.tile_pool(name="msg", bufs=2) as mpool, \
             tc.tile_pool(name="psA", bufs=4, space="PSUM") as psA, \
             tc.tile_pool(name="psB", bufs=1, space="PSUM") as psB, \
             tc.tile_pool(name="psT", bufs=2, space="PSUM") as psT:

            # ---- constants into SBUF ----
            t_wf = cpool.tile([2 * in_dim, ROW2], BF16)
            nc.sync.dma_start(t_wf[:], d_wf[:])
            t_wt = cpool.tile([in_dim, 4], F32)
            nc.sync.dma_start(t_wt[:], d_wt[:])
            t_bb = cpool.tile([P, HC], F32)
            nc.sync.dma_start(t_bb[:], d_bb[:])
            t_pc = cpool.tile([P, n_dst_tiles * 4], F32)
            nc.sync.dma_start(t_pc[:], d_pc[:])

            # ---- phase B: tiles processed in groups sharing L (the few
            # high-degree tiles run solo; the rest in groups of 4).
            # Software-pipelined: group i's DMA/matmul/copy/z/exp stage (A)
            # is emitted before group i-1's softmax/message stage (B), so
            # the in-order DVE stream has group i-1's heavy message work to
            # run while ACT computes group i's exponentials. ----
            ps_q = psB.tile([P, HC], F32, space="PSUM", tag="q")
            state = {}
            off_h = [0]

            def emit_A(idx):
                t0, gs = groups[idx]
                L = L_list[t0]        # shared within group, multiple of 4
                Lh = L // 2
                GL = gs * L
                xe_sb = xpool.tile([2 * in_dim, gs * Lh * P], BF16, tag="xe")
                nc.sync.dma_start(
                    xe_sb[:], d_xe[:, off_h[0] * P:(off_h[0] + gs * Lh) * P])
                off_h[0] += gs * Lh

                # a_t for this group's tiles (interleaved A2; per-group
                # x_t slice so the first group isn't gated on a monolithic
                # x_t load)
                xt_g = wpool.tile([in_dim, gs * P], F32, tag="xt")
                nc.sync.dma_start(xt_g[:], d_xt[:, t0 * P:(t0 + gs) * P])
                t_atg = wpool.tile([P, 4 * gs], F32, tag="at")
                psa2 = psT.tile([P, 4 * gs], F32, space="PSUM", tag="psat")
                for j in range(gs):
                    nc.tensor.matmul(
                        psa2[:, j * 4:(j + 1) * 4],
                        lhsT=xt_g[:, j * P:(j + 1) * P],
                        rhs=t_wt[:], start=True, stop=True)
                nc.scalar.copy(t_atg[:], psa2[:])

                # per-edge rows via PE: one K=128 matmul per slot PAIR
                g = gpool.tile([P, GL * ROW], BF16, tag="G")
                for st in range(gs):
                    for h0 in range(0, Lh, GP):
                        nh = min(GP, Lh - h0)
                        ps = psA.tile([P, GP * ROW2], F32, space="PSUM", tag="psa")
                        for j in range(nh):
                            nc.tensor.matmul(
                                ps[:, j * ROW2:(j + 1) * ROW2],
                                lhsT=xe_sb[:, (st * Lh + h0 + j) * P:
                                           (st * Lh + h0 + j + 1) * P],
                                rhs=t_wf[:],
                                start=True, stop=True)
                        nc.scalar.copy(
                            g[:, (st * Lh + h0) * ROW2:
                              (st * Lh + h0 + nh) * ROW2],
                            ps[:, :nh * ROW2])

                g4 = g[:].rearrange("p (s l c) -> p s l c", s=gs, c=ROW)

                # z = a_s + a_t  (layout (s, l, h)); exps on ACT
                tZ = wpool.tile([P, GL * HEADS], F32, tag="Z")
                Z4 = tZ[:].rearrange("p (s l h) -> p s l h", s=gs, h=HEADS)
                at_b = (t_atg[:].rearrange("p (s h) -> p s h", h=4)
                        [:, :, 0:HEADS]
                        .unsqueeze(2).to_broadcast((P, gs, L, HEADS)))
                nc.vector.tensor_tensor(
                    out=Z4[:], in0=g4[:, :, :, HC:HC + HEADS], in1=at_b,
                    op=mybir.AluOpType.add)
                tE = wpool.tile([P, GL * HEADS], F32, tag="E")
                tT = wpool.tile([P, GL * HEADS], F32, tag="T")
                nc.scalar.activation(
                    tT[:], tZ[:], mybir.ActivationFunctionType.Exp)
                nc.scalar.activation(
                    tE[:], tZ[:], mybir.ActivationFunctionType.Exp,
                    scale=NEG_SLOPE)
                state[idx] = (t0, gs, L, GL, g, tE, tT, t_atg)

            def emit_B(idx):
                t0, gs, L, GL, g, tE, tT, t_atg = state.pop(idx)
                nc.vector.tensor_tensor(
                    out=tE[:], in0=tE[:], in1=tT[:], op=mybir.AluOpType.max)
                tEb = wpool.tile([P, GL * HEADS], BF16, tag="Eb")
                nc.scalar.copy(tEb[:], tE[:])

                # denominators + pad correction + reciprocal
                t_den = wpool.tile([P, 4 * gs], F32, tag="den")
                t_rec = wpool.tile([P, 4 * gs], F32, tag="rec")
                nc.vector.memset(t_den[:], 1.0)
                nc.vector.tensor_reduce(
                    out=t_den[:].rearrange("p (s h) -> p s h", h=4)
                    [:, :, 0:HEADS],
                    in_=tE[:].rearrange("p (s l h) -> p s l h", s=gs, h=HEADS)
                    .transpose([0, 1, 3, 2]),
                    axis=mybir.AxisListType.X, op=mybir.AluOpType.add)
                # pad correction: cor = padc * max(exp(a_t), exp(0.2 a_t))
                t_c2 = wpool.tile([P, 4 * gs], F32, tag="c2")
                t_c3 = wpool.tile([P, 4 * gs], F32, tag="c3")
                nc.vector.tensor_scalar_mul(t_c2[:], t_atg[:], NEG_SLOPE)
                nc.scalar.activation(
                    t_c3[:], t_c2[:], mybir.ActivationFunctionType.Exp)
                nc.scalar.activation(
                    t_c2[:], t_atg[:], mybir.ActivationFunctionType.Exp)
                nc.vector.tensor_tensor(
                    out=t_c2[:], in0=t_c2[:], in1=t_c3[:],
                    op=mybir.AluOpType.max)
                nc.vector.tensor_tensor(
                    out=t_c2[:], in0=t_c2[:],
                    in1=t_pc[:, t0 * 4:(t0 + gs) * 4],
                    op=mybir.AluOpType.mult)
                dv = (t_den[:].rearrange("p (s h) -> p s h", h=4)
                      [:, :, 0:HEADS])
                cview = (t_c2[:].rearrange("p (s h) -> p s h", h=4)
                         [:, :, 0:HEADS])
                nc.vector.tensor_tensor(
                    out=dv, in0=dv, in1=cview, op=mybir.AluOpType.subtract)
                nc.vector.tensor_scalar_max(t_den[:], t_den[:], 1e-30)
                nc.vector.reciprocal(t_rec[:], t_den[:])

                # weighted messages M = e * h (bf16; c-major h block),
                # one multiply for the whole group ((s,l) merged), then two
                # group-wide pairwise tree-add levels + one reduce tail
                tM = mpool.tile([P, GL * HC], BF16, tag="M")
                tU = wpool.tile([P, gs * HC], F32, tag="U")
                M4v = tM[:].rearrange("p (q c h) -> p q c h", c=CH, h=HEADS)
                e_b = (tEb[:].rearrange("p (q h) -> p q h", h=HEADS)
                       .unsqueeze(2).to_broadcast((P, GL, CH, HEADS)))
                gh = (g[:].rearrange("p (q c) -> p q c", c=ROW)[:, :, 0:HC]
                      .rearrange("p q (c h) -> p q c h", h=HEADS))
                nc.vector.tensor_tensor(
                    out=M4v[:], in0=gh, in1=e_b, op=mybir.AluOpType.mult)
                for n in (L // 2, L // 4):
                    sv = tM[:].rearrange("p (s q) -> p s q", s=gs)
                    src = (sv[:, :, :2 * n * HC]
                           .rearrange("p s (n two c) -> p s n two c",
                                      two=2, c=HC))
                    dstv = (sv[:, :, :n * HC]
                            .rearrange("p s (n c) -> p s n c", c=HC))
                    nc.vector.tensor_tensor(
                        out=dstv, in0=src[:, :, :, 0, :],
                        in1=src[:, :, :, 1, :], op=mybir.AluOpType.add)
                nc.vector.tensor_reduce(
                    out=tU[:].rearrange("p (s c) -> p s c", s=gs),
                    in_=tM[:].rearrange("p (s q) -> p s q", s=gs)
                    [:, :, :(L // 4) * HC]
                    .rearrange("p s (n c) -> p s n c", c=HC)
                    .transpose([0, 1, 3, 2]),
                    axis=mybir.AxisListType.X, op=mybir.AluOpType.add)

                # V = relu(U / denom + bias); relu on ACT
                tV = wpool.tile([P, gs * HC], F32, tag="V")
                rec_b = (t_rec[:].rearrange("p (s h) -> p s h", h=4)
                         [:, :, 0:HEADS].unsqueeze(2)
                         .to_broadcast((P, gs, CH, HEADS)))
                nc.vector.tensor_tensor(
                    out=tV[:].rearrange("p (s c h) -> p s c h", c=CH, h=HEADS),
                    in0=tU[:].rearrange("p (s c h) -> p s c h", c=CH, h=HEADS),
                    in1=rec_b, op=mybir.AluOpType.mult)
                bb_b = t_bb[:].unsqueeze(1).to_broadcast((P, gs, HC))
                nc.vector.tensor_tensor(
                    out=tV[:].rearrange("p (s c) -> p s c", s=gs),
                    in0=tV[:].rearrange("p (s c) -> p s c", s=gs),
                    in1=bb_b, op=mybir.AluOpType.add)
                nc.scalar.activation(
                    tV[:], tV[:], mybir.ActivationFunctionType.Relu)

                # pool into batches: q[b, 36] += onehot_t^T @ V, PSUM-accum
                t_oh = wpool.tile([P, gs * P], F32, tag="oh")
                nc.sync.dma_start(t_oh[:], d_oh[:, t0 * P:(t0 + gs) * P])
                for st in range(gs):
                    t = t0 + st
                    nc.tensor.matmul(
                        ps_q[:], lhsT=t_oh[:, st * P:(st + 1) * P],
                        rhs=tV[:, st * HC:(st + 1) * HC],
                        start=(t == 0), stop=(t == n_dst_tiles - 1))

            for idx in range(len(groups)):
                emit_A(idx)
                if idx > 0:
                    emit_B(idx - 1)
            emit_B(len(groups) - 1)

            t_q = cpool.tile([P, HC], F32)
            nc.vector.tensor_copy(t_q[:], ps_q[:])
            nc.sync.dma_start(d_q[:], t_q[:])
    nc.finalize()
    _nc_cache[key] = nc
    return nc


def kernel(**inputs):
    x_s = np.asarray(inputs["x_s"], np.float32)
    x_t = np.asarray(inputs["x_t"], np.float32)
    edge_index = np.asarray(inputs["edge_index"])
    x_s_batch = np.asarray(inputs["x_s_batch"]).astype(np.int64)
    W = np.asarray(inputs["W"], np.float32)
    att_src = np.asarray(inputs["att_src"], np.float32)
    att_dst = np.asarray(inputs["att_dst"], np.float32)
    bias = np.asarray(inputs["bias"], np.float32)
    fc1_w = np.asarray(inputs["fc1_w"], np.float32)
    fc1_b = np.asarray(inputs["fc1_b"], np.float32)
    fc3_w = np.asarray(inputs["fc3_w"], np.float32)
    fc3_b = np.asarray(inputs["fc3_b"], np.float32)

    n_nodes, in_dim = x_s.shape
    src = edge_index[0].astype(np.int64)
    dst = edge_index[1].astype(np.int64)

    # ---- host: edge bucketing by destination (index/layout prep only) ----
    deg = np.bincount(dst, minlength=n_nodes)
    order = np.argsort(-deg, kind="stable")          # nodes by degree desc
    inv_order = np.empty(n_nodes, np.int64)
    inv_order[order] = np.arange(n_nodes)
    nodes_per_core = (n_nodes + N_CORES - 1) // N_CORES
    n_dst_tiles = (nodes_per_core + P - 1) // P
    n_dst_tiles = (n_dst_tiles + 3) // 4 * 4         # whole groups of 4
    L_list = []
    for t in range(n_dst_tiles):
        r0 = t * P * N_CORES
        L = max(4, int(deg[order[min(r0, n_nodes - 1)]]))
        L_list.append((L + 3) // 4 * 4)              # multiple of 4
    k = 0                                            # solo tiles (big L)
    while k < n_dst_tiles and L_list[k] > 44:
        k += 1
    k = min((k + 3) // 4 * 4, n_dst_tiles)
    groups = [(t, 1) for t in range(k)]
    for g in range(k, n_dst_tiles, 4):
        Lg = max(L_list[g:g + 4])                    # shared within group
        for t in range(g, g + 4):
            L_list[t] = Lg
        groups.append((g, 4))
    groups = tuple(groups)
    off_arr = np.concatenate([[0], np.cumsum(L_list)]).astype(np.int64)
    slot_tot = int(off_arr[-1])
    half_tot = slot_tot // 2
    n_xt_cols = n_dst_tiles * P

    # edges sorted by dst -> per-node contiguous src runs
    e_order = np.argsort(dst, kind="stable")
    dst_sorted = dst[e_order]
    src_sorted = src[e_order].astype(np.int64)
    starts = np.searchsorted(dst_sorted, np.arange(n_nodes))
    slot_within = np.arange(len(dst_sorted)) - starts[dst_sorted]

    k_global = inv_order[dst_sorted]
    core_of = (k_global % N_CORES).astype(np.int64)
    k_local = k_global // N_CORES
    t_of = k_local // P
    p_of = k_local % P
    col_of = off_arr[t_of] + slot_within

    # fold weights (host weight prep).  W/bias/w2 columns permuted c-major:
    # folded col (c*HEADS + h) <- original col (h*CH + c).
    cm = np.array([h * CH + c for c in range(CH) for h in range(HEADS)])
    W_cm = W[:, cm]
    bias_cm = bias[cm]
    w2_cm = (fc1_w @ fc3_w)[:, 0].astype(np.float32)[cm]

    wa_t = np.einsum("khc,hc->kh", W.reshape(in_dim, HEADS, CH), att_dst)
    wa_s = np.einsum("khc,hc->kh", W.reshape(in_dim, HEADS, CH), att_src)
    wfold = np.zeros((in_dim, ROW), np.float32)
    wfold[:, :HC] = W_cm
    wfold[:, HC:HC + HEADS] = wa_s
    wfbd = np.zeros((2 * in_dim, ROW2), np.float32)
    wfbd[:in_dim, :ROW] = wfold
    wfbd[in_dim:, ROW:] = wfold
    wfbd = wfbd.astype(ml_dtypes.bfloat16)
    wat = np.zeros((in_dim, 4), np.float32)
    wat[:, :HEADS] = wa_t
    biasb = np.tile(bias_cm[None, :], (P, 1)).astype(np.float32)

    xsb_ext = np.zeros((n_nodes + 1, in_dim), ml_dtypes.bfloat16)
    xsb_ext[:n_nodes] = x_s.astype(ml_dtypes.bfloat16)
    SENT = n_nodes

    in_maps = []
    cnts = []
    for c in range(N_CORES):
        node_ids = order[c::N_CORES]                 # this core's dst nodes
        ncnt = len(node_ids)
        m = core_of == c
        SRC = np.full((P, slot_tot), SENT, np.int64)
        SRC[p_of[m], col_of[m]] = src_sorted[m]

        # per-edge lhsT layout: rows 0:64 even slots, 64:128 odd slots
        xe = np.empty((2 * in_dim, half_tot * P), ml_dtypes.bfloat16)
        for par in range(2):
            S = SRC[:, par::2]                       # [P, half_tot]
            blk = xsb_ext[S]                         # [P, half_tot, in_dim]
            xe[par * in_dim:(par + 1) * in_dim] = (
                blk.transpose(2, 1, 0).reshape(in_dim, half_tot * P))

        padc4 = np.zeros((P, n_dst_tiles * 4), np.float32)
        oh = np.zeros((P, n_dst_tiles * P), np.float32)
        xt_t = np.zeros((in_dim, n_xt_cols), np.float32)
        kk = np.arange(n_dst_tiles * P)
        tt, pp = kk // P, kk % P
        present = kk < ncnt
        nid = np.where(present, node_ids[np.minimum(kk, ncnt - 1)], 0)
        Leff = np.asarray(L_list, np.float32)[tt]
        pc = np.where(present, Leff - deg[nid], Leff)
        for j in range(4):
            padc4[pp, 4 * tt + j] = pc
        bid = x_s_batch[nid]
        oh[pp[present], tt[present] * P + bid[present]] = 1.0
        cnts.append(np.bincount(bid[present], minlength=P).astype(np.float64))
        xt_t[:, :ncnt] = x_t[node_ids].T
        in_maps.append({
            "xe": xe, "xt_t": xt_t, "padc4": padc4, "oh": oh, "wfbd": wfbd,
            "wat": wat, "biasb": biasb,
        })

    nc = _build_nc(in_dim, n_dst_tiles, L_list, half_tot, n_xt_cols, groups)
    res = run_bass_kernel_spmd(nc, in_maps, core_ids=list(range(N_CORES)))

    q = np.zeros((P, HC), np.float64)
    cnt = np.zeros(P, np.float64)
    for c in range(N_CORES):
        q += res.results[c]["q_out"]
        cnt += cnts[c]
    num = q @ w2_cm.astype(np.float64)
    out = num / np.maximum(cnt, 1.0)
    const = float(fc1_b @ fc3_w[:, 0] + fc3_b[0])
    return (out + const).astype(np.float32)


# revision 41
# speedup vs baseline: 14.8926x; 1.0015x over previous
"""GAT (bipartite GATConv + mean-pool + 2 FC) on 8 Trainium2 NeuronCores.

Strategy: edges are sharded per destination node; destination nodes are
dealt round-robin (degree-sorted) across the 8 cores so the segment softmax
is fully local to a core.  Per the sharding hint each device holds its edge
shard with the source-node features replicated into matmul-ready per-edge
layout (host does only index manipulation / np.take layout; every model
FLOP runs on device):

  Phase A2: a_t = x_t @ (W att_dst) for this core's dst nodes (PE).
  Phase B: dst nodes are processed in tiles of 128 (one node per partition,
           nodes degree-sorted so tiles have uniform run lengths L).  The
           per-edge source features arrive as bf16 lhsT tiles [128, L/2*128]
           (slot pair 2j/2j+1 stacked as two K=64 halves); one K=128 matmul
           against a block-diagonal [wfold|0 / 0|wfold] rhs computes BOTH
           slots' rows [h_s (36, c-major) | a_s (3) | pad] into PSUM.  ACT
           casts them to bf16 SBUF; E = exp(leaky_relu(a_s+a_t)) =
           max(exp(z), exp(0.2 z)) via two ACT Exp passes over a DVE-added
           z.  The weighted message sum runs in bf16 on DVE's packed 2x
           path: W columns are stored c-major/h-minor so the E broadcast's
           innermost dim is the packed head dim, and the slot reduction is
           two packed tree-add levels plus a short tensor_reduce tail.
           Batch pooling is a PE matmul against host-shipped one-hot
           columns, accumulated over all tiles in PSUM; the final fc1@fc3
           contraction and count division happen on host partials.
           Pad slots carry x=0 => h=0, a_s=0; their exp(leaky_relu(a_t))
           denominator contribution is subtracted exactly via a
           host-precomputed pad-count correction.

Device-side per-edge gathers are avoided entirely: one [P,1]-offset
indirect-DMA gather costs ~1.1us of SWDGE descriptor generation on the Pool
engine (994ns fixed + 0.34ns/desc, 128 descriptors max per instruction) and
the batched-gather ucode (InstDMAGatherAnt etc.) is excluded from bedrock
images, so any gather-based design is floored at ~3.3ms/core.  Sequential
streaming of the pre-laid-out edge shard runs at DMA bandwidth instead.

HW pitfalls (probed): matmuls that switch PE row groups (partition-offset
lhsT/rhs) within one PSUM tile crash the device (the K=128 block-diagonal
formulation sidesteps row groups); Pool-engine TensorTensor is rejected by
this lowering; softmax denominators accumulated from bf16 exps lose ~6x
final accuracy (E stays f32; bf16 is cast only for the message multiply).
"""

import numpy as np
import ml_dtypes

import concourse.bacc as bacc
import concourse.tile as tile
from concourse import mybir
from concourse.bass_utils import run_bass_kernel_spmd

F32 = mybir.dt.float32
BF16 = mybir.dt.bfloat16

N_CORES = 8
P = 128
HEADS = 3
CH = 12
HC = HEADS * CH          # 36
ROW = HC + 4             # matmul output row: 36 h | 3 a_s | 1 pad = 40
ROW2 = 2 * ROW           # block-diagonal pair output
NEG_SLOPE = 0.2
GP = 6                   # slot pairs per PSUM tile (6*80 = 480 f32 <= 512)

_nc_cache = {}


def _build_nc(in_dim, n_dst_tiles, L_list, half_tot, n_xt_cols, groups):
    key = (in_dim, n_dst_tiles, tuple(L_list), half_tot, n_xt_cols, tuple(groups))
    if key in _nc_cache:
        return _nc_cache[key]

    nc = bacc.Bacc("TRN2", target_bir_lowering=False, debug=False)
    d_xe = nc.dram_tensor("xe", [2 * in_dim, half_tot * P], BF16, kind="ExternalInput")
    d_xt = nc.dram_tensor("xt_t", [in_dim, n_xt_cols], F32, kind="ExternalInput")
    d_pc = nc.dram_tensor("padc4", [P, n_dst_tiles * 4], F32, kind="ExternalInput")
    d_oh = nc.dram_tensor("oh", [P, n_dst_tiles * P], F32, kind="ExternalInput")
    d_wf = nc.dram_tensor("wfbd", [2 * in_dim, ROW2], BF16, kind="ExternalInput")
    d_wt = nc.dram_tensor("wat", [in_dim, 4], F32, kind="ExternalInput")
    d_bb = nc.dram_tensor("biasb", [P, HC], F32, kind="ExternalInput")
    d_q = nc.dram_tensor("q_out", [P, HC], F32, kind="ExternalOutput")

    with tile.TileContext(nc) as tc:
        with tc.tile_pool(name="const", bufs=1) as cpool, \
             tc.tile_pool(name="xload", bufs=2) as xpool, \
             tc.tile_pool(name="gat", bufs=3) as gpool, \
             tc.tile_pool(name="work", bufs=3) as wpool, \
             tc.tile_pool(name="msg", bufs=2) as mpool, \
             tc.tile_pool(name="psA", bufs=4, space="PSUM") as psA, \
             tc.tile_pool(name="psB", bufs=1, space="PSUM") as psB, \
             tc.tile_pool(name="psT", bufs=2, space="PSUM") as psT:

            # ---- constants into SBUF ----
            t_wf = cpool.tile([2 * in_dim, ROW2], BF16)
            nc.sync.dma_start(t_wf[:], d_wf[:])
            t_wt = cpool.tile([in_dim, 4], F32)
            nc.sync.dma_start(t_wt[:], d_wt[:])
            t_bb = cpool.tile([P, HC], F32)
            nc.sync.dma_start(t_bb[:], d_bb[:])
            t_pc = cpool.tile([P, n_dst_tiles * 4], F32)
            nc.sync.dma_start(t_pc[:], d_pc[:])

            # ---- phase B: tiles processed in groups sharing L (the few
            # high-degree tiles run solo; the rest in groups of 4).
            # Software-pipelined: group i's DMA/matmul/copy/z/exp stage (A)
            # is emitted before group i-1's softmax/message stage (B), so
            # the in-order DVE stream has group i-1's heavy message work to
            # run while ACT computes group i's exponentials. ----
            ps_q = psB.tile([P, HC], F32, space="PSUM", tag="q")
            state = {}
            off_h = [0]

            def emit_A(idx):
                t0, gs = groups[idx]
                L = L_list[t0]        # shared within group, multiple of 4
                Lh = L // 2
                GL = gs * L
                xe_sb = xpool.tile([2 * in_dim, gs * Lh * P], BF16, tag="xe")
                nc.sync.dma_start(
                    xe_sb[:], d_xe[:, off_h[0] * P:(off_h[0] + gs * Lh) * P])
                off_h[0] += gs * Lh

                # a_t for this group's tiles (interleaved A2; per-group
                # x_t slice so the first group isn't gated on a monolithic
                # x_t load)
                xt_g = wpool.tile([in_dim, gs * P], F32, tag="xt")
                nc.sync.dma_start(xt_g[:], d_xt[:, t0 * P:(t0 + gs) * P])
                t_atg = wpool.tile([P, 4 * gs], F32, tag="at")
                psa2 = psT.tile([P, 4 * gs], F32, space="PSUM", tag="psat")
                for j in range(gs):
                    nc.tensor.matmul(
                        psa2[:, j * 4:(j + 1) * 4],
                        lhsT=xt_g[:, j * P:(j + 1) * P],
                        rhs=t_wt[:], start=True, stop=True)
                nc.scalar.copy(t_atg[:], psa2[:])

                # per-edge rows via PE: one K=128 matmul per slot PAIR
                g = gpool.tile([P, GL * ROW], BF16, tag="G")
                for st in range(gs):
                    for h0 in range(0, Lh, GP):
                        nh = min(GP, Lh - h0)
                        ps = psA.tile([P, GP * ROW2], F32, space="PSUM", tag="psa")
                        for j in range(nh):
                            nc.tensor.matmul(
                                ps[:, j * ROW2:(j + 1) * ROW2],
                                lhsT=xe_sb[:, (st * Lh + h0 + j) * P:
                                           (st * Lh + h0 + j + 1) * P],
                                rhs=t_wf[:],
                                start=True, stop=True)
                        nc.scalar.copy(
                            g[:, (st * Lh + h0) * ROW2:
                              (st * Lh + h0 + nh) * ROW2],
                            ps[:, :nh * ROW2])

                g4 = g[:].rearrange("p (s l c) -> p s l c", s=gs, c=ROW)

                # z = a_s + a_t  (layout (s, l, h)); exps on ACT
                tZ = wpool.tile([P, GL * HEADS], F32, tag="Z")
                Z4 = tZ[:].rearrange("p (s l h) -> p s l h", s=gs, h=HEADS)
                at_b = (t_atg[:].rearrange("p (s h) -> p s h", h=4)
                        [:, :, 0:HEADS]
                        .unsqueeze(2).to_broadcast((P, gs, L, HEADS)))
                nc.vector.tensor_tensor(
                    out=Z4[:], in0=g4[:, :, :, HC:HC + HEADS], in1=at_b,
                    op=mybir.AluOpType.add)
                tE = wpool.tile([P, GL * HEADS], F32, tag="E")
                tT = wpool.tile([P, GL * HEADS], F32, tag="T")
                nc.scalar.activation(
                    tT[:], tZ[:], mybir.ActivationFunctionType.Exp)
                nc.scalar.activation(
                    tE[:], tZ[:], mybir.ActivationFunctionType.Exp,
                    scale=NEG_SLOPE)
                state[idx] = (t0, gs, L, GL, g, tE, tT, t_atg)

            def emit_B(idx):
                t0, gs, L, GL, g, tE, tT, t_atg = state.pop(idx)
                nc.vector.tensor_tensor(
                    out=tE[:], in0=tE[:], in1=tT[:], op=mybir.AluOpType.max)
                tEb = wpool.tile([P, GL * HEADS], BF16, tag="Eb")
                nc.scalar.copy(tEb[:], tE[:])

                # denominators + pad correction + reciprocal
                t_den = wpool.tile([P, 4 * gs], F32, tag="den")
                t_rec = wpool.tile([P, 4 * gs], F32, tag="rec")
                nc.vector.memset(t_den[:], 1.0)
                nc.vector.tensor_reduce(
                    out=t_den[:].rearrange("p (s h) -> p s h", h=4)
                    [:, :, 0:HEADS],
                    in_=tE[:].rearrange("p (s l h) -> p s l h", s=gs, h=HEADS)
                    .transpose([0, 1, 3, 2]),
                    axis=mybir.AxisListType.X, op=mybir.AluOpType.add)
                # pad correction: cor = padc * max(exp(a_t), exp(0.2 a_t))
                t_c2 = wpool.tile([P, 4 * gs], F32, tag="c2")
                t_c3 = wpool.tile([P, 4 * gs], F32, tag="c3")
                nc.vector.tensor_scalar_mul(t_c2[:], t_atg[:], NEG_SLOPE)
                nc.scalar.activation(
                    t_c3[:], t_c2[:], mybir.ActivationFunctionType.Exp)
                nc.scalar.activation(
                    t_c2[:], t_atg[:], mybir.ActivationFunctionType.Exp)
                nc.vector.tensor_tensor(
                    out=t_c2[:], in0=t_c2[:], in1=t_c3[:],
                    op=mybir.AluOpType.max)
                nc.vector.tensor_tensor(
                    out=t_c2[:], in0=t_c2[:],
                    in1=t_pc[:, t0 * 4:(t0 + gs) * 4],
                    op=mybir.AluOpType.mult)
                dv = (t_den[:].rearrange("p (s h) -> p s h", h=4)
                      [:, :, 0:HEADS])
                cview = (t_c2[:].rearrange("p (s h) -> p s h", h=4)
                         [:, :, 0:HEADS])
                nc.vector.tensor_tensor(
                    out=dv, in0=dv, in1=cview, op=mybir.AluOpType.subtract)
                nc.vector.tensor_scalar_max(t_den[:], t_den[:], 1e-30)
                nc.vector.reciprocal(t_rec[:], t_den[:])

                # weighted messages M = e * h (bf16; c-major h block),
                # one multiply for the whole group ((s,l) merged), then two
                # group-wide pairwise tree-add levels + one reduce tail
                tM = mpool.tile([P, GL * HC], BF16, tag="M")
                tU = wpool.tile([P, gs * HC], F32, tag="U")
                M4v = tM[:].rearrange("p (q c h) -> p q c h", c=CH, h=HEADS)
                e_b = (tEb[:].rearrange("p (q h) -> p q h", h=HEADS)
                       .unsqueeze(2).to_broadcast((P, GL, CH, HEADS)))
                gh = (g[:].rearrange("p (q c) -> p q c", c=ROW)[:, :, 0:HC]
                      .rearrange("p q (c h) -> p q c h", h=HEADS))
                nc.vector.tensor_tensor(
                    out=M4v[:], in0=gh, in1=e_b, op=mybir.AluOpType.mult)
                for n in (L // 2, L // 4):
                    sv = tM[:].rearrange("p (s q) -> p s q", s=gs)
                    src = (sv[:, :, :2 * n * HC]
                           .rearrange("p s (n two c) -> p s n two c",
                                      two=2, c=HC))
                    dstv = (sv[:, :, :n * HC]
                            .rearrange("p s (n c) -> p s n c", c=HC))
                    nc.vector.tensor_tensor(
                        out=dstv, in0=src[:, :, :, 0, :],
                        in1=src[:, :, :, 1, :], op=mybir.AluOpType.add)
                nc.vector.tensor_reduce(
                    out=tU[:].rearrange("p (s c) -> p s c", s=gs),
                    in_=tM[:].rearrange("p (s q) -> p s q", s=gs)
                    [:, :, :(L // 4) * HC]
                    .rearrange("p s (n c) -> p s n c", c=HC)
                    .transpose([0, 1, 3, 2]),
                    axis=mybir.AxisListType.X, op=mybir.AluOpType.add)

                # V = relu(U / denom + bias); relu on ACT
                tV = wpool.tile([P, gs * HC], F32, tag="V")
                rec_b = (t_rec[:].rearrange("p (s h) -> p s h", h=4)
                         [:, :, 0:HEADS].unsqueeze(2)
                         .to_broadcast((P, gs, CH, HEADS)))
                nc.vector.tensor_tensor(
                    out=tV[:].rearrange("p (s c h) -> p s c h", c=CH, h=HEADS),
                    in0=tU[:].rearrange("p (s c h) -> p s c h", c=CH, h=HEADS),
                    in1=rec_b, op=mybir.AluOpType.mult)
                bb_b = t_bb[:].unsqueeze(1).to_broadcast((P, gs, HC))
                nc.vector.tensor_tensor(
                    out=tV[:].rearrange("p (s c) -> p s c", s=gs),
                    in0=tV[:].rearrange("p (s c) -> p s c", s=gs),
                    in1=bb_b, op=mybir.AluOpType.add)
                nc.scalar.activation(
                    tV[:], tV[:], mybir.ActivationFunctionType.Relu)

                # pool into batches: q[b, 36] += onehot_t^T @ V, PSUM-accum
                t_oh = wpool.tile([P, gs * P], F32, tag="oh")
                nc.sync.dma_start(t_oh[:], d_oh[:, t0 * P:(t0 + gs) * P])
                for st in range(gs):
                    t = t0 + st
                    nc.tensor.matmul(
                        ps_q[:], lhsT=t_oh[:, st * P:(st + 1) * P],
                        rhs=tV[:, st * HC:(st + 1) * HC],
                        start=(t == 0), stop=(t == n_dst_tiles - 1))

            for idx in range(len(groups)):
                emit_A(idx)
                if idx > 0:
                    emit_B(idx - 1)
            emit_B(len(groups) - 1)

            t_q = cpool.tile([P, HC], F32)
            nc.vector.tensor_copy(t_q[:], ps_q[:])
            nc.sync.dma_start(d_q[:], t_q[:])
    nc.finalize()
    _nc_cache[key] = nc
    return nc


def kernel(**inputs):
    x_s = np.asarray(inputs["x_s"], np.float32)
    x_t = np.asarray(inputs["x_t"], np.float32)
    edge_index = np.asarray(inputs["edge_index"])
    x_s_batch = np.asarray(inputs["x_s_batch"]).astype(np.int64)
    W = np.asarray(inputs["W"], np.float32)
    att_src = np.asarray(inputs["att_src"], np.float32)
    att_dst = np.asarray(inputs["att_dst"], np.float32)
    bias = np.asarray(inputs["bias"], np.float32)
    fc1_w = np.asarray(inputs["fc1_w"], np.float32)
    fc1_b = np.asarray(inputs["fc1_b"], np.float32)
    fc3_w = np.asarray(inputs["fc3_w"], np.float32)
    fc3_b = np.asarray(inputs["fc3_b"], np.float32)

    n_nodes, in_dim = x_s.shape
    src = edge_index[0].astype(np.int64)
    dst = edge_index[1].astype(np.int64)

    # ---- host: edge bucketing by destination (index/layout prep only) ----
    deg = np.bincount(dst, minlength=n_nodes)
    order = np.argsort(-deg, kind="stable")          # nodes by degree desc
    inv_order = np.empty(n_nodes, np.int64)
    inv_order[order] = np.arange(n_nodes)
    nodes_per_core = (n_nodes + N_CORES - 1) // N_CORES
    n_dst_tiles = (nodes_per_core + P - 1) // P
    n_dst_tiles = (n_dst_tiles + 3) // 4 * 4         # whole groups of 4
    L_list = []
    for t in range(n_dst_tiles):
        r0 = t * P * N_CORES
        L = max(4, int(deg[order[min(r0, n_nodes - 1)]]))
        L_list.append((L + 3) // 4 * 4)              # multiple of 4
    k = 0                                            # solo tiles (big L)
    while k < n_dst_tiles and L_list[k] > 44:
        k += 1
    k = min((k + 3) // 4 * 4, n_dst_tiles)
    groups = []
    for t in range(0, k, 2):                         # pair the big-L tiles
        Lg = max(L_list[t:t + 2])
        L_list[t] = L_list[t + 1] = Lg
        groups.append((t, 2))
    for g in range(k, n_dst_tiles, 4):
        Lg = max(L_list[g:g + 4])                    # shared within group
        for t in range(g, g + 4):
            L_list[t] = Lg
        groups.append((g, 4))
    groups = tuple(groups)
    off_arr = np.concatenate([[0], np.cumsum(L_list)]).astype(np.int64)
    slot_tot = int(off_arr[-1])
    half_tot = slot_tot // 2
    n_xt_cols = n_dst_tiles * P

    # edges sorted by dst -> per-node contiguous src runs
    e_order = np.argsort(dst, kind="stable")
    dst_sorted = dst[e_order]
    src_sorted = src[e_order].astype(np.int64)
    starts = np.searchsorted(dst_sorted, np.arange(n_nodes))
    slot_within = np.arange(len(dst_sorted)) - starts[dst_sorted]

    k_global = inv_order[dst_sorted]
    core_of = (k_global % N_CORES).astype(np.int64)
    k_local = k_global // N_CORES
    t_of = k_local // P
    p_of = k_local % P
    col_of = off_arr[t_of] + slot_within

    # fold weights (host weight prep).  W/bias/w2 columns permuted c-major:
    # folded col (c*HEADS + h) <- original col (h*CH + c).
    cm = np.array([h * CH + c for c in range(CH) for h in range(HEADS)])
    W_cm = W[:, cm]
    bias_cm = bias[cm]
    w2_cm = (fc1_w @ fc3_w)[:, 0].astype(np.float32)[cm]

    wa_t = np.einsum("khc,hc->kh", W.reshape(in_dim, HEADS, CH), att_dst)
    wa_s = np.einsum("khc,hc->kh", W.reshape(in_dim, HEADS, CH), att_src)
    wfold = np.zeros((in_dim, ROW), np.float32)
    wfold[:, :HC] = W_cm
    wfold[:, HC:HC + HEADS] = wa_s
    wfbd = np.zeros((2 * in_dim, ROW2), np.float32)
    wfbd[:in_dim, :ROW] = wfold
    wfbd[in_dim:, ROW:] = wfold
    wfbd = wfbd.astype(ml_dtypes.bfloat16)
    wat = np.zeros((in_dim, 4), np.float32)
    wat[:, :HEADS] = wa_t
    biasb = np.tile(bias_cm[None, :], (P, 1)).astype(np.float32)

    xsb_ext = np.zeros((n_nodes + 1, in_dim), ml_dtypes.bfloat16)
    xsb_ext[:n_nodes] = x_s.astype(ml_dtypes.bfloat16)
    SENT = n_nodes

    in_maps = []
    cnts = []
    for c in range(N_CORES):
        node_ids = order[c::N_CORES]                 # this core's dst nodes
        ncnt = len(node_ids)
        m = core_of == c
        SRC = np.full((P, slot_tot), SENT, np.int64)
        SRC[p_of[m], col_of[m]] = src_sorted[m]

        # per-edge lhsT layout: rows 0:64 even slots, 64:128 odd slots
        xe = np.empty((2 * in_dim, half_tot * P), ml_dtypes.bfloat16)
        for par in range(2):
            S = SRC[:, par::2]                       # [P, half_tot]
            blk = xsb_ext[S]                         # [P, half_tot, in_dim]
            xe[par * in_dim:(par + 1) * in_dim] = (
                blk.transpose(2, 1, 0).reshape(in_dim, half_tot * P))

        padc4 = np.zeros((P, n_dst_tiles * 4), np.float32)
        oh = np.zeros((P, n_dst_tiles * P), np.float32)
        xt_t = np.zeros((in_dim, n_xt_cols), np.float32)
        kk = np.arange(n_dst_tiles * P)
        tt, pp = kk // P, kk % P
        present = kk < ncnt
        nid = np.where(present, node_ids[np.minimum(kk, ncnt - 1)], 0)
        Leff = np.asarray(L_list, np.float32)[tt]
        pc = np.where(present, Leff - deg[nid], Leff)
        for j in range(4):
            padc4[pp, 4 * tt + j] = pc
        bid = x_s_batch[nid]
        oh[pp[present], tt[present] * P + bid[present]] = 1.0
        cnts.append(np.bincount(bid[present], minlength=P).astype(np.float64))
        xt_t[:, :ncnt] = x_t[node_ids].T
        in_maps.append({
            "xe": xe, "xt_t": xt_t, "padc4": padc4, "oh": oh, "wfbd": wfbd,
            "wat": wat, "biasb": biasb,
        })

    nc = _build_nc(in_dim, n_dst_tiles, L_list, half_tot, n_xt_cols, groups)
    res = run_bass_kernel_spmd(nc, in_maps, core_ids=list(range(N_CORES)))

    q = np.zeros((P, HC), np.float64)
    cnt = np.zeros(P, np.float64)
    for c in range(N_CORES):
        q += res.results[c]["q_out"]
        cnt += cnts[c]
    num = q @ w2_cm.astype(np.float64)
    out = num / np.maximum(cnt, 1.0)
    const = float(fc1_b @ fc3_w[:, 0] + fc3_b[0])
    return (out + const).astype(np.float32)
